# revision 17
# baseline (speedup 1.0000x reference)
"""Trainium2 Bass kernel for a dense transformer block (pre-LN, MHA + MLP).

Full inputs in, full outputs out. Sharding: 8 cores = (batch, seq-half).
Each core computes K/V over its batch element's full 1024 tokens and
Q/attention/MLP over its own 512 tokens (host permutes tokens so the core's
own half is always rows 0..511 — softmax over keys is permutation invariant).
No collectives needed.

v2: fp8 (e4m3) DoubleRow matmuls for QKV / AV / proj (and optionally
fc1/fc2), which stream 2 contraction rows per PE pass. Weights are
quantized per-output-column on the host (absmax -> +-240); dequant scales
fold into the existing bias-add / activation ops, or (for V) into the proj
weights themselves. V is computed in natural [token, feat] layout directly
(h stationary, weights moving), eliminating the separate V transpose pass.
The softmax denominator comes from a constant ones-column appended to V.

Host-side preprocessing folds LayerNorm affine params into the following
matmul weights:  (xhat*g + b) @ W == xhat @ (diag(g) W) + b @ W, and the
V bias into the proj bias: (o/d + vb) @ pw + pb == (o/d) @ pw + (vb@pw + pb).
"""

import sys

sys.path.insert(0, "/opt/trn_rl_repo")

import numpy as np

import concourse.bass as bass
import concourse.bacc as bacc
import concourse.mybir as mybir
import concourse.tile as tile
from concourse.bass_utils import run_bass_kernel_spmd
from concourse.masks import make_identity

P = 128
C = 1024
HEADS = 16
DH = 64
HID = 4096
NTOK = 1024  # tokens per batch element (kv length)
NOWN = 512  # tokens owned by this core (q length)
SCALE = DH ** -0.5
EPS = 1e-5

F32 = mybir.dt.float32
F32R = mybir.dt.float32r
BF16 = mybir.dt.bfloat16
F8 = mybir.dt.float8e4
AF = mybir.ActivationFunctionType
OP = mybir.AluOpType
DROW = mybir.MatmulPerfMode.DoubleRow

CT = C // P  # 8 column tiles of the model dim
CP = CT // 2  # 4 column-tile pairs
TT = NTOK // P  # 8 token tiles (kv)
QT = NOWN // P  # 4 token tiles (own)
HT = HID // P  # 32 hidden tiles
HP = HT // 2  # 16 hidden-tile pairs

VPAD = 128  # per-head padded width of the V tile (DH + ones col + pad);
# padded to 128 so the AV matmul streams full-height (65-row outputs
# measured ~43% slower per instruction)

E4M3_MAX = 240.0

# --- dtype config for the two MLP GEMMs (attention GEMMs are always fp8;
# the error sim shows attention fp8 contributes ~nothing to final error) ---
FC1_FP8 = False
# fc2 split-K: hidden-tile pairs [0, F2SPLIT) run fp8 DoubleRow, the rest
# bf16. Error sim: F2SPLIT=8 -> 1.31e-2 total (gate 2e-2).
F2SPLIT = 8


def build_program():
    nc = bacc.Bacc("TRN2", target_bir_lowering=False)
    mf1 = F8 if FC1_FP8 else BF16

    io = {}
    io["x"] = nc.dram_tensor("x", (NTOK, C), F32, kind="ExternalInput")
    # pre-permuted weights (host layout matches SBUF slabs)
    io["qw"] = nc.dram_tensor("qw", (P, CT, CT, P), F8, kind="ExternalInput")
    io["kw"] = nc.dram_tensor("kw", (P, CT, CT, P), F8, kind="ExternalInput")
    io["vw"] = nc.dram_tensor("vw", (P, CT, C), F8, kind="ExternalInput")
    io["pw"] = nc.dram_tensor("pw", (P, CT, C), F8, kind="ExternalInput")
    io["f1w"] = nc.dram_tensor("f1w", (P, HT, CT, P), mf1, kind="ExternalInput")
    io["f2w8"] = nc.dram_tensor(
        "f2w8", (P, 2, 2 * F2SPLIT, NOWN), F8, kind="ExternalInput"
    )
    io["f2wb"] = nc.dram_tensor(
        "f2wb", (P, 2, HT - 2 * F2SPLIT, NOWN), BF16, kind="ExternalInput"
    )
    # per-partition bias/scale tables, [128, n] layouts
    io["qbt"] = nc.dram_tensor("qbt", (P, CT), F32, kind="ExternalInput")
    io["kbt"] = nc.dram_tensor("kbt", (P, CT), F32, kind="ExternalInput")
    io["qst"] = nc.dram_tensor("qst", (P, CT), F32, kind="ExternalInput")
    io["kst"] = nc.dram_tensor("kst", (P, CT), F32, kind="ExternalInput")
    io["f1bt"] = nc.dram_tensor("f1bt", (P, HT), F32, kind="ExternalInput")
    io["f1st"] = nc.dram_tensor("f1st", (P, HT), F32, kind="ExternalInput")
    # free-dim vectors (broadcast across partitions on chip)
    io["vg"] = nc.dram_tensor("vg", (C,), F32, kind="ExternalInput")
    io["psinv"] = nc.dram_tensor("psinv", (C,), F32, kind="ExternalInput")
    io["pb"] = nc.dram_tensor("pb", (C,), F32, kind="ExternalInput")
    io["f2sinv"] = nc.dram_tensor("f2sinv", (C,), F32, kind="ExternalInput")
    io["f2b"] = nc.dram_tensor("f2b", (C,), F32, kind="ExternalInput")
    io["out"] = nc.dram_tensor("out", (NOWN, C), F32, kind="ExternalOutput")

    with tile.TileContext(nc) as tc:
        _emit(nc, tc, io)
    nc.compile()
    return nc


def _emit(nc, tc, io):
    x_d, out_d = io["x"], io["out"]
    mf1 = F8 if FC1_FP8 else BF16

    with (
        tc.tile_pool(name="consts", bufs=1) as consts,
        tc.tile_pool(name="persist", bufs=1) as persist,
        tc.tile_pool(name="big", bufs=1) as big,
        tc.tile_pool(name="psum_wide", bufs=2, space="PSUM") as psum_wide,
    ):
        # ---- constants (unique tags: each gets its own persistent slot) ----
        ident_f32 = consts.tile([P, P], F32, tag="idf")
        make_identity(nc, ident_f32)
        ident = consts.tile([P, P], F32R, tag="idr")
        nc.vector.tensor_copy(out=ident, in_=ident_f32)
        eps_tile = consts.tile([P, 1], F32, tag="eps")
        nc.vector.memset(eps_tile, EPS)
        qbT = consts.tile([P, CT], F32, tag="qbT")
        nc.sync.dma_start(qbT, io["qbt"][:, :])
        kbT = consts.tile([P, CT], F32, tag="kbT")
        nc.sync.dma_start(kbT, io["kbt"][:, :])
        qsT = consts.tile([P, CT], F32, tag="qsT")
        nc.sync.dma_start(qsT, io["qst"][:, :])
        ksT = consts.tile([P, CT], F32, tag="ksT")
        nc.sync.dma_start(ksT, io["kst"][:, :])
        f1bT = consts.tile([P, HT], F32, tag="f1bT")
        nc.sync.dma_start(f1bT, io["f1bt"][:, :])
        f1sT = consts.tile([P, HT], F32, tag="f1sT")
        nc.sync.dma_start(f1sT, io["f1st"][:, :])

        def bcast_const(src_d, n, tag):
            t = consts.tile([P, n], F32, tag=tag, name=tag)
            src = bass.AP(tensor=src_d, offset=0, ap=[[0, P], [1, n]])
            nc.sync.dma_start(t, src)
            return t

        vg_bc = bcast_const(io["vg"], C, "vg")
        psinv_bc = bcast_const(io["psinv"], C, "psv")
        pb_bc = bcast_const(io["pb"], C, "pbb")
        f2sinv_bc = bcast_const(io["f2sinv"], C, "f2s")
        f2b_bc = bcast_const(io["f2b"], C, "f2bb")

        # own x tiles (fp32, kept for the residual), one tile per token tile;
        # proj writes x2 = x + pb + proj_out back IN PLACE (saves SBUF)
        x_own = []
        for t in range(QT):
            xo = persist.tile([P, C], F32, tag=f"xo{t}", name=f"xo{t}")
            nc.sync.dma_start(xo, x_d[t * P : (t + 1) * P, :])
            x_own.append(xo)
        x2 = x_own

        # persistent weight slabs (single DMA each, reused across sweeps)
        kwslab = persist.tile([P, CT, CT, P], F8, tag="kws", name="kws")
        nc.sync.dma_start(kwslab, io["kw"][:, :, :, :])
        qwslab = persist.tile([P, CT, CT, P], F8, tag="qws", name="qws")
        nc.sync.dma_start(qwslab, io["qw"][:, :, :, :])
        vwslab = persist.tile([P, CT, C], F8, tag="vws", name="vws")
        nc.sync.dma_start(vwslab, io["vw"][:, :, :])
        pslab = persist.tile([P, CT, C], F8, tag="pws", name="pws")
        nc.sync.dma_start(pslab, io["pw"][:, :, :])

        def layernorm_tile(temps, xt):
            """xt: [128, C] fp32 -> returns normalized f32r tile [128, C]."""
            stats = temps.tile([P, 2, 6], F32, tag="ln_stats", name="st")
            for sg in range(2):
                nc.vector.bn_stats(
                    out=stats[:, sg, :], in_=xt[:, sg * 512 : (sg + 1) * 512]
                )
            mv = temps.tile([P, 2], F32, tag="ln_mv", name="mv")
            nc.vector.bn_aggr(out=mv[:], in_=stats[:])
            # rstd = exp(-0.5*ln(var+eps)); avoids the (slow) DVE reciprocal
            # and keeps Act on the exp/ln table the whole kernel
            lnv = temps.tile([P, 1], F32, tag="ln_lnv", name="lnv")
            nc.scalar.activation(
                out=lnv, in_=mv[:, 1:2], func=AF.Ln, bias=eps_tile, scale=1.0
            )
            rstd = temps.tile([P, 1], F32, tag="ln_rstd", name="rstd")
            nc.scalar.activation(out=rstd, in_=lnv, func=AF.Exp, scale=-0.5)
            nmr = temps.tile([P, 1], F32, tag="ln_nmr", name="nmr")
            nc.vector.tensor_tensor(nmr, mv[:, 0:1], rstd, OP.mult)
            nc.vector.tensor_scalar_mul(nmr, nmr, -1.0)
            h = temps.tile([P, C], F32R, tag="ln_h", name="h")
            nc.vector.tensor_scalar(
                out=h,
                in0=xt,
                scalar1=rstd,
                scalar2=nmr,
                op0=OP.mult,
                op1=OP.add,
            )
            return h

        # ---- persistent activation tiles ----
        # hT2[(cp, t2)]: [P, 2, 512] fp8 — transposed LN1 output, c-tile pairs
        hT2 = {
            (cp, t2): big.tile(
                [P, 2, NOWN], F8, tag=f"hT{cp}_{t2}", name=f"hT{cp}_{t2}"
            )
            for cp in range(CP)
            for t2 in range(2)
        }
        # kT[(ft, t2)]: [P, 512] bf16 (QK stays bf16)
        kT = {
            (ft, t2): big.tile(
                [P, NOWN], BF16, tag=f"kT{ft}_{t2}", name=f"kT{ft}_{t2}"
            )
            for ft in range(CT)
            for t2 in range(2)
        }
        qT = [
            big.tile([P, NOWN], BF16, tag=f"qT{ft}", name=f"qT{ft}")
            for ft in range(CT)
        ]
        # vh[t2]: [P, 4, HEADS, VPAD] fp8 — V in natural token layout,
        # per-head padded; col DH holds 1.0 (softmax denominator trick)
        vh = [
            big.tile([P, QT, HEADS, VPAD], F8, tag=f"vh{t2}", name=f"vh{t2}")
            for t2 in range(2)
        ]
        # oT2[fp]: [P, 2, 512] fp8 — attention output, feature-tile pairs
        oT2 = [
            big.tile([P, 2, NOWN], F8, tag=f"oT{fp}", name=f"oT{fp}")
            for fp in range(CP)
        ]
        h2T2 = [
            big.tile([P, 2, NOWN], mf1, tag=f"h2T{cp}", name=f"h2T{cp}")
            for cp in range(CP)
        ]
        actT2 = [
            big.tile(
                [P, 2, NOWN],
                F8 if hp < F2SPLIT else BF16,
                tag=f"aT{hp}",
                name=f"aT{hp}",
            )
            for hp in range(HP)
        ]

        for t2 in range(2):
            nc.vector.memset(vh[t2][:], 0.0)
            nc.vector.memset(vh[t2][:, :, :, DH : DH + 1], 1.0)

        # ================= Phase 1: LN1 -> hT2 =================
        with (
            tc.tile_pool(name="ln1", bufs=2) as ln1,
            tc.tile_pool(name="xtmp", bufs=2) as xtmp,
        ):
            for t in range(TT):
                if t < QT:
                    xt = x_own[t]
                else:
                    xt = xtmp.tile([P, C], F32, tag="xt", name="xt")
                    nc.sync.dma_start(xt, x_d[t * P : (t + 1) * P, :])
                h = layernorm_tile(ln1, xt)
                t2, tb = t // QT, t % QT
                ps = psum_wide.tile([P, C], F32R, tag="w", name=f"trp{t}")
                for ft in range(CT):
                    nc.tensor.transpose(
                        ps[:, ft * P : (ft + 1) * P],
                        h[:, ft * P : (ft + 1) * P],
                        ident,
                    )
                for cp in range(CP):
                    nc.any.tensor_copy(
                        out=hT2[(cp, t2)][:, :, tb * P : (tb + 1) * P],
                        in_=ps[:, cp * 2 * P : (cp + 1) * 2 * P].rearrange(
                            "p (two f) -> p two f", two=2
                        ),
                    )

        # ================= Phase 2: QKV =================
        HB = HEADS // 2  # heads per 512-wide V block
        with tc.tile_pool(name="qkv_psum", bufs=4, space="PSUM") as qkv_psum:

            def kq_sweep(t2, wslab, bT, sT, dst):
                """K or Q: transposed-output sweep; dst[ft] <- [P,512] bf16"""
                for ft in range(CT):
                    ps = qkv_psum.tile([P, NOWN], F32, tag="kvps", name="kvps")
                    for cp in range(CP):
                        nc.tensor.matmul(
                            ps,
                            lhsT=wslab[:, ft, 2 * cp : 2 * cp + 2, :],
                            rhs=hT2[(cp, t2)],
                            start=(cp == 0),
                            stop=(cp == CP - 1),
                            perf_mode=DROW,
                        )
                    nc.vector.tensor_scalar(
                        out=dst[ft],
                        in0=ps,
                        scalar1=sT[:, ft : ft + 1],
                        scalar2=bT[:, ft : ft + 1],
                        op0=OP.mult,
                        op1=OP.add,
                    )

            def v_sweep(t2):
                """V in natural layout: h stationary, vw moving."""
                for tb in range(QT):
                    ps = psum_wide.tile(
                        [P, HEADS, DH], F32, tag="w", name=f"vps{t2}_{tb}"
                    )
                    for cp in range(CP):
                        for blk in range(2):
                            nc.tensor.matmul(
                                ps[:, blk * HB : (blk + 1) * HB, :],
                                lhsT=hT2[(cp, t2)][
                                    :, :, tb * P : (tb + 1) * P
                                ],
                                rhs=vwslab[
                                    :,
                                    2 * cp : 2 * cp + 2,
                                    blk * 512 : (blk + 1) * 512,
                                ],
                                start=(cp == 0),
                                stop=(cp == CP - 1),
                                perf_mode=DROW,
                            )
                    nc.any.tensor_tensor(
                        vh[t2][:, tb, :, :DH],
                        ps,
                        vg_bc[:, :].rearrange("p (h d) -> p h d", h=HEADS),
                        OP.mult,
                    )

            kq_sweep(0, kwslab, kbT, ksT, [kT[(f, 0)] for f in range(CT)])
            v_sweep(0)
            kq_sweep(0, qwslab, qbT, qsT, qT)
            kq_sweep(1, kwslab, kbT, ksT, [kT[(f, 1)] for f in range(CT)])
            v_sweep(1)

        # ================= Phase 3: attention =================
        with (
            tc.tile_pool(name="attn", bufs=2) as attn_pool,
            tc.tile_pool(name="attn_ot", bufs=2, space="PSUM") as attn_ot,
        ):
            for h in range(HEADS):
                prow = (h % 2) * DH
                ftile = h // 2
                p_sb = attn_pool.tile([P, TT, NOWN], F8, tag="p_sb", name="p")
                for cp in range(CP):
                    st = psum_wide.tile(
                        [P, 2, NOWN], F32, tag="w", name=f"st{h}_{cp}"
                    )
                    for j in range(2):
                        c = cp * 2 + j
                        kv_slice = kT[(ftile, c // QT)][
                            prow : prow + DH, (c % QT) * P : (c % QT + 1) * P
                        ]
                        nc.tensor.matmul(
                            st[:, j, :],
                            lhsT=kv_slice,
                            rhs=qT[ftile][prow : prow + DH, :],
                            start=True,
                            stop=True,
                        )
                    # p = exp(SCALE * s)   (fp8 out)
                    nc.scalar.activation(
                        out=p_sb[:, 2 * cp : 2 * cp + 2, :],
                        in_=st,
                        func=AF.Exp,
                        scale=SCALE,
                    )
                ot = attn_ot.tile([P, NOWN], F32, tag="ot", name="ot")
                for cp in range(CP):
                    t2, c2 = cp // 2, cp % 2
                    nc.tensor.matmul(
                        ot,
                        lhsT=vh[t2][:, 2 * c2 : 2 * c2 + 2, h, :],
                        rhs=p_sb[:, 2 * cp : 2 * cp + 2, :],
                        start=(cp == 0),
                        stop=(cp == CP - 1),
                        perf_mode=DROW,
                    )
                # softmax denominators arrive in row DH (ones column of vh);
                # 1/d = exp(-ln(d)) on Act — a [1,512] DVE reciprocal runs
                # serially on one lane (~2.8us), the Act pair is ~0.9us
                lnd = attn_pool.tile([1, NOWN], F32, tag="lnd", name="lnd")
                nc.scalar.activation(out=lnd, in_=ot[DH : DH + 1, :], func=AF.Ln)
                rs = attn_pool.tile([1, NOWN], F32, tag="rs", name="rs")
                nc.scalar.activation(out=rs, in_=lnd, func=AF.Exp, scale=-1.0)
                rsb = attn_pool.tile([DH, NOWN], F32, tag="rsb", name="rsb")
                nc.gpsimd.partition_broadcast(rsb, rs)
                nc.any.tensor_tensor(
                    oT2[ftile // 2][prow : prow + DH, ftile % 2, :],
                    ot[:DH, :],
                    rsb,
                    OP.mult,
                )

        # ================= Phase 4: proj + residual -> x2 (in place) ========
        # fold pb into x_own first (x_own already consumed by LN1; tile deps
        # order this correctly)
        for tq in range(QT):
            nc.any.tensor_tensor(x_own[tq], x_own[tq], pb_bc, OP.add)
        with tc.tile_pool(name="proj_ps", bufs=2, space="PSUM") as proj_ps:
            for ns in range(2):
                nsl = slice(ns * 512, (ns + 1) * 512)
                for tq in range(QT):
                    ps = proj_ps.tile([P, 512], F32, tag="pps", name="pps")
                    for fp in range(CP):
                        nc.tensor.matmul(
                            ps,
                            lhsT=oT2[fp][:, :, tq * P : (tq + 1) * P],
                            rhs=pslab[:, 2 * fp : 2 * fp + 2, nsl],
                            start=(fp == 0),
                            stop=(fp == CP - 1),
                            perf_mode=DROW,
                        )
                    nc.vector.tensor_tensor(ps, ps, psinv_bc[:, nsl], OP.mult)
                    nc.vector.tensor_tensor(
                        x2[tq][:, nsl], ps, x_own[tq][:, nsl], OP.add
                    )

        # ================= Phase 5: LN2 -> h2T2 =================
        with tc.tile_pool(name="ln2", bufs=2) as ln2:
            for t in range(QT):
                h = layernorm_tile(ln2, x2[t])
                ps = psum_wide.tile([P, C], F32R, tag="w", name=f"tr2{t}")
                for ft in range(CT):
                    nc.tensor.transpose(
                        ps[:, ft * P : (ft + 1) * P],
                        h[:, ft * P : (ft + 1) * P],
                        ident,
                    )
                for cp in range(CP):
                    nc.any.tensor_copy(
                        out=h2T2[cp][:, :, t * P : (t + 1) * P],
                        in_=ps[:, cp * 2 * P : (cp + 1) * 2 * P].rearrange(
                            "p (two f) -> p two f", two=2
                        ),
                    )

        # ================= Phase 6: FC1 + gelu -> actT2 =================
        with (
            tc.tile_pool(name="f1c", bufs=8) as f1c,
            tc.tile_pool(name="f1_ps", bufs=4, space="PSUM") as f1_ps,
        ):
            for hf in range(HT):
                ps = f1_ps.tile([P, NOWN], F32, tag="f1ps", name="f1ps")
                slab = f1c.tile([P, CT, P], mf1, tag="f1w", name="f1slab")
                nc.sync.dma_start(slab, io["f1w"][:, hf])
                if FC1_FP8:
                    for cp in range(CP):
                        nc.tensor.matmul(
                            ps,
                            lhsT=slab[:, 2 * cp : 2 * cp + 2, :],
                            rhs=h2T2[cp],
                            start=(cp == 0),
                            stop=(cp == CP - 1),
                            perf_mode=DROW,
                        )
                else:
                    for c in range(CT):
                        nc.tensor.matmul(
                            ps,
                            lhsT=slab[:, c, :],
                            rhs=h2T2[c // 2][:, c % 2, :],
                            start=(c == 0),
                            stop=(c == CT - 1),
                        )
                # gelu(ps * s + b), fused dequant+bias via activation
                nc.scalar.activation(
                    out=actT2[hf // 2][:, hf % 2, :],
                    in_=ps,
                    func=AF.Gelu,
                    bias=f1bT[:, hf : hf + 1],
                    scale=f1sT[:, hf : hf + 1],
                )

        # ================= Phase 7: FC2 + residual -> out =================
        # split-K: hidden tiles [0, 2*F2SPLIT) in fp8 DoubleRow, rest bf16
        NG = 4  # hidden-tile groups per DMA chunk
        N8G = 2 * F2SPLIT // NG  # fp8 groups
        NBG = (HT - 2 * F2SPLIT) // NG  # bf16 groups
        with (
            tc.tile_pool(name="f2c", bufs=3) as f2c,
            tc.tile_pool(name="f2_ps", bufs=1, space="PSUM") as f2_ps,
            tc.tile_pool(name="out_sb", bufs=2) as out_pool,
        ):
            for ns in range(2):
                nsl = slice(ns * 512, (ns + 1) * 512)
                pss = [
                    f2_ps.tile([P, 512], F32, tag=f"f2ps{tq}", name=f"f2ps{tq}")
                    for tq in range(QT)
                ]
                for g in range(N8G):
                    gw = f2c.tile([P, NG, 512], F8, tag="f2w8", name=f"f2w8g{g}")
                    nc.sync.dma_start(
                        gw, io["f2w8"][:, ns, g * NG : (g + 1) * NG, :]
                    )
                    for tq in range(QT):
                        for i in range(NG // 2):
                            hp = (g * NG) // 2 + i
                            nc.tensor.matmul(
                                pss[tq],
                                lhsT=actT2[hp][:, :, tq * P : (tq + 1) * P],
                                rhs=gw[:, 2 * i : 2 * i + 2, :],
                                start=(g == 0 and i == 0),
                                stop=False,
                                perf_mode=DROW,
                            )
                for g in range(NBG):
                    gw = f2c.tile(
                        [P, NG, 512], BF16, tag="f2wb", name=f"f2wbg{g}"
                    )
                    nc.sync.dma_start(
                        gw, io["f2wb"][:, ns, g * NG : (g + 1) * NG, :]
                    )
                    for tq in range(QT):
                        for i in range(NG):
                            hc = 2 * F2SPLIT + g * NG + i
                            nc.tensor.matmul(
                                pss[tq],
                                lhsT=actT2[hc // 2][
                                    :, hc % 2, tq * P : (tq + 1) * P
                                ],
                                rhs=gw[:, i, :],
                                start=False,
                                stop=(g == NBG - 1 and i == NG - 1),
                            )
                for tq in range(QT):
                    ot2 = out_pool.tile([P, 512], F32, tag="out_t", name="o")
                    nc.vector.tensor_tensor(
                        ot2, pss[tq], f2sinv_bc[:, nsl], OP.mult
                    )
                    nc.vector.tensor_tensor(ot2, ot2, f2b_bc[:, nsl], OP.add)
                    nc.vector.tensor_tensor(ot2, ot2, x2[tq][:, nsl], OP.add)
                    nc.sync.dma_start(out_d[tq * P : (tq + 1) * P, nsl], ot2)


_PROGRAM = None


def _get_program():
    global _PROGRAM
    if _PROGRAM is None:
        _PROGRAM = build_program()
    return _PROGRAM


def _quant_cols(w, dtype):
    """per-output-column absmax quantization; returns (w_q, dequant_scales)"""
    import ml_dtypes

    w = np.asarray(w, np.float64)
    if dtype == "fp8":
        amax = np.abs(w).max(axis=0)
        amax = np.where(amax == 0, 1.0, amax)
        s = E4M3_MAX / amax
        wq = np.clip(w * s, -E4M3_MAX, E4M3_MAX).astype(ml_dtypes.float8_e4m3)
        return wq, (1.0 / s).astype(np.float32)
    else:
        wq = w.astype(ml_dtypes.bfloat16)
        return wq, np.ones(w.shape[1], np.float32)


def build_in_maps(inputs):
    import ml_dtypes

    x = np.asarray(inputs["x"], np.float32)  # [4, 1024, 1024]
    ln1_g = np.asarray(inputs["ln1_g"], np.float64)
    ln1_b = np.asarray(inputs["ln1_b"], np.float64)
    ln2_g = np.asarray(inputs["ln2_g"], np.float64)
    ln2_b = np.asarray(inputs["ln2_b"], np.float64)
    qkv_w = np.asarray(inputs["qkv_w"], np.float64)
    qkv_b = np.asarray(inputs["qkv_b"], np.float64)
    proj_w = np.asarray(inputs["proj_w"], np.float64)
    proj_b = np.asarray(inputs["proj_b"], np.float64)
    fc1_w = np.asarray(inputs["fc1_w"], np.float64)
    fc1_b = np.asarray(inputs["fc1_b"], np.float64)
    fc2_w = np.asarray(inputs["fc2_w"], np.float64)
    fc2_b = np.asarray(inputs["fc2_b"], np.float64)

    # Fold LN affine into the following matmul:
    #   (xhat*g + b) @ W == xhat @ (diag(g) W) + b @ W
    qkv_w_f = ln1_g[:, None] * qkv_w
    qkv_b_f = qkv_b + ln1_b @ qkv_w
    f1w_f = ln2_g[:, None] * fc1_w
    f1b_f = fc1_b + ln2_b @ fc1_w

    qw = qkv_w_f[:, :C]
    kw = qkv_w_f[:, C : 2 * C]
    vw = qkv_w_f[:, 2 * C :]
    vb = qkv_b_f[2 * C :]

    # --- Q/K: per-column fp8 quant, dequant scale applied on chip ---
    qw8, qsinv = _quant_cols(qw, "fp8")
    kw8, ksinv = _quant_cols(kw, "fp8")

    # --- V: per-column fp8 quant; on-chip the psum is rescaled by vg so the
    # fp8 V tile holds v*t with t = 24/||vw_col||; t and the v bias both fold
    # into the proj weights/bias ---
    vw8, vsinv = _quant_cols(vw, "fp8")
    vnorm = np.linalg.norm(vw, axis=0)
    vnorm = np.where(vnorm == 0, 1.0, vnorm)
    t_v = 24.0 / vnorm
    vg = (vsinv * t_v).astype(np.float32)  # psum -> fp8 V scaling

    # --- proj: fold t_v and v bias; per-column fp8 quant ---
    pw_eff = proj_w / t_v[:, None]
    pb_eff = proj_b + vb @ proj_w
    pw8, psinv = _quant_cols(pw_eff, "fp8")

    # --- fc1 ---
    f1w8, f1sinv = _quant_cols(f1w_f, "fp8" if FC1_FP8 else "bf16")

    # --- fc2 split-K: rows [0, 256*F2SPLIT) fp8 (col-scaled), rest bf16
    # pre-scaled by the same column scales so one dequant applies to both ---
    k8 = 2 * F2SPLIT * P
    amax = np.abs(fc2_w[:k8]).max(axis=0)
    amax = np.where(amax == 0, 1.0, amax)
    s2 = E4M3_MAX / amax
    f2hi = np.clip(fc2_w[:k8] * s2, -E4M3_MAX, E4M3_MAX).astype(
        ml_dtypes.float8_e4m3
    )
    f2lo = (fc2_w[k8:] * s2).astype(ml_dtypes.bfloat16)
    f2sinv = (1.0 / s2).astype(np.float32)

    # --- permute weights into SBUF slab layouts ---
    # q/k: [p, ft, c, f] from w[c*128+p, ft*128+f]
    def perm_kq(w8):
        return np.ascontiguousarray(
            w8.reshape(CT, P, CT, P).transpose(1, 2, 0, 3)
        )

    # v/proj: [p, c, n] from w[c*128+p, n]
    def perm_cn(w8):
        return np.ascontiguousarray(w8.reshape(CT, P, C).transpose(1, 0, 2))

    # fc1: [p, hf, c, f] from w[c*128+p, hf*128+f]
    f1wP = np.ascontiguousarray(
        f1w8.reshape(CT, P, HT, P).transpose(1, 2, 0, 3)
    )
    # fc2: [p, ns, hc, n] from w[hc*128+p, ns*512+n]
    f2wP8 = np.ascontiguousarray(
        f2hi.reshape(2 * F2SPLIT, P, 2, NOWN).transpose(1, 2, 0, 3)
    )
    f2wPb = np.ascontiguousarray(
        f2lo.reshape(HT - 2 * F2SPLIT, P, 2, NOWN).transpose(1, 2, 0, 3)
    )

    def tbias(b):  # [n*128] -> [128, n] per-partition layout
        return np.ascontiguousarray(
            np.asarray(b, np.float32).reshape(-1, P).T
        )

    common = dict(
        qw=perm_kq(qw8),
        kw=perm_kq(kw8),
        vw=perm_cn(vw8),
        pw=perm_cn(pw8),
        f1w=f1wP,
        f2w8=f2wP8,
        f2wb=f2wPb,
        qbt=tbias(qkv_b_f[:C]),
        kbt=tbias(qkv_b_f[C : 2 * C]),
        qst=tbias(qsinv),
        kst=tbias(ksinv),
        f1bt=tbias(f1b_f),
        f1st=tbias(f1sinv),
        vg=vg,
        psinv=psinv.astype(np.float32),
        pb=pb_eff.astype(np.float32),
        f2sinv=f2sinv.astype(np.float32),
        f2b=fc2_b.astype(np.float32),
    )
    in_maps = []
    for core in range(8):
        b, half = core // 2, core % 2
        own = x[b, half * NOWN : (half + 1) * NOWN, :]
        other = x[b, (1 - half) * NOWN : (2 - half) * NOWN, :]
        xp = np.ascontiguousarray(np.concatenate([own, other], axis=0))
        in_maps.append({**common, "x": xp})
    return in_maps


def kernel(**inputs):
    in_maps = build_in_maps(inputs)
    nc = _get_program()
    res = run_bass_kernel_spmd(nc, in_maps, core_ids=list(range(8)))
    outs = res.results

    y = np.empty((4, NTOK, C), np.float32)
    for core in range(8):
        b, half = core // 2, core % 2
        y[b, half * NOWN : (half + 1) * NOWN, :] = outs[core]["out"]
    return y


if __name__ == "__main__":
    prog = build_program()
    print("program built OK")


# revision 24
# speedup vs baseline: 1.1889x; 1.1889x over previous
"""Trainium2 Bass kernel for a dense transformer block (pre-LN, MHA + MLP).

Full inputs in, full outputs out. Sharding: 8 cores = (batch, seq-half).
Each core computes K/V over its batch element's full 1024 tokens and
Q/attention/MLP over its own 512 tokens (host permutes tokens so the core's
own half is always rows 0..511 — softmax over keys is permutation invariant).
No collectives needed.

v2: fp8 (e4m3) DoubleRow matmuls for QKV / AV / proj (and optionally
fc1/fc2), which stream 2 contraction rows per PE pass. Weights are
quantized per-output-column on the host (absmax -> +-240); dequant scales
fold into the existing bias-add / activation ops, or (for V) into the proj
weights themselves. V is computed in natural [token, feat] layout directly
(h stationary, weights moving), eliminating the separate V transpose pass.
The softmax denominator comes from a constant ones-column appended to V.

Host-side preprocessing folds LayerNorm affine params into the following
matmul weights:  (xhat*g + b) @ W == xhat @ (diag(g) W) + b @ W, and the
V bias into the proj bias: (o/d + vb) @ pw + pb == (o/d) @ pw + (vb@pw + pb).
"""

import sys

sys.path.insert(0, "/opt/trn_rl_repo")

import numpy as np

import concourse.bass as bass
import concourse.bacc as bacc
import concourse.mybir as mybir
import concourse.tile as tile
from concourse.bass_utils import run_bass_kernel_spmd
from concourse.masks import make_identity

P = 128
C = 1024
HEADS = 16
DH = 64
HID = 4096
NTOK = 1024  # tokens per batch element (kv length)
NOWN = 512  # tokens owned by this core (q length)
SCALE = DH ** -0.5
EPS = 1e-5

F32 = mybir.dt.float32
F32R = mybir.dt.float32r
BF16 = mybir.dt.bfloat16
F8 = mybir.dt.float8e4
AF = mybir.ActivationFunctionType
OP = mybir.AluOpType
DROW = mybir.MatmulPerfMode.DoubleRow

CT = C // P  # 8 column tiles of the model dim
CP = CT // 2  # 4 column-tile pairs
TT = NTOK // P  # 8 token tiles (kv)
QT = NOWN // P  # 4 token tiles (own)
HT = HID // P  # 32 hidden tiles
HP = HT // 2  # 16 hidden-tile pairs

VPAD = 128  # per-head padded width of the V tile (DH + ones col + pad);
# padded to 128 so the AV matmul streams full-height (65-row outputs
# measured ~43% slower per instruction)

E4M3_MAX = 240.0

# --- dtype config for the two MLP GEMMs (attention GEMMs are always fp8;
# the error sim shows attention fp8 contributes ~nothing to final error) ---
FC1_FP8 = False
# fc2 split-K: hidden-tile pairs [0, F2SPLIT) run fp8 DoubleRow, the rest
# bf16. Error sim: F2SPLIT=8 -> 1.31e-2 total (gate 2e-2).
F2SPLIT = 8


def build_program():
    nc = bacc.Bacc("TRN2", target_bir_lowering=False)
    mf1 = F8 if FC1_FP8 else BF16

    io = {}
    io["x"] = nc.dram_tensor("x", (NTOK, C), F32, kind="ExternalInput")
    # pre-permuted weights (host layout matches SBUF slabs)
    io["qw"] = nc.dram_tensor("qw", (P, CT, CT, P), F8, kind="ExternalInput")
    io["kw"] = nc.dram_tensor("kw", (P, CT, CT, P), F8, kind="ExternalInput")
    io["vw"] = nc.dram_tensor("vw", (P, CT, C), F8, kind="ExternalInput")
    io["pw"] = nc.dram_tensor("pw", (P, CT, C), F8, kind="ExternalInput")
    io["f1w"] = nc.dram_tensor("f1w", (P, HT, CT, P), mf1, kind="ExternalInput")
    io["f2w8"] = nc.dram_tensor(
        "f2w8", (P, 2, 2 * F2SPLIT, NOWN), F8, kind="ExternalInput"
    )
    io["f2wb"] = nc.dram_tensor(
        "f2wb", (P, 2, HT - 2 * F2SPLIT, NOWN), BF16, kind="ExternalInput"
    )
    # per-partition bias/scale tables, [128, n] layouts
    io["qbt"] = nc.dram_tensor("qbt", (P, CT), F32, kind="ExternalInput")
    io["kbt"] = nc.dram_tensor("kbt", (P, CT), F32, kind="ExternalInput")
    io["qst"] = nc.dram_tensor("qst", (P, CT), F32, kind="ExternalInput")
    io["kst"] = nc.dram_tensor("kst", (P, CT), F32, kind="ExternalInput")
    io["f1bt"] = nc.dram_tensor("f1bt", (P, HT), F32, kind="ExternalInput")
    io["f1st"] = nc.dram_tensor("f1st", (P, HT), F32, kind="ExternalInput")
    # free-dim vectors (broadcast across partitions on chip)
    io["vg"] = nc.dram_tensor("vg", (C,), F32, kind="ExternalInput")
    io["psinv"] = nc.dram_tensor("psinv", (C,), F32, kind="ExternalInput")
    io["pb"] = nc.dram_tensor("pb", (C,), F32, kind="ExternalInput")
    io["f2sinv"] = nc.dram_tensor("f2sinv", (C,), F32, kind="ExternalInput")
    io["f2b"] = nc.dram_tensor("f2b", (C,), F32, kind="ExternalInput")
    io["out"] = nc.dram_tensor("out", (NOWN, C), F32, kind="ExternalOutput")

    with tile.TileContext(nc) as tc:
        _emit(nc, tc, io)
    nc.compile()
    return nc


def _emit(nc, tc, io):
    x_d, out_d = io["x"], io["out"]
    mf1 = F8 if FC1_FP8 else BF16

    with (
        tc.tile_pool(name="consts", bufs=1) as consts,
        tc.tile_pool(name="persist", bufs=1) as persist,
        tc.tile_pool(name="big", bufs=1) as big,
        tc.tile_pool(name="psum_wide", bufs=2, space="PSUM") as psum_wide,
    ):
        # ---- constants (unique tags: each gets its own persistent slot) ----
        ident_f32 = consts.tile([P, P], F32, tag="idf")
        make_identity(nc, ident_f32)
        ident = consts.tile([P, P], BF16, tag="idr")
        nc.vector.tensor_copy(out=ident, in_=ident_f32)
        eps_tile = consts.tile([P, 1], F32, tag="eps")
        nc.vector.memset(eps_tile, EPS)
        qbT = consts.tile([P, CT], F32, tag="qbT")
        nc.sync.dma_start(qbT, io["qbt"][:, :])
        kbT = consts.tile([P, CT], F32, tag="kbT")
        nc.sync.dma_start(kbT, io["kbt"][:, :])
        qsT = consts.tile([P, CT], F32, tag="qsT")
        nc.sync.dma_start(qsT, io["qst"][:, :])
        ksT = consts.tile([P, CT], F32, tag="ksT")
        nc.sync.dma_start(ksT, io["kst"][:, :])
        f1bT = consts.tile([P, HT], F32, tag="f1bT")
        nc.sync.dma_start(f1bT, io["f1bt"][:, :])
        f1sT = consts.tile([P, HT], F32, tag="f1sT")
        nc.sync.dma_start(f1sT, io["f1st"][:, :])

        def bcast_const(src_d, n, tag):
            t = consts.tile([P, n], F32, tag=tag, name=tag)
            src = bass.AP(tensor=src_d, offset=0, ap=[[0, P], [1, n]])
            nc.sync.dma_start(t, src)
            return t

        vg_bc = bcast_const(io["vg"], C, "vg")
        psinv_bc = bcast_const(io["psinv"], C, "psv")
        pb_bc = bcast_const(io["pb"], C, "pbb")
        f2sinv_bc = bcast_const(io["f2sinv"], C, "f2s")
        f2b_bc = bcast_const(io["f2b"], C, "f2bb")

        # own x tiles (fp32, kept for the residual), one tile per token tile;
        # proj writes x2 = x + pb + proj_out back IN PLACE (saves SBUF)
        x_own = []
        for t in range(QT):
            xo = persist.tile([P, C], F32, tag=f"xo{t}", name=f"xo{t}")
            nc.sync.dma_start(xo, x_d[t * P : (t + 1) * P, :])
            x_own.append(xo)
        x2 = x_own

        # persistent weight slabs (single DMA each, reused across sweeps)
        kwslab = persist.tile([P, CT, CT, P], F8, tag="kws", name="kws")
        nc.sync.dma_start(kwslab, io["kw"][:, :, :, :])
        qwslab = persist.tile([P, CT, CT, P], F8, tag="qws", name="qws")
        nc.sync.dma_start(qwslab, io["qw"][:, :, :, :])
        vwslab = persist.tile([P, CT, C], F8, tag="vws", name="vws")
        nc.sync.dma_start(vwslab, io["vw"][:, :, :])
        pslab = persist.tile([P, CT, C], F8, tag="pws", name="pws")
        nc.sync.dma_start(pslab, io["pw"][:, :, :])

        def layernorm_tile(temps, xt):
            """xt: [128, C] fp32 -> returns normalized f32r tile [128, C]."""
            stats = temps.tile([P, 2, 6], F32, tag="ln_stats", name="st")
            for sg in range(2):
                nc.vector.bn_stats(
                    out=stats[:, sg, :], in_=xt[:, sg * 512 : (sg + 1) * 512]
                )
            mv = temps.tile([P, 2], F32, tag="ln_mv", name="mv")
            nc.vector.bn_aggr(out=mv[:], in_=stats[:])
            # (ln/exp-based rsqrt thrashes the Act tables against the
            # attention exp — 1283ns per reload; Sqrt + [P,1] DVE reciprocal
            # is cheap, the reciprocal runs 1 elem/lane on 128 lanes)
            rstd = temps.tile([P, 1], F32, tag="ln_rstd", name="rstd")
            nc.scalar.activation(
                out=rstd, in_=mv[:, 1:2], func=AF.Sqrt, bias=eps_tile, scale=1.0
            )
            nc.vector.reciprocal(out=rstd, in_=rstd)
            nmr = temps.tile([P, 1], F32, tag="ln_nmr", name="nmr")
            nc.vector.tensor_tensor(nmr, mv[:, 0:1], rstd, OP.mult)
            nc.vector.tensor_scalar_mul(nmr, nmr, -1.0)
            h = temps.tile([P, C], F32R, tag="ln_h", name="h")
            nc.vector.tensor_scalar(
                out=h,
                in0=xt,
                scalar1=rstd,
                scalar2=nmr,
                op0=OP.mult,
                op1=OP.add,
            )
            return h

        # ---- persistent activation tiles ----
        # hT2[(cp, t2)]: [P, 2, 512] fp8 — transposed LN1 output, c-tile pairs
        hT2 = {
            (cp, t2): big.tile(
                [P, 2, NOWN], F8, tag=f"hT{cp}_{t2}", name=f"hT{cp}_{t2}"
            )
            for cp in range(CP)
            for t2 in range(2)
        }
        # kT[(ft, t2)]: [P, 512] bf16 (QK stays bf16)
        kT = {
            (ft, t2): big.tile(
                [P, NOWN], BF16, tag=f"kT{ft}_{t2}", name=f"kT{ft}_{t2}"
            )
            for ft in range(CT)
            for t2 in range(2)
        }
        # qT: one zero-padded tile per head (own head's 64 rows at its
        # natural partition offset, other head's rows zero). QK can then run
        # full-height 128-contraction matmuls — measured ~35% faster than
        # the 64-row form — with kT packed as-is.
        qT = [
            big.tile([P, NOWN], BF16, tag=f"qT{hq}", name=f"qT{hq}")
            for hq in range(HEADS)
        ]
        for hq in range(HEADS):
            nc.vector.memset(qT[hq][:], 0.0)
        # vh[t2]: [P, 4, HEADS, VPAD] fp8 — V in natural token layout,
        # per-head padded; col DH holds 1.0 (softmax denominator trick)
        vh = [
            big.tile([P, QT, HEADS, VPAD], F8, tag=f"vh{t2}", name=f"vh{t2}")
            for t2 in range(2)
        ]
        # oT2[fp]: [P, 2, 512] fp8 — attention output, feature-tile pairs
        oT2 = [
            big.tile([P, 2, NOWN], F8, tag=f"oT{fp}", name=f"oT{fp}")
            for fp in range(CP)
        ]
        h2T2 = [
            big.tile([P, 2, NOWN], mf1, tag=f"h2T{cp}", name=f"h2T{cp}")
            for cp in range(CP)
        ]
        actT2 = [
            big.tile(
                [P, 2, NOWN],
                F8 if hp < F2SPLIT else BF16,
                tag=f"aT{hp}",
                name=f"aT{hp}",
            )
            for hp in range(HP)
        ]

        for t2 in range(2):
            nc.vector.memset(vh[t2][:], 0.0)
            nc.vector.memset(vh[t2][:, :, :, DH : DH + 1], 1.0)

        # ================= Phase 1: LN1 -> hT2 =================
        with (
            tc.tile_pool(name="ln1", bufs=2) as ln1,
            tc.tile_pool(name="xtmp", bufs=2) as xtmp,
        ):
            for t in range(TT):
                if t < QT:
                    xt = x_own[t]
                else:
                    xt = xtmp.tile([P, C], F32, tag="xt", name="xt")
                    nc.sync.dma_start(xt, x_d[t * P : (t + 1) * P, :])
                h = layernorm_tile(ln1, xt)
                t2, tb = t // QT, t % QT
                ps = psum_wide.tile([P, C], F32R, tag="w", name=f"trp{t}")
                for ft in range(CT):
                    nc.tensor.transpose(
                        ps[:, ft * P : (ft + 1) * P],
                        h[:, ft * P : (ft + 1) * P],
                        ident,
                    )
                for cp in range(CP):
                    nc.any.tensor_copy(
                        out=hT2[(cp, t2)][:, :, tb * P : (tb + 1) * P],
                        in_=ps[:, cp * 2 * P : (cp + 1) * 2 * P].rearrange(
                            "p (two f) -> p two f", two=2
                        ),
                    )

        # ================= Phase 2: QKV =================
        HB = HEADS // 2  # heads per 512-wide V block
        with tc.tile_pool(name="qkv_psum", bufs=4, space="PSUM") as qkv_psum:

            def kq_sweep(t2, wslab, bT, sT, dst, per_head=False):
                """K or Q: transposed-output sweep; dst[ft] <- [P,512] bf16"""
                for ft in range(CT):
                    ps = qkv_psum.tile([P, NOWN], F32, tag="kvps", name="kvps")
                    for cp in range(CP):
                        nc.tensor.matmul(
                            ps,
                            lhsT=wslab[:, ft, 2 * cp : 2 * cp + 2, :],
                            rhs=hT2[(cp, t2)],
                            start=(cp == 0),
                            stop=(cp == CP - 1),
                            perf_mode=DROW,
                        )
                    if per_head:
                        # split into the two heads' zero-padded tiles,
                        # partition-aligned (head j keeps rows j*64..)
                        for j in range(2):
                            rows = slice(j * DH, (j + 1) * DH)
                            nc.any.tensor_scalar(
                                out=dst[2 * ft + j][rows, :],
                                in0=ps[rows, :],
                                scalar1=sT[rows, ft : ft + 1],
                                scalar2=bT[rows, ft : ft + 1],
                                op0=OP.mult,
                                op1=OP.add,
                            )
                    else:
                        nc.any.tensor_scalar(
                            out=dst[ft],
                            in0=ps,
                            scalar1=sT[:, ft : ft + 1],
                            scalar2=bT[:, ft : ft + 1],
                            op0=OP.mult,
                            op1=OP.add,
                        )

            def v_sweep(t2):
                """V in natural layout: h stationary, vw moving."""
                for tb in range(QT):
                    ps = psum_wide.tile(
                        [P, HEADS, DH], F32, tag="w", name=f"vps{t2}_{tb}"
                    )
                    for cp in range(CP):
                        for blk in range(2):
                            nc.tensor.matmul(
                                ps[:, blk * HB : (blk + 1) * HB, :],
                                lhsT=hT2[(cp, t2)][
                                    :, :, tb * P : (tb + 1) * P
                                ],
                                rhs=vwslab[
                                    :,
                                    2 * cp : 2 * cp + 2,
                                    blk * 512 : (blk + 1) * 512,
                                ],
                                start=(cp == 0),
                                stop=(cp == CP - 1),
                                perf_mode=DROW,
                            )
                    nc.any.tensor_tensor(
                        vh[t2][:, tb, :, :DH],
                        ps,
                        vg_bc[:, :].rearrange("p (h d) -> p h d", h=HEADS),
                        OP.mult,
                    )

            kq_sweep(0, kwslab, kbT, ksT, [kT[(f, 0)] for f in range(CT)])
            v_sweep(0)
            kq_sweep(0, qwslab, qbT, qsT, qT, per_head=True)
            kq_sweep(1, kwslab, kbT, ksT, [kT[(f, 1)] for f in range(CT)])
            v_sweep(1)

        # ================= Phase 3: attention =================
        with (
            tc.tile_pool(name="attn", bufs=2) as attn_pool,
            tc.tile_pool(name="attn_ot", bufs=2, space="PSUM") as attn_ot,
        ):
            for h in range(HEADS):
                prow = (h % 2) * DH
                ftile = h // 2
                p_sb = attn_pool.tile([P, TT, NOWN], F8, tag="p_sb", name="p")
                for cp in range(CP):
                    st = psum_wide.tile(
                        [P, 2, NOWN], F32, tag="w", name=f"st{h}_{cp}"
                    )
                    for j in range(2):
                        c = cp * 2 + j
                        # full-height lhsT: the other head's q rows are zero
                        kv_slice = kT[(ftile, c // QT)][
                            :, (c % QT) * P : (c % QT + 1) * P
                        ]
                        nc.tensor.matmul(
                            st[:, j, :],
                            lhsT=kv_slice,
                            rhs=qT[h],
                            start=True,
                            stop=True,
                        )
                    # p = exp(SCALE * s)   (fp8 out)
                    nc.scalar.activation(
                        out=p_sb[:, 2 * cp : 2 * cp + 2, :],
                        in_=st,
                        func=AF.Exp,
                        scale=SCALE,
                    )
                ot = attn_ot.tile([P, NOWN], F32, tag="ot", name="ot")
                for cp in range(CP):
                    t2, c2 = cp // 2, cp % 2
                    nc.tensor.matmul(
                        ot,
                        lhsT=vh[t2][:, 2 * c2 : 2 * c2 + 2, h, :],
                        rhs=p_sb[:, 2 * cp : 2 * cp + 2, :],
                        start=(cp == 0),
                        stop=(cp == CP - 1),
                        perf_mode=DROW,
                    )
                # softmax denominators arrive in row DH (ones column of vh)
                rs = attn_pool.tile([1, NOWN], F32, tag="rs", name="rs")
                nc.vector.reciprocal(out=rs, in_=ot[DH : DH + 1, :])
                rsb = attn_pool.tile([DH, NOWN], F32, tag="rsb", name="rsb")
                nc.gpsimd.partition_broadcast(rsb, rs)
                nc.any.tensor_tensor(
                    oT2[ftile // 2][prow : prow + DH, ftile % 2, :],
                    ot[:DH, :],
                    rsb,
                    OP.mult,
                )

        # ================= Phase 4: proj + residual -> x2 (in place) ========
        # fold pb into x_own first (x_own already consumed by LN1; tile deps
        # order this correctly)
        for tq in range(QT):
            nc.any.tensor_tensor(x_own[tq], x_own[tq], pb_bc, OP.add)
        with tc.tile_pool(name="proj_ps", bufs=2, space="PSUM") as proj_ps:
            for ns in range(2):
                nsl = slice(ns * 512, (ns + 1) * 512)
                for tq in range(QT):
                    ps = proj_ps.tile([P, 512], F32, tag="pps", name="pps")
                    for fp in range(CP):
                        nc.tensor.matmul(
                            ps,
                            lhsT=oT2[fp][:, :, tq * P : (tq + 1) * P],
                            rhs=pslab[:, 2 * fp : 2 * fp + 2, nsl],
                            start=(fp == 0),
                            stop=(fp == CP - 1),
                            perf_mode=DROW,
                        )
                    nc.vector.tensor_tensor(ps, ps, psinv_bc[:, nsl], OP.mult)
                    nc.vector.tensor_tensor(
                        x2[tq][:, nsl], ps, x_own[tq][:, nsl], OP.add
                    )

        # ================= Phase 5: LN2 -> h2T2 =================
        with tc.tile_pool(name="ln2", bufs=2) as ln2:
            for t in range(QT):
                h = layernorm_tile(ln2, x2[t])
                ps = psum_wide.tile([P, C], F32R, tag="w", name=f"tr2{t}")
                for ft in range(CT):
                    nc.tensor.transpose(
                        ps[:, ft * P : (ft + 1) * P],
                        h[:, ft * P : (ft + 1) * P],
                        ident,
                    )
                for cp in range(CP):
                    nc.any.tensor_copy(
                        out=h2T2[cp][:, :, t * P : (t + 1) * P],
                        in_=ps[:, cp * 2 * P : (cp + 1) * 2 * P].rearrange(
                            "p (two f) -> p two f", two=2
                        ),
                    )

        # ================= Phase 6: FC1 + gelu -> actT2 =================
        with (
            tc.tile_pool(name="f1c", bufs=8) as f1c,
            tc.tile_pool(name="f1_ps", bufs=4, space="PSUM") as f1_ps,
        ):
            for hf in range(HT):
                ps = f1_ps.tile([P, NOWN], F32, tag="f1ps", name="f1ps")
                slab = f1c.tile([P, CT, P], mf1, tag="f1w", name="f1slab")
                nc.sync.dma_start(slab, io["f1w"][:, hf])
                if FC1_FP8:
                    for cp in range(CP):
                        nc.tensor.matmul(
                            ps,
                            lhsT=slab[:, 2 * cp : 2 * cp + 2, :],
                            rhs=h2T2[cp],
                            start=(cp == 0),
                            stop=(cp == CP - 1),
                            perf_mode=DROW,
                        )
                else:
                    for c in range(CT):
                        nc.tensor.matmul(
                            ps,
                            lhsT=slab[:, c, :],
                            rhs=h2T2[c // 2][:, c % 2, :],
                            start=(c == 0),
                            stop=(c == CT - 1),
                        )
                # gelu(ps * s + b), fused dequant+bias via activation
                nc.scalar.activation(
                    out=actT2[hf // 2][:, hf % 2, :],
                    in_=ps,
                    func=AF.Gelu,
                    bias=f1bT[:, hf : hf + 1],
                    scale=f1sT[:, hf : hf + 1],
                )

        # ================= Phase 7: FC2 + residual -> out =================
        # split-K: hidden tiles [0, 2*F2SPLIT) in fp8 DoubleRow, rest bf16
        NG = 4  # hidden-tile groups per DMA chunk
        N8G = 2 * F2SPLIT // NG  # fp8 groups
        NBG = (HT - 2 * F2SPLIT) // NG  # bf16 groups
        with (
            tc.tile_pool(name="f2c", bufs=3) as f2c,
            tc.tile_pool(name="f2_ps", bufs=1, space="PSUM") as f2_ps,
            tc.tile_pool(name="out_sb", bufs=2) as out_pool,
        ):
            for ns in range(2):
                nsl = slice(ns * 512, (ns + 1) * 512)
                pss = [
                    f2_ps.tile([P, 512], F32, tag=f"f2ps{tq}", name=f"f2ps{tq}")
                    for tq in range(QT)
                ]
                for g in range(N8G):
                    gw = f2c.tile([P, NG, 512], F8, tag="f2w8", name=f"f2w8g{g}")
                    nc.sync.dma_start(
                        gw, io["f2w8"][:, ns, g * NG : (g + 1) * NG, :]
                    )
                    for tq in range(QT):
                        for i in range(NG // 2):
                            hp = (g * NG) // 2 + i
                            nc.tensor.matmul(
                                pss[tq],
                                lhsT=actT2[hp][:, :, tq * P : (tq + 1) * P],
                                rhs=gw[:, 2 * i : 2 * i + 2, :],
                                start=(g == 0 and i == 0),
                                stop=False,
                                perf_mode=DROW,
                            )
                for g in range(NBG):
                    gw = f2c.tile(
                        [P, NG, 512], BF16, tag="f2wb", name=f"f2wbg{g}"
                    )
                    nc.sync.dma_start(
                        gw, io["f2wb"][:, ns, g * NG : (g + 1) * NG, :]
                    )
                    for tq in range(QT):
                        for i in range(NG):
                            hc = 2 * F2SPLIT + g * NG + i
                            nc.tensor.matmul(
                                pss[tq],
                                lhsT=actT2[hc // 2][
                                    :, hc % 2, tq * P : (tq + 1) * P
                                ],
                                rhs=gw[:, i, :],
                                start=False,
                                stop=(g == NBG - 1 and i == NG - 1),
                            )
                for tq in range(QT):
                    ot2 = out_pool.tile([P, 512], F32, tag="out_t", name="o")
                    nc.vector.tensor_tensor(
                        ot2, pss[tq], f2sinv_bc[:, nsl], OP.mult
                    )
                    nc.vector.tensor_tensor(ot2, ot2, f2b_bc[:, nsl], OP.add)
                    nc.vector.tensor_tensor(ot2, ot2, x2[tq][:, nsl], OP.add)
                    nc.sync.dma_start(out_d[tq * P : (tq + 1) * P, nsl], ot2)


_PROGRAM = None


def _get_program():
    global _PROGRAM
    if _PROGRAM is None:
        _PROGRAM = build_program()
    return _PROGRAM


def _quant_cols(w, dtype):
    """per-output-column absmax quantization; returns (w_q, dequant_scales)"""
    import ml_dtypes

    w = np.asarray(w, np.float64)
    if dtype == "fp8":
        amax = np.abs(w).max(axis=0)
        amax = np.where(amax == 0, 1.0, amax)
        s = E4M3_MAX / amax
        wq = np.clip(w * s, -E4M3_MAX, E4M3_MAX).astype(ml_dtypes.float8_e4m3)
        return wq, (1.0 / s).astype(np.float32)
    else:
        wq = w.astype(ml_dtypes.bfloat16)
        return wq, np.ones(w.shape[1], np.float32)


def build_in_maps(inputs):
    import ml_dtypes

    x = np.asarray(inputs["x"], np.float32)  # [4, 1024, 1024]
    ln1_g = np.asarray(inputs["ln1_g"], np.float64)
    ln1_b = np.asarray(inputs["ln1_b"], np.float64)
    ln2_g = np.asarray(inputs["ln2_g"], np.float64)
    ln2_b = np.asarray(inputs["ln2_b"], np.float64)
    qkv_w = np.asarray(inputs["qkv_w"], np.float64)
    qkv_b = np.asarray(inputs["qkv_b"], np.float64)
    proj_w = np.asarray(inputs["proj_w"], np.float64)
    proj_b = np.asarray(inputs["proj_b"], np.float64)
    fc1_w = np.asarray(inputs["fc1_w"], np.float64)
    fc1_b = np.asarray(inputs["fc1_b"], np.float64)
    fc2_w = np.asarray(inputs["fc2_w"], np.float64)
    fc2_b = np.asarray(inputs["fc2_b"], np.float64)

    # Fold LN affine into the following matmul:
    #   (xhat*g + b) @ W == xhat @ (diag(g) W) + b @ W
    qkv_w_f = ln1_g[:, None] * qkv_w
    qkv_b_f = qkv_b + ln1_b @ qkv_w
    f1w_f = ln2_g[:, None] * fc1_w
    f1b_f = fc1_b + ln2_b @ fc1_w

    qw = qkv_w_f[:, :C]
    kw = qkv_w_f[:, C : 2 * C]
    vw = qkv_w_f[:, 2 * C :]
    vb = qkv_b_f[2 * C :]

    # --- Q/K: per-column fp8 quant, dequant scale applied on chip ---
    qw8, qsinv = _quant_cols(qw, "fp8")
    kw8, ksinv = _quant_cols(kw, "fp8")

    # --- V: per-column fp8 quant; on-chip the psum is rescaled by vg so the
    # fp8 V tile holds v*t with t = 24/||vw_col||; t and the v bias both fold
    # into the proj weights/bias ---
    vw8, vsinv = _quant_cols(vw, "fp8")
    vnorm = np.linalg.norm(vw, axis=0)
    vnorm = np.where(vnorm == 0, 1.0, vnorm)
    t_v = 24.0 / vnorm
    vg = (vsinv * t_v).astype(np.float32)  # psum -> fp8 V scaling

    # --- proj: fold t_v and v bias; per-column fp8 quant ---
    pw_eff = proj_w / t_v[:, None]
    pb_eff = proj_b + vb @ proj_w
    pw8, psinv = _quant_cols(pw_eff, "fp8")

    # --- fc1 ---
    f1w8, f1sinv = _quant_cols(f1w_f, "fp8" if FC1_FP8 else "bf16")

    # --- fc2 split-K: rows [0, 256*F2SPLIT) fp8 (col-scaled), rest bf16
    # pre-scaled by the same column scales so one dequant applies to both ---
    k8 = 2 * F2SPLIT * P
    amax = np.abs(fc2_w[:k8]).max(axis=0)
    amax = np.where(amax == 0, 1.0, amax)
    s2 = E4M3_MAX / amax
    f2hi = np.clip(fc2_w[:k8] * s2, -E4M3_MAX, E4M3_MAX).astype(
        ml_dtypes.float8_e4m3
    )
    f2lo = (fc2_w[k8:] * s2).astype(ml_dtypes.bfloat16)
    f2sinv = (1.0 / s2).astype(np.float32)

    # --- permute weights into SBUF slab layouts ---
    # q/k: [p, ft, c, f] from w[c*128+p, ft*128+f]
    def perm_kq(w8):
        return np.ascontiguousarray(
            w8.reshape(CT, P, CT, P).transpose(1, 2, 0, 3)
        )

    # v/proj: [p, c, n] from w[c*128+p, n]
    def perm_cn(w8):
        return np.ascontiguousarray(w8.reshape(CT, P, C).transpose(1, 0, 2))

    # fc1: [p, hf, c, f] from w[c*128+p, hf*128+f]
    f1wP = np.ascontiguousarray(
        f1w8.reshape(CT, P, HT, P).transpose(1, 2, 0, 3)
    )
    # fc2: [p, ns, hc, n] from w[hc*128+p, ns*512+n]
    f2wP8 = np.ascontiguousarray(
        f2hi.reshape(2 * F2SPLIT, P, 2, NOWN).transpose(1, 2, 0, 3)
    )
    f2wPb = np.ascontiguousarray(
        f2lo.reshape(HT - 2 * F2SPLIT, P, 2, NOWN).transpose(1, 2, 0, 3)
    )

    def tbias(b):  # [n*128] -> [128, n] per-partition layout
        return np.ascontiguousarray(
            np.asarray(b, np.float32).reshape(-1, P).T
        )

    common = dict(
        qw=perm_kq(qw8),
        kw=perm_kq(kw8),
        vw=perm_cn(vw8),
        pw=perm_cn(pw8),
        f1w=f1wP,
        f2w8=f2wP8,
        f2wb=f2wPb,
        qbt=tbias(qkv_b_f[:C]),
        kbt=tbias(qkv_b_f[C : 2 * C]),
        qst=tbias(qsinv),
        kst=tbias(ksinv),
        f1bt=tbias(f1b_f),
        f1st=tbias(f1sinv),
        vg=vg,
        psinv=psinv.astype(np.float32),
        pb=pb_eff.astype(np.float32),
        f2sinv=f2sinv.astype(np.float32),
        f2b=fc2_b.astype(np.float32),
    )
    in_maps = []
    for core in range(8):
        b, half = core // 2, core % 2
        own = x[b, half * NOWN : (half + 1) * NOWN, :]
        other = x[b, (1 - half) * NOWN : (2 - half) * NOWN, :]
        xp = np.ascontiguousarray(np.concatenate([own, other], axis=0))
        in_maps.append({**common, "x": xp})
    return in_maps


def kernel(**inputs):
    in_maps = build_in_maps(inputs)
    nc = _get_program()
    res = run_bass_kernel_spmd(nc, in_maps, core_ids=list(range(8)))
    outs = res.results

    y = np.empty((4, NTOK, C), np.float32)
    for core in range(8):
        b, half = core // 2, core % 2
        y[b, half * NOWN : (half + 1) * NOWN, :] = outs[core]["out"]
    return y


if __name__ == "__main__":
    prog = build_program()
    print("program built OK")


# revision 31
# speedup vs baseline: 1.2025x; 1.0115x over previous
"""Trainium2 Bass kernel for a dense transformer block (pre-LN, MHA + MLP).

Full inputs in, full outputs out. Sharding: 8 cores = (batch, seq-half).
Each core computes K/V over its batch element's full 1024 tokens and
Q/attention/MLP over its own 512 tokens (host permutes tokens so the core's
own half is always rows 0..511 — softmax over keys is permutation invariant).
No collectives needed.

v2: fp8 (e4m3) DoubleRow matmuls for QKV / AV / proj (and optionally
fc1/fc2), which stream 2 contraction rows per PE pass. Weights are
quantized per-output-column on the host (absmax -> +-240); dequant scales
fold into the existing bias-add / activation ops, or (for V) into the proj
weights themselves. V is computed in natural [token, feat] layout directly
(h stationary, weights moving), eliminating the separate V transpose pass.
The softmax denominator comes from a constant ones-column appended to V.

Host-side preprocessing folds LayerNorm affine params into the following
matmul weights:  (xhat*g + b) @ W == xhat @ (diag(g) W) + b @ W, and the
V bias into the proj bias: (o/d + vb) @ pw + pb == (o/d) @ pw + (vb@pw + pb).
"""

import sys

sys.path.insert(0, "/opt/trn_rl_repo")

import numpy as np

import concourse.bass as bass
import concourse.bacc as bacc
import concourse.mybir as mybir
import concourse.tile as tile
from concourse.bass_utils import run_bass_kernel_spmd
from concourse.masks import make_identity

P = 128
C = 1024
HEADS = 16
DH = 64
HID = 4096
NTOK = 1024  # tokens per batch element (kv length)
NOWN = 512  # tokens owned by this core (q length)
SCALE = DH ** -0.5
EPS = 1e-5

F32 = mybir.dt.float32
F32R = mybir.dt.float32r
BF16 = mybir.dt.bfloat16
F8 = mybir.dt.float8e4
AF = mybir.ActivationFunctionType
OP = mybir.AluOpType
DROW = mybir.MatmulPerfMode.DoubleRow

CT = C // P  # 8 column tiles of the model dim
CP = CT // 2  # 4 column-tile pairs
TT = NTOK // P  # 8 token tiles (kv)
QT = NOWN // P  # 4 token tiles (own)
HT = HID // P  # 32 hidden tiles
HP = HT // 2  # 16 hidden-tile pairs

VPAD = 128  # per-head padded width of the V tile (DH + ones col + pad);
# padded to 128 so the AV matmul streams full-height (65-row outputs
# measured ~43% slower per instruction)

E4M3_MAX = 240.0

# --- dtype config for the two MLP GEMMs (attention GEMMs are always fp8;
# the error sim shows attention fp8 contributes ~nothing to final error) ---
FC1_FP8 = False
# fc2 split-K: hidden-tile pairs [0, F2SPLIT) run fp8 DoubleRow, the rest
# bf16. Error sim: F2SPLIT=8 -> 1.31e-2 total (gate 2e-2).
F2SPLIT = 8


def build_program():
    nc = bacc.Bacc("TRN2", target_bir_lowering=False)
    mf1 = F8 if FC1_FP8 else BF16

    io = {}
    io["x"] = nc.dram_tensor("x", (NTOK, C), F32, kind="ExternalInput")
    # pre-permuted weights (host layout matches SBUF slabs)
    io["qw"] = nc.dram_tensor("qw", (P, CT, CT, P), F8, kind="ExternalInput")
    io["kw"] = nc.dram_tensor("kw", (P, CT, CT, P), F8, kind="ExternalInput")
    io["vw"] = nc.dram_tensor("vw", (P, CT, C), F8, kind="ExternalInput")
    io["pw"] = nc.dram_tensor("pw", (P, CT, C), F8, kind="ExternalInput")
    io["f1w"] = nc.dram_tensor("f1w", (P, HT, CT, P), mf1, kind="ExternalInput")
    io["f2w8"] = nc.dram_tensor(
        "f2w8", (P, 2, 2 * F2SPLIT, NOWN), F8, kind="ExternalInput"
    )
    io["f2wb"] = nc.dram_tensor(
        "f2wb", (P, 2, HT - 2 * F2SPLIT, NOWN), BF16, kind="ExternalInput"
    )
    # per-partition bias/scale tables, [128, n] layouts
    io["qbt"] = nc.dram_tensor("qbt", (P, CT), F32, kind="ExternalInput")
    io["kbt"] = nc.dram_tensor("kbt", (P, CT), F32, kind="ExternalInput")
    io["qst"] = nc.dram_tensor("qst", (P, CT), F32, kind="ExternalInput")
    io["kst"] = nc.dram_tensor("kst", (P, CT), F32, kind="ExternalInput")
    io["f1bt"] = nc.dram_tensor("f1bt", (P, HT), F32, kind="ExternalInput")
    io["f1st"] = nc.dram_tensor("f1st", (P, HT), F32, kind="ExternalInput")
    # free-dim vectors (broadcast across partitions on chip)
    io["vg"] = nc.dram_tensor("vg", (C,), F32, kind="ExternalInput")
    io["psinv"] = nc.dram_tensor("psinv", (C,), F32, kind="ExternalInput")
    io["pb"] = nc.dram_tensor("pb", (C,), F32, kind="ExternalInput")
    io["f2sinv"] = nc.dram_tensor("f2sinv", (C,), F32, kind="ExternalInput")
    io["f2b"] = nc.dram_tensor("f2b", (C,), F32, kind="ExternalInput")
    io["out"] = nc.dram_tensor("out", (NOWN, C), F32, kind="ExternalOutput")

    with tile.TileContext(nc) as tc:
        _emit(nc, tc, io)
    nc.compile()
    return nc


def _emit(nc, tc, io):
    x_d, out_d = io["x"], io["out"]
    mf1 = F8 if FC1_FP8 else BF16

    with (
        tc.tile_pool(name="consts", bufs=1) as consts,
        tc.tile_pool(name="persist", bufs=1) as persist,
        tc.tile_pool(name="big", bufs=1) as big,
        tc.tile_pool(name="psum_wide", bufs=2, space="PSUM") as psum_wide,
    ):
        # ---- constants (unique tags: each gets its own persistent slot) ----
        ident_f32 = consts.tile([P, P], F32, tag="idf")
        make_identity(nc, ident_f32)
        ident = consts.tile([P, P], BF16, tag="idr")
        nc.vector.tensor_copy(out=ident, in_=ident_f32)
        eps_tile = consts.tile([P, 1], F32, tag="eps")
        nc.vector.memset(eps_tile, EPS)
        qbT = consts.tile([P, CT], F32, tag="qbT")
        nc.sync.dma_start(qbT, io["qbt"][:, :])
        kbT = consts.tile([P, CT], F32, tag="kbT")
        nc.sync.dma_start(kbT, io["kbt"][:, :])
        qsT = consts.tile([P, CT], F32, tag="qsT")
        nc.sync.dma_start(qsT, io["qst"][:, :])
        ksT = consts.tile([P, CT], F32, tag="ksT")
        nc.sync.dma_start(ksT, io["kst"][:, :])
        f1bT = consts.tile([P, HT], F32, tag="f1bT")
        nc.sync.dma_start(f1bT, io["f1bt"][:, :])
        f1sT = consts.tile([P, HT], F32, tag="f1sT")
        nc.sync.dma_start(f1sT, io["f1st"][:, :])

        def bcast_const(src_d, n, tag):
            t = consts.tile([P, n], F32, tag=tag, name=tag)
            src = bass.AP(tensor=src_d, offset=0, ap=[[0, P], [1, n]])
            nc.sync.dma_start(t, src)
            return t

        vg_bc = bcast_const(io["vg"], C, "vg")
        psinv_bc = bcast_const(io["psinv"], C, "psv")
        pb_bc = bcast_const(io["pb"], C, "pbb")
        f2sinv_bc = bcast_const(io["f2sinv"], C, "f2s")
        f2b_bc = bcast_const(io["f2b"], C, "f2bb")

        # own x tiles (fp32, kept for the residual), one tile per token tile;
        # proj writes x2 = x + pb + proj_out back IN PLACE (saves SBUF)
        x_own = []
        for t in range(QT):
            xo = persist.tile([P, C], F32, tag=f"xo{t}", name=f"xo{t}")
            nc.sync.dma_start(xo, x_d[t * P : (t + 1) * P, :])
            x_own.append(xo)
        x2 = x_own

        # persistent weight slabs (single DMA each, reused across sweeps)
        kwslab = persist.tile([P, CT, CT, P], F8, tag="kws", name="kws")
        nc.sync.dma_start(kwslab, io["kw"][:, :, :, :])
        qwslab = persist.tile([P, CT, CT, P], F8, tag="qws", name="qws")
        nc.sync.dma_start(qwslab, io["qw"][:, :, :, :])
        vwslab = persist.tile([P, CT, C], F8, tag="vws", name="vws")
        nc.sync.dma_start(vwslab, io["vw"][:, :, :])
        pslab = persist.tile([P, CT, C], F8, tag="pws", name="pws")
        nc.sync.dma_start(pslab, io["pw"][:, :, :])

        def layernorm_tile(temps, xt):
            """xt: [128, C] fp32 -> returns normalized f32r tile [128, C]."""
            stats = temps.tile([P, 2, 6], F32, tag="ln_stats", name="st")
            for sg in range(2):
                nc.vector.bn_stats(
                    out=stats[:, sg, :], in_=xt[:, sg * 512 : (sg + 1) * 512]
                )
            mv = temps.tile([P, 2], F32, tag="ln_mv", name="mv")
            nc.vector.bn_aggr(out=mv[:], in_=stats[:])
            # (ln/exp-based rsqrt thrashes the Act tables against the
            # attention exp — 1283ns per reload; Sqrt + [P,1] DVE reciprocal
            # is cheap, the reciprocal runs 1 elem/lane on 128 lanes)
            rstd = temps.tile([P, 1], F32, tag="ln_rstd", name="rstd")
            nc.scalar.activation(
                out=rstd, in_=mv[:, 1:2], func=AF.Sqrt, bias=eps_tile, scale=1.0
            )
            nc.vector.reciprocal(out=rstd, in_=rstd)
            nmr = temps.tile([P, 1], F32, tag="ln_nmr", name="nmr")
            nc.vector.tensor_tensor(nmr, mv[:, 0:1], rstd, OP.mult)
            nc.vector.tensor_scalar_mul(nmr, nmr, -1.0)
            # bf16 h: the PE transposes stream 1.0 c/row for bf16 vs 1.5 for
            # f32r, and the transpose PSUM halves to one bank
            h = temps.tile([P, C], BF16, tag="ln_h", name="h")
            nc.any.tensor_scalar(
                out=h,
                in0=xt,
                scalar1=rstd,
                scalar2=nmr,
                op0=OP.mult,
                op1=OP.add,
            )
            return h

        # ---- persistent activation tiles ----
        # hT2[(cp, t2)]: [P, 2, 512] fp8 — transposed LN1 output, c-tile pairs
        hT2 = {
            (cp, t2): big.tile(
                [P, 2, NOWN], F8, tag=f"hT{cp}_{t2}", name=f"hT{cp}_{t2}"
            )
            for cp in range(CP)
            for t2 in range(2)
        }
        # kT[(ft, t2)]: [P, 512] bf16 (QK stays bf16)
        kT = {
            (ft, t2): big.tile(
                [P, NOWN], BF16, tag=f"kT{ft}_{t2}", name=f"kT{ft}_{t2}"
            )
            for ft in range(CT)
            for t2 in range(2)
        }
        # qT: one zero-padded tile per head (own head's 64 rows at its
        # natural partition offset, other head's rows zero). QK can then run
        # full-height 128-contraction matmuls — measured ~35% faster than
        # the 64-row form — with kT packed as-is.
        qT = [
            big.tile([P, NOWN], BF16, tag=f"qT{hq}", name=f"qT{hq}")
            for hq in range(HEADS)
        ]
        for hq in range(HEADS):
            nc.vector.memset(qT[hq][:], 0.0)
        # vh[t2]: [P, 4, HEADS, VPAD] fp8 — V in natural token layout,
        # per-head padded; col DH holds 1.0 (softmax denominator trick)
        vh = [
            big.tile([P, QT, HEADS, VPAD], F8, tag=f"vh{t2}", name=f"vh{t2}")
            for t2 in range(2)
        ]
        # oT2[fp]: [P, 2, 512] fp8 — attention output, feature-tile pairs
        oT2 = [
            big.tile([P, 2, NOWN], F8, tag=f"oT{fp}", name=f"oT{fp}")
            for fp in range(CP)
        ]
        h2T2 = [
            big.tile([P, 2, NOWN], mf1, tag=f"h2T{cp}", name=f"h2T{cp}")
            for cp in range(CP)
        ]
        actT2 = [
            big.tile(
                [P, 2, NOWN],
                F8 if hp < F2SPLIT else BF16,
                tag=f"aT{hp}",
                name=f"aT{hp}",
            )
            for hp in range(HP)
        ]

        for t2 in range(2):
            nc.vector.memset(vh[t2][:], 0.0)
            nc.vector.memset(vh[t2][:, :, :, DH : DH + 1], 1.0)

        # ================= Phase 1: LN1 -> hT2 =================
        with (
            tc.tile_pool(name="ln1", bufs=2) as ln1,
            tc.tile_pool(name="xtmp", bufs=2) as xtmp,
        ):
            for t in range(TT):
                if t < QT:
                    xt = x_own[t]
                else:
                    xt = xtmp.tile([P, C], F32, tag="xt", name="xt")
                    nc.sync.dma_start(xt, x_d[t * P : (t + 1) * P, :])
                h = layernorm_tile(ln1, xt)
                t2, tb = t // QT, t % QT
                ps = psum_wide.tile([P, C], BF16, tag="w", name=f"trp{t}")
                for ft in range(CT):
                    nc.tensor.transpose(
                        ps[:, ft * P : (ft + 1) * P],
                        h[:, ft * P : (ft + 1) * P],
                        ident,
                    )
                for cp in range(CP):
                    nc.any.tensor_copy(
                        out=hT2[(cp, t2)][:, :, tb * P : (tb + 1) * P],
                        in_=ps[:, cp * 2 * P : (cp + 1) * 2 * P].rearrange(
                            "p (two f) -> p two f", two=2
                        ),
                    )

        # ================= Phase 2: QKV =================
        HB = HEADS // 2  # heads per 512-wide V block
        with tc.tile_pool(name="qkv_psum", bufs=4, space="PSUM") as qkv_psum:

            def kq_sweep(t2, wslab, bT, sT, dst, per_head=False):
                """K or Q: transposed-output sweep; dst[ft] <- [P,512] bf16"""
                for ft in range(CT):
                    ps = qkv_psum.tile([P, NOWN], F32, tag="kvps", name="kvps")
                    for cp in range(CP):
                        nc.tensor.matmul(
                            ps,
                            lhsT=wslab[:, ft, 2 * cp : 2 * cp + 2, :],
                            rhs=hT2[(cp, t2)],
                            start=(cp == 0),
                            stop=(cp == CP - 1),
                            perf_mode=DROW,
                        )
                    if per_head:
                        # split into the two heads' zero-padded tiles,
                        # partition-aligned (head j keeps rows j*64..)
                        for j in range(2):
                            rows = slice(j * DH, (j + 1) * DH)
                            nc.any.tensor_scalar(
                                out=dst[2 * ft + j][rows, :],
                                in0=ps[rows, :],
                                scalar1=sT[rows, ft : ft + 1],
                                scalar2=bT[rows, ft : ft + 1],
                                op0=OP.mult,
                                op1=OP.add,
                            )
                    else:
                        nc.any.tensor_scalar(
                            out=dst[ft],
                            in0=ps,
                            scalar1=sT[:, ft : ft + 1],
                            scalar2=bT[:, ft : ft + 1],
                            op0=OP.mult,
                            op1=OP.add,
                        )

            def v_sweep(t2):
                """V in natural layout: h stationary, vw moving."""
                for tb in range(QT):
                    ps = psum_wide.tile(
                        [P, HEADS, DH], F32, tag="w", name=f"vps{t2}_{tb}"
                    )
                    for cp in range(CP):
                        for blk in range(2):
                            nc.tensor.matmul(
                                ps[:, blk * HB : (blk + 1) * HB, :],
                                lhsT=hT2[(cp, t2)][
                                    :, :, tb * P : (tb + 1) * P
                                ],
                                rhs=vwslab[
                                    :,
                                    2 * cp : 2 * cp + 2,
                                    blk * 512 : (blk + 1) * 512,
                                ],
                                start=(cp == 0),
                                stop=(cp == CP - 1),
                                perf_mode=DROW,
                            )
                    nc.any.tensor_tensor(
                        vh[t2][:, tb, :, :DH],
                        ps,
                        vg_bc[:, :].rearrange("p (h d) -> p h d", h=HEADS),
                        OP.mult,
                    )

            kq_sweep(0, kwslab, kbT, ksT, [kT[(f, 0)] for f in range(CT)])
            v_sweep(0)
            kq_sweep(0, qwslab, qbT, qsT, qT, per_head=True)
            kq_sweep(1, kwslab, kbT, ksT, [kT[(f, 1)] for f in range(CT)])
            v_sweep(1)

        # ================= Phase 3: attention =================
        with (
            tc.tile_pool(name="attn", bufs=3) as attn_pool,
            tc.tile_pool(name="attn_ot", bufs=4, space="PSUM") as attn_ot,
        ):
            for h in range(HEADS):
                prow = (h % 2) * DH
                ftile = h // 2
                p_sb = attn_pool.tile([P, TT, NOWN], F8, tag="p_sb", name="p")
                for cp in range(CP):
                    st = psum_wide.tile(
                        [P, 2, NOWN], F32, tag="w", name=f"st{h}_{cp}"
                    )
                    for j in range(2):
                        c = cp * 2 + j
                        # full-height lhsT: the other head's q rows are zero
                        kv_slice = kT[(ftile, c // QT)][
                            :, (c % QT) * P : (c % QT + 1) * P
                        ]
                        nc.tensor.matmul(
                            st[:, j, :],
                            lhsT=kv_slice,
                            rhs=qT[h],
                            start=True,
                            stop=True,
                        )
                    # p = exp(SCALE * s)   (fp8 out)
                    nc.scalar.activation(
                        out=p_sb[:, 2 * cp : 2 * cp + 2, :],
                        in_=st,
                        func=AF.Exp,
                        scale=SCALE,
                    )
                ot = attn_ot.tile([P, NOWN], F32, tag="ot", name="ot")
                for cp in range(CP):
                    t2, c2 = cp // 2, cp % 2
                    nc.tensor.matmul(
                        ot,
                        lhsT=vh[t2][:, 2 * c2 : 2 * c2 + 2, h, :],
                        rhs=p_sb[:, 2 * cp : 2 * cp + 2, :],
                        start=(cp == 0),
                        stop=(cp == CP - 1),
                        perf_mode=DROW,
                    )
                # softmax denominators arrive in row DH (ones column of vh)
                rs = attn_pool.tile([1, NOWN], F32, tag="rs", name="rs")
                nc.vector.reciprocal(out=rs, in_=ot[DH : DH + 1, :])
                rsb = attn_pool.tile([DH, NOWN], F32, tag="rsb", name="rsb")
                nc.gpsimd.partition_broadcast(rsb, rs)
                nc.any.tensor_tensor(
                    oT2[ftile // 2][prow : prow + DH, ftile % 2, :],
                    ot[:DH, :],
                    rsb,
                    OP.mult,
                )

        # ================= Phase 4: proj + residual -> x2 (in place) ========
        # fold pb into x_own first (x_own already consumed by LN1; tile deps
        # order this correctly)
        for tq in range(QT):
            nc.any.tensor_tensor(x_own[tq], x_own[tq], pb_bc, OP.add)
        # tq-outer so x2[tq] completes both halves early -> LN2 tile tq can
        # start while proj continues on later tq
        with tc.tile_pool(name="proj_ps", bufs=4, space="PSUM") as proj_ps:
            for tq in range(QT):
                for ns in range(2):
                    nsl = slice(ns * 512, (ns + 1) * 512)
                    ps = proj_ps.tile([P, 512], F32, tag="pps", name="pps")
                    for fp in range(CP):
                        nc.tensor.matmul(
                            ps,
                            lhsT=oT2[fp][:, :, tq * P : (tq + 1) * P],
                            rhs=pslab[:, 2 * fp : 2 * fp + 2, nsl],
                            start=(fp == 0),
                            stop=(fp == CP - 1),
                            perf_mode=DROW,
                        )
                    nc.vector.tensor_tensor(ps, ps, psinv_bc[:, nsl], OP.mult)
                    nc.vector.tensor_tensor(
                        x2[tq][:, nsl], ps, x_own[tq][:, nsl], OP.add
                    )

        # ================= Phase 5: LN2 -> h2T2 =================
        with tc.tile_pool(name="ln2", bufs=2) as ln2:
            for t in range(QT):
                h = layernorm_tile(ln2, x2[t])
                ps = psum_wide.tile([P, C], BF16, tag="w", name=f"tr2{t}")
                for ft in range(CT):
                    nc.tensor.transpose(
                        ps[:, ft * P : (ft + 1) * P],
                        h[:, ft * P : (ft + 1) * P],
                        ident,
                    )
                for cp in range(CP):
                    nc.any.tensor_copy(
                        out=h2T2[cp][:, :, t * P : (t + 1) * P],
                        in_=ps[:, cp * 2 * P : (cp + 1) * 2 * P].rearrange(
                            "p (two f) -> p two f", two=2
                        ),
                    )

        # fold the fc2 bias into the residual once LN2 has consumed x2
        # (shortens the fc2 output chain to mult+add)
        for tq in range(QT):
            nc.any.tensor_tensor(x2[tq], x2[tq], f2b_bc, OP.add)

        # ================= Phase 6: FC1 + gelu -> actT2 =================
        with (
            tc.tile_pool(name="f1c", bufs=8) as f1c,
            tc.tile_pool(name="f1_ps", bufs=4, space="PSUM") as f1_ps,
        ):
            for hf in range(HT):
                ps = f1_ps.tile([P, NOWN], F32, tag="f1ps", name="f1ps")
                slab = f1c.tile([P, CT, P], mf1, tag="f1w", name="f1slab")
                nc.sync.dma_start(slab, io["f1w"][:, hf])
                if FC1_FP8:
                    for cp in range(CP):
                        nc.tensor.matmul(
                            ps,
                            lhsT=slab[:, 2 * cp : 2 * cp + 2, :],
                            rhs=h2T2[cp],
                            start=(cp == 0),
                            stop=(cp == CP - 1),
                            perf_mode=DROW,
                        )
                else:
                    for c in range(CT):
                        nc.tensor.matmul(
                            ps,
                            lhsT=slab[:, c, :],
                            rhs=h2T2[c // 2][:, c % 2, :],
                            start=(c == 0),
                            stop=(c == CT - 1),
                        )
                # gelu(ps * s + b), fused dequant+bias via activation
                nc.scalar.activation(
                    out=actT2[hf // 2][:, hf % 2, :],
                    in_=ps,
                    func=AF.Gelu,
                    bias=f1bT[:, hf : hf + 1],
                    scale=f1sT[:, hf : hf + 1],
                )

        # ================= Phase 7: FC2 + residual -> out =================
        # split-K: hidden tiles [0, 2*F2SPLIT) in fp8 DoubleRow, rest bf16
        NG = 4  # hidden-tile groups per DMA chunk
        N8G = 2 * F2SPLIT // NG  # fp8 groups
        NBG = (HT - 2 * F2SPLIT) // NG  # bf16 groups
        with (
            tc.tile_pool(name="f2c", bufs=3) as f2c,
            tc.tile_pool(name="f2_ps", bufs=1, space="PSUM") as f2_ps,
            tc.tile_pool(name="out_sb", bufs=2) as out_pool,
        ):
            for ns in range(2):
                nsl = slice(ns * 512, (ns + 1) * 512)
                pss = [
                    f2_ps.tile([P, 512], F32, tag=f"f2ps{tq}", name=f"f2ps{tq}")
                    for tq in range(QT)
                ]
                for g in range(N8G):
                    gw = f2c.tile([P, NG, 512], F8, tag="f2w8", name=f"f2w8g{g}")
                    nc.sync.dma_start(
                        gw, io["f2w8"][:, ns, g * NG : (g + 1) * NG, :]
                    )
                    for tq in range(QT):
                        for i in range(NG // 2):
                            hp = (g * NG) // 2 + i
                            nc.tensor.matmul(
                                pss[tq],
                                lhsT=actT2[hp][:, :, tq * P : (tq + 1) * P],
                                rhs=gw[:, 2 * i : 2 * i + 2, :],
                                start=(g == 0 and i == 0),
                                stop=False,
                                perf_mode=DROW,
                            )
                for g in range(NBG):
                    gw = f2c.tile(
                        [P, NG, 512], BF16, tag="f2wb", name=f"f2wbg{g}"
                    )
                    nc.sync.dma_start(
                        gw, io["f2wb"][:, ns, g * NG : (g + 1) * NG, :]
                    )
                    for tq in range(QT):
                        for i in range(NG):
                            hc = 2 * F2SPLIT + g * NG + i
                            nc.tensor.matmul(
                                pss[tq],
                                lhsT=actT2[hc // 2][
                                    :, hc % 2, tq * P : (tq + 1) * P
                                ],
                                rhs=gw[:, i, :],
                                start=False,
                                stop=(g == NBG - 1 and i == NG - 1),
                            )
                for tq in range(QT):
                    ot2 = out_pool.tile([P, 512], F32, tag="out_t", name="o")
                    nc.vector.tensor_tensor(
                        ot2, pss[tq], f2sinv_bc[:, nsl], OP.mult
                    )
                    nc.vector.tensor_tensor(ot2, ot2, x2[tq][:, nsl], OP.add)
                    nc.sync.dma_start(out_d[tq * P : (tq + 1) * P, nsl], ot2)


_PROGRAM = None


def _get_program():
    global _PROGRAM
    if _PROGRAM is None:
        _PROGRAM = build_program()
    return _PROGRAM


def _quant_cols(w, dtype):
    """per-output-column absmax quantization; returns (w_q, dequant_scales)"""
    import ml_dtypes

    w = np.asarray(w, np.float64)
    if dtype == "fp8":
        amax = np.abs(w).max(axis=0)
        amax = np.where(amax == 0, 1.0, amax)
        s = E4M3_MAX / amax
        wq = np.clip(w * s, -E4M3_MAX, E4M3_MAX).astype(ml_dtypes.float8_e4m3)
        return wq, (1.0 / s).astype(np.float32)
    else:
        wq = w.astype(ml_dtypes.bfloat16)
        return wq, np.ones(w.shape[1], np.float32)


def build_in_maps(inputs):
    import ml_dtypes

    x = np.asarray(inputs["x"], np.float32)  # [4, 1024, 1024]
    ln1_g = np.asarray(inputs["ln1_g"], np.float64)
    ln1_b = np.asarray(inputs["ln1_b"], np.float64)
    ln2_g = np.asarray(inputs["ln2_g"], np.float64)
    ln2_b = np.asarray(inputs["ln2_b"], np.float64)
    qkv_w = np.asarray(inputs["qkv_w"], np.float64)
    qkv_b = np.asarray(inputs["qkv_b"], np.float64)
    proj_w = np.asarray(inputs["proj_w"], np.float64)
    proj_b = np.asarray(inputs["proj_b"], np.float64)
    fc1_w = np.asarray(inputs["fc1_w"], np.float64)
    fc1_b = np.asarray(inputs["fc1_b"], np.float64)
    fc2_w = np.asarray(inputs["fc2_w"], np.float64)
    fc2_b = np.asarray(inputs["fc2_b"], np.float64)

    # Fold LN affine into the following matmul:
    #   (xhat*g + b) @ W == xhat @ (diag(g) W) + b @ W
    qkv_w_f = ln1_g[:, None] * qkv_w
    qkv_b_f = qkv_b + ln1_b @ qkv_w
    f1w_f = ln2_g[:, None] * fc1_w
    f1b_f = fc1_b + ln2_b @ fc1_w

    qw = qkv_w_f[:, :C]
    kw = qkv_w_f[:, C : 2 * C]
    vw = qkv_w_f[:, 2 * C :]
    vb = qkv_b_f[2 * C :]

    # --- Q/K: per-column fp8 quant, dequant scale applied on chip ---
    qw8, qsinv = _quant_cols(qw, "fp8")
    kw8, ksinv = _quant_cols(kw, "fp8")

    # --- V: per-column fp8 quant; on-chip the psum is rescaled by vg so the
    # fp8 V tile holds v*t with t = 24/||vw_col||; t and the v bias both fold
    # into the proj weights/bias ---
    vw8, vsinv = _quant_cols(vw, "fp8")
    vnorm = np.linalg.norm(vw, axis=0)
    vnorm = np.where(vnorm == 0, 1.0, vnorm)
    t_v = 24.0 / vnorm
    vg = (vsinv * t_v).astype(np.float32)  # psum -> fp8 V scaling

    # --- proj: fold t_v and v bias; per-column fp8 quant ---
    pw_eff = proj_w / t_v[:, None]
    pb_eff = proj_b + vb @ proj_w
    pw8, psinv = _quant_cols(pw_eff, "fp8")

    # --- fc1 ---
    f1w8, f1sinv = _quant_cols(f1w_f, "fp8" if FC1_FP8 else "bf16")

    # --- fc2 split-K: rows [0, 256*F2SPLIT) fp8 (col-scaled), rest bf16
    # pre-scaled by the same column scales so one dequant applies to both ---
    k8 = 2 * F2SPLIT * P
    amax = np.abs(fc2_w[:k8]).max(axis=0)
    amax = np.where(amax == 0, 1.0, amax)
    s2 = E4M3_MAX / amax
    f2hi = np.clip(fc2_w[:k8] * s2, -E4M3_MAX, E4M3_MAX).astype(
        ml_dtypes.float8_e4m3
    )
    f2lo = (fc2_w[k8:] * s2).astype(ml_dtypes.bfloat16)
    f2sinv = (1.0 / s2).astype(np.float32)

    # --- permute weights into SBUF slab layouts ---
    # q/k: [p, ft, c, f] from w[c*128+p, ft*128+f]
    def perm_kq(w8):
        return np.ascontiguousarray(
            w8.reshape(CT, P, CT, P).transpose(1, 2, 0, 3)
        )

    # v/proj: [p, c, n] from w[c*128+p, n]
    def perm_cn(w8):
        return np.ascontiguousarray(w8.reshape(CT, P, C).transpose(1, 0, 2))

    # fc1: [p, hf, c, f] from w[c*128+p, hf*128+f]
    f1wP = np.ascontiguousarray(
        f1w8.reshape(CT, P, HT, P).transpose(1, 2, 0, 3)
    )
    # fc2: [p, ns, hc, n] from w[hc*128+p, ns*512+n]
    f2wP8 = np.ascontiguousarray(
        f2hi.reshape(2 * F2SPLIT, P, 2, NOWN).transpose(1, 2, 0, 3)
    )
    f2wPb = np.ascontiguousarray(
        f2lo.reshape(HT - 2 * F2SPLIT, P, 2, NOWN).transpose(1, 2, 0, 3)
    )

    def tbias(b):  # [n*128] -> [128, n] per-partition layout
        return np.ascontiguousarray(
            np.asarray(b, np.float32).reshape(-1, P).T
        )

    common = dict(
        qw=perm_kq(qw8),
        kw=perm_kq(kw8),
        vw=perm_cn(vw8),
        pw=perm_cn(pw8),
        f1w=f1wP,
        f2w8=f2wP8,
        f2wb=f2wPb,
        qbt=tbias(qkv_b_f[:C]),
        kbt=tbias(qkv_b_f[C : 2 * C]),
        qst=tbias(qsinv),
        kst=tbias(ksinv),
        f1bt=tbias(f1b_f),
        f1st=tbias(f1sinv),
        vg=vg,
        psinv=psinv.astype(np.float32),
        pb=pb_eff.astype(np.float32),
        f2sinv=f2sinv.astype(np.float32),
        f2b=fc2_b.astype(np.float32),
    )
    in_maps = []
    for core in range(8):
        b, half = core // 2, core % 2
        own = x[b, half * NOWN : (half + 1) * NOWN, :]
        other = x[b, (1 - half) * NOWN : (2 - half) * NOWN, :]
        xp = np.ascontiguousarray(np.concatenate([own, other], axis=0))
        in_maps.append({**common, "x": xp})
    return in_maps


def kernel(**inputs):
    in_maps = build_in_maps(inputs)
    nc = _get_program()
    res = run_bass_kernel_spmd(nc, in_maps, core_ids=list(range(8)))
    outs = res.results

    y = np.empty((4, NTOK, C), np.float32)
    for core in range(8):
        b, half = core // 2, core % 2
        y[b, half * NOWN : (half + 1) * NOWN, :] = outs[core]["out"]
    return y


if __name__ == "__main__":
    prog = build_program()
    print("program built OK")


# revision 35
# speedup vs baseline: 1.2316x; 1.0242x over previous
"""Trainium2 Bass kernel for a dense transformer block (pre-LN, MHA + MLP).

Full inputs in, full outputs out. Sharding: 8 cores = (batch, seq-half).
Each core computes K/V over its batch element's full 1024 tokens and
Q/attention/MLP over its own 512 tokens (host permutes tokens so the core's
own half is always rows 0..511 — softmax over keys is permutation invariant).
No collectives needed.

v2: fp8 (e4m3) DoubleRow matmuls for QKV / AV / proj (and optionally
fc1/fc2), which stream 2 contraction rows per PE pass. Weights are
quantized per-output-column on the host (absmax -> +-240); dequant scales
fold into the existing bias-add / activation ops, or (for V) into the proj
weights themselves. V is computed in natural [token, feat] layout directly
(h stationary, weights moving), eliminating the separate V transpose pass.
The softmax denominator comes from a constant ones-column appended to V.

Host-side preprocessing folds LayerNorm affine params into the following
matmul weights:  (xhat*g + b) @ W == xhat @ (diag(g) W) + b @ W, and the
V bias into the proj bias: (o/d + vb) @ pw + pb == (o/d) @ pw + (vb@pw + pb).
"""

import sys

sys.path.insert(0, "/opt/trn_rl_repo")

import numpy as np

import concourse.bass as bass
import concourse.bacc as bacc
import concourse.mybir as mybir
import concourse.tile as tile
from concourse.bass_utils import run_bass_kernel_spmd
from concourse.masks import make_identity

P = 128
C = 1024
HEADS = 16
DH = 64
HID = 4096
NTOK = 1024  # tokens per batch element (kv length)
NOWN = 512  # tokens owned by this core (q length)
SCALE = DH ** -0.5
EPS = 1e-5

F32 = mybir.dt.float32
F32R = mybir.dt.float32r
BF16 = mybir.dt.bfloat16
F8 = mybir.dt.float8e4
AF = mybir.ActivationFunctionType
OP = mybir.AluOpType
DROW = mybir.MatmulPerfMode.DoubleRow

CT = C // P  # 8 column tiles of the model dim
CP = CT // 2  # 4 column-tile pairs
TT = NTOK // P  # 8 token tiles (kv)
QT = NOWN // P  # 4 token tiles (own)
HT = HID // P  # 32 hidden tiles
HP = HT // 2  # 16 hidden-tile pairs

VPAD = 128  # per-head padded width of the V tile (DH + ones col + pad);
# padded to 128 so the AV matmul streams full-height (65-row outputs
# measured ~43% slower per instruction)

E4M3_MAX = 240.0

# --- dtype config for the two MLP GEMMs (attention GEMMs are always fp8;
# the error sim shows attention fp8 contributes ~nothing to final error) ---
FC1_FP8 = False
# fc2 split-K: hidden-tile pairs [0, F2SPLIT) run fp8 DoubleRow, the rest
# bf16. Error sim: F2SPLIT=8 -> 1.31e-2 total (gate 2e-2).
F2SPLIT = 8


def build_program():
    nc = bacc.Bacc("TRN2", target_bir_lowering=False)
    mf1 = F8 if FC1_FP8 else BF16

    io = {}
    io["x"] = nc.dram_tensor("x", (NTOK, C), F32, kind="ExternalInput")
    # pre-permuted weights (host layout matches SBUF slabs)
    io["qw"] = nc.dram_tensor("qw", (P, CT, CT, P), F8, kind="ExternalInput")
    io["kw"] = nc.dram_tensor("kw", (P, CT, CT, P), F8, kind="ExternalInput")
    io["vw"] = nc.dram_tensor("vw", (P, CT, C), F8, kind="ExternalInput")
    io["pw"] = nc.dram_tensor("pw", (P, CT, C), F8, kind="ExternalInput")
    io["f1w"] = nc.dram_tensor("f1w", (P, HT, CT, P), mf1, kind="ExternalInput")
    io["f2w8"] = nc.dram_tensor(
        "f2w8", (P, 2, 2 * F2SPLIT, NOWN), F8, kind="ExternalInput"
    )
    io["f2wb"] = nc.dram_tensor(
        "f2wb", (P, 2, HT - 2 * F2SPLIT, NOWN), BF16, kind="ExternalInput"
    )
    # per-partition bias/scale tables, [128, n] layouts
    io["qbt"] = nc.dram_tensor("qbt", (P, CT), F32, kind="ExternalInput")
    io["kbt"] = nc.dram_tensor("kbt", (P, CT), F32, kind="ExternalInput")
    io["qst"] = nc.dram_tensor("qst", (P, CT), F32, kind="ExternalInput")
    io["kst"] = nc.dram_tensor("kst", (P, CT), F32, kind="ExternalInput")
    io["f1bt"] = nc.dram_tensor("f1bt", (P, HT), F32, kind="ExternalInput")
    io["f1st"] = nc.dram_tensor("f1st", (P, HT), F32, kind="ExternalInput")
    # free-dim vectors (broadcast across partitions on chip)
    io["vg"] = nc.dram_tensor("vg", (C,), F32, kind="ExternalInput")
    io["psinv"] = nc.dram_tensor("psinv", (C,), F32, kind="ExternalInput")
    io["pb"] = nc.dram_tensor("pb", (C,), F32, kind="ExternalInput")
    io["f2sinv"] = nc.dram_tensor("f2sinv", (C,), F32, kind="ExternalInput")
    io["f2b"] = nc.dram_tensor("f2b", (C,), F32, kind="ExternalInput")
    io["out"] = nc.dram_tensor("out", (NOWN, C), F32, kind="ExternalOutput")

    with tile.TileContext(nc) as tc:
        _emit(nc, tc, io)
    nc.compile()
    return nc


def _emit(nc, tc, io):
    x_d, out_d = io["x"], io["out"]
    mf1 = F8 if FC1_FP8 else BF16

    with (
        tc.tile_pool(name="consts", bufs=1) as consts,
        tc.tile_pool(name="persist", bufs=1) as persist,
        tc.tile_pool(name="big", bufs=1) as big,
        tc.tile_pool(name="psum_wide", bufs=2, space="PSUM") as psum_wide,
    ):
        # ---- constants (unique tags: each gets its own persistent slot) ----
        ident_f32 = consts.tile([P, P], F32, tag="idf")
        make_identity(nc, ident_f32)
        ident = consts.tile([P, P], BF16, tag="idr")
        nc.vector.tensor_copy(out=ident, in_=ident_f32)
        eps_tile = consts.tile([P, 1], F32, tag="eps")
        nc.vector.memset(eps_tile, EPS)
        qbT = consts.tile([P, CT], F32, tag="qbT")
        nc.sync.dma_start(qbT, io["qbt"][:, :])
        kbT = consts.tile([P, CT], F32, tag="kbT")
        nc.sync.dma_start(kbT, io["kbt"][:, :])
        qsT = consts.tile([P, CT], F32, tag="qsT")
        nc.sync.dma_start(qsT, io["qst"][:, :])
        ksT = consts.tile([P, CT], F32, tag="ksT")
        nc.sync.dma_start(ksT, io["kst"][:, :])
        f1bT = consts.tile([P, HT], F32, tag="f1bT")
        nc.sync.dma_start(f1bT, io["f1bt"][:, :])
        f1sT = consts.tile([P, HT], F32, tag="f1sT")
        nc.sync.dma_start(f1sT, io["f1st"][:, :])

        def bcast_const(src_d, n, tag):
            t = consts.tile([P, n], F32, tag=tag, name=tag)
            src = bass.AP(tensor=src_d, offset=0, ap=[[0, P], [1, n]])
            nc.sync.dma_start(t, src)
            return t

        vg_bc = bcast_const(io["vg"], C, "vg")
        psinv_bc = bcast_const(io["psinv"], C, "psv")
        pb_bc = bcast_const(io["pb"], C, "pbb")
        f2sinv_bc = bcast_const(io["f2sinv"], C, "f2s")
        f2b_bc = bcast_const(io["f2b"], C, "f2bb")

        # own x tiles (fp32, kept for the residual), one tile per token tile;
        # proj writes x2 = x + pb + proj_out back IN PLACE (saves SBUF)
        x_own = []
        for t in range(QT):
            xo = persist.tile([P, C], F32, tag=f"xo{t}", name=f"xo{t}")
            nc.sync.dma_start(xo, x_d[t * P : (t + 1) * P, :])
            x_own.append(xo)
        x2 = x_own

        # persistent weight slabs (single DMA each, reused across sweeps)
        kwslab = persist.tile([P, CT, CT, P], F8, tag="kws", name="kws")
        nc.sync.dma_start(kwslab, io["kw"][:, :, :, :])
        qwslab = persist.tile([P, CT, CT, P], F8, tag="qws", name="qws")
        nc.sync.dma_start(qwslab, io["qw"][:, :, :, :])
        vwslab = persist.tile([P, CT, C], F8, tag="vws", name="vws")
        nc.sync.dma_start(vwslab, io["vw"][:, :, :])
        pslab = persist.tile([P, CT, C], F8, tag="pws", name="pws")
        nc.sync.dma_start(pslab, io["pw"][:, :, :])

        def layernorm_tile(temps, xt):
            """xt: [128, C] fp32 -> returns normalized f32r tile [128, C]."""
            stats = temps.tile([P, 2, 6], F32, tag="ln_stats", name="st")
            for sg in range(2):
                nc.vector.bn_stats(
                    out=stats[:, sg, :], in_=xt[:, sg * 512 : (sg + 1) * 512]
                )
            mv = temps.tile([P, 2], F32, tag="ln_mv", name="mv")
            nc.vector.bn_aggr(out=mv[:], in_=stats[:])
            # (ln/exp-based rsqrt thrashes the Act tables against the
            # attention exp — 1283ns per reload; Sqrt + [P,1] DVE reciprocal
            # is cheap, the reciprocal runs 1 elem/lane on 128 lanes)
            rstd = temps.tile([P, 1], F32, tag="ln_rstd", name="rstd")
            nc.scalar.activation(
                out=rstd, in_=mv[:, 1:2], func=AF.Sqrt, bias=eps_tile, scale=1.0
            )
            nc.vector.reciprocal(out=rstd, in_=rstd)
            nmr = temps.tile([P, 1], F32, tag="ln_nmr", name="nmr")
            nc.vector.tensor_tensor(nmr, mv[:, 0:1], rstd, OP.mult)
            nc.vector.tensor_scalar_mul(nmr, nmr, -1.0)
            # bf16 h: the PE transposes stream 1.0 c/row for bf16 vs 1.5 for
            # f32r, and the transpose PSUM halves to one bank
            h = temps.tile([P, C], BF16, tag="ln_h", name="h")
            nc.any.tensor_scalar(
                out=h,
                in0=xt,
                scalar1=rstd,
                scalar2=nmr,
                op0=OP.mult,
                op1=OP.add,
            )
            return h

        # ---- persistent activation tiles ----
        # hT2[(cp, t2)]: [P, 2, 512] fp8 — transposed LN1 output, c-tile pairs
        hT2 = {
            (cp, t2): big.tile(
                [P, 2, NOWN], F8, tag=f"hT{cp}_{t2}", name=f"hT{cp}_{t2}"
            )
            for cp in range(CP)
            for t2 in range(2)
        }
        # kT[(ft, t2)]: [P, 512] bf16 (QK stays bf16)
        kT = {
            (ft, t2): big.tile(
                [P, NOWN], BF16, tag=f"kT{ft}_{t2}", name=f"kT{ft}_{t2}"
            )
            for ft in range(CT)
            for t2 in range(2)
        }
        # qT: one zero-padded tile per head (own head's 64 rows at its
        # natural partition offset, other head's rows zero). QK can then run
        # full-height 128-contraction matmuls — measured ~35% faster than
        # the 64-row form — with kT packed as-is.
        qT = [
            big.tile([P, NOWN], BF16, tag=f"qT{hq}", name=f"qT{hq}")
            for hq in range(HEADS)
        ]
        for hq in range(HEADS):
            nc.vector.memset(qT[hq][:], 0.0)
        # vh[t2]: [P, 4, HEADS, VPAD] fp8 — V in natural token layout,
        # per-head padded; col DH holds 1.0 (softmax denominator trick)
        vh = [
            big.tile([P, QT, HEADS, VPAD], F8, tag=f"vh{t2}", name=f"vh{t2}")
            for t2 in range(2)
        ]
        # oT2[fp]: [P, 2, 512] fp8 — attention output, feature-tile pairs
        oT2 = [
            big.tile([P, 2, NOWN], F8, tag=f"oT{fp}", name=f"oT{fp}")
            for fp in range(CP)
        ]
        h2T2 = [
            big.tile([P, 2, NOWN], mf1, tag=f"h2T{cp}", name=f"h2T{cp}")
            for cp in range(CP)
        ]
        actT2 = [
            big.tile(
                [P, 2, NOWN],
                F8 if hp < F2SPLIT else BF16,
                tag=f"aT{hp}",
                name=f"aT{hp}",
            )
            for hp in range(HP)
        ]

        for t2 in range(2):
            nc.vector.memset(vh[t2][:], 0.0)
            nc.vector.memset(vh[t2][:, :, :, DH : DH + 1], 1.0)

        # ================= Phase 1: LN1 -> hT2 =================
        with (
            tc.tile_pool(name="ln1", bufs=2) as ln1,
            tc.tile_pool(name="xtmp", bufs=2) as xtmp,
        ):
            for t in range(TT):
                if t < QT:
                    xt = x_own[t]
                else:
                    xt = xtmp.tile([P, C], F32, tag="xt", name="xt")
                    nc.sync.dma_start(xt, x_d[t * P : (t + 1) * P, :])
                h = layernorm_tile(ln1, xt)
                t2, tb = t // QT, t % QT
                ps = psum_wide.tile([P, C], BF16, tag="w", name=f"trp{t}")
                for ft in range(CT):
                    nc.tensor.transpose(
                        ps[:, ft * P : (ft + 1) * P],
                        h[:, ft * P : (ft + 1) * P],
                        ident,
                    )
                for cp in range(CP):
                    nc.any.tensor_copy(
                        out=hT2[(cp, t2)][:, :, tb * P : (tb + 1) * P],
                        in_=ps[:, cp * 2 * P : (cp + 1) * 2 * P].rearrange(
                            "p (two f) -> p two f", two=2
                        ),
                    )

        # ================= Phase 2: QKV =================
        HB = HEADS // 2  # heads per 512-wide V block
        with tc.tile_pool(name="qkv_psum", bufs=4, space="PSUM") as qkv_psum:

            def kq_sweep(t2, wslab, bT, sT, dst, per_head=False):
                """K or Q: transposed-output sweep; dst[ft] <- [P,512] bf16"""
                for ft in range(CT):
                    ps = qkv_psum.tile([P, NOWN], F32, tag="kvps", name="kvps")
                    for cp in range(CP):
                        nc.tensor.matmul(
                            ps,
                            lhsT=wslab[:, ft, 2 * cp : 2 * cp + 2, :],
                            rhs=hT2[(cp, t2)],
                            start=(cp == 0),
                            stop=(cp == CP - 1),
                            perf_mode=DROW,
                        )
                    if per_head:
                        # split into the two heads' zero-padded tiles,
                        # partition-aligned (head j keeps rows j*64..)
                        for j in range(2):
                            rows = slice(j * DH, (j + 1) * DH)
                            nc.any.tensor_scalar(
                                out=dst[2 * ft + j][rows, :],
                                in0=ps[rows, :],
                                scalar1=sT[rows, ft : ft + 1],
                                scalar2=bT[rows, ft : ft + 1],
                                op0=OP.mult,
                                op1=OP.add,
                            )
                    else:
                        nc.any.tensor_scalar(
                            out=dst[ft],
                            in0=ps,
                            scalar1=sT[:, ft : ft + 1],
                            scalar2=bT[:, ft : ft + 1],
                            op0=OP.mult,
                            op1=OP.add,
                        )

            def v_sweep(t2):
                """V in natural layout: h stationary, vw moving."""
                for tb in range(QT):
                    ps = psum_wide.tile(
                        [P, HEADS, DH], F32, tag="w", name=f"vps{t2}_{tb}"
                    )
                    for cp in range(CP):
                        for blk in range(2):
                            nc.tensor.matmul(
                                ps[:, blk * HB : (blk + 1) * HB, :],
                                lhsT=hT2[(cp, t2)][
                                    :, :, tb * P : (tb + 1) * P
                                ],
                                rhs=vwslab[
                                    :,
                                    2 * cp : 2 * cp + 2,
                                    blk * 512 : (blk + 1) * 512,
                                ],
                                start=(cp == 0),
                                stop=(cp == CP - 1),
                                perf_mode=DROW,
                            )
                    nc.any.tensor_tensor(
                        vh[t2][:, tb, :, :DH],
                        ps,
                        vg_bc[:, :].rearrange("p (h d) -> p h d", h=HEADS),
                        OP.mult,
                    )

            kq_sweep(0, kwslab, kbT, ksT, [kT[(f, 0)] for f in range(CT)])
            v_sweep(0)
            kq_sweep(0, qwslab, qbT, qsT, qT, per_head=True)
            kq_sweep(1, kwslab, kbT, ksT, [kT[(f, 1)] for f in range(CT)])
            v_sweep(1)

        # ================= Phase 3: attention =================
        with (
            tc.tile_pool(name="attn", bufs=3) as attn_pool,
            tc.tile_pool(name="attn_ot", bufs=4, space="PSUM") as attn_ot,
        ):
            for h in range(HEADS):
                prow = (h % 2) * DH
                ftile = h // 2
                p_sb = attn_pool.tile([P, TT, NOWN], F8, tag="p_sb", name="p")
                for cp in range(CP):
                    st = psum_wide.tile(
                        [P, 2, NOWN], F32, tag="w", name=f"st{h}_{cp}"
                    )
                    for j in range(2):
                        c = cp * 2 + j
                        # full-height lhsT: the other head's q rows are zero
                        kv_slice = kT[(ftile, c // QT)][
                            :, (c % QT) * P : (c % QT + 1) * P
                        ]
                        nc.tensor.matmul(
                            st[:, j, :],
                            lhsT=kv_slice,
                            rhs=qT[h],
                            start=True,
                            stop=True,
                        )
                    # p = exp(SCALE * s)   (fp8 out)
                    nc.scalar.activation(
                        out=p_sb[:, 2 * cp : 2 * cp + 2, :],
                        in_=st,
                        func=AF.Exp,
                        scale=SCALE,
                    )
                ot = attn_ot.tile([P, NOWN], F32, tag="ot", name="ot")
                for cp in range(CP):
                    t2, c2 = cp // 2, cp % 2
                    nc.tensor.matmul(
                        ot,
                        lhsT=vh[t2][:, 2 * c2 : 2 * c2 + 2, h, :],
                        rhs=p_sb[:, 2 * cp : 2 * cp + 2, :],
                        start=(cp == 0),
                        stop=(cp == CP - 1),
                        perf_mode=DROW,
                    )
                # softmax denominators arrive in row DH (ones column of vh)
                rs = attn_pool.tile([1, NOWN], F32, tag="rs", name="rs")
                nc.vector.reciprocal(out=rs, in_=ot[DH : DH + 1, :])
                rsb = attn_pool.tile([DH, NOWN], F32, tag="rsb", name="rsb")
                nc.gpsimd.partition_broadcast(rsb, rs)
                nc.any.tensor_tensor(
                    oT2[ftile // 2][prow : prow + DH, ftile % 2, :],
                    ot[:DH, :],
                    rsb,
                    OP.mult,
                )

        # ================= Phase 4: proj + residual -> x2 (in place) ========
        # fold pb into x_own first (x_own already consumed by LN1; tile deps
        # order this correctly)
        for tq in range(QT):
            nc.any.tensor_tensor(x_own[tq], x_own[tq], pb_bc, OP.add)
        # tq-outer so x2[tq] completes both halves early -> LN2 tile tq can
        # start while proj continues on later tq
        with tc.tile_pool(name="proj_ps", bufs=4, space="PSUM") as proj_ps:
            for tq in range(QT):
                for ns in range(2):
                    nsl = slice(ns * 512, (ns + 1) * 512)
                    ps = proj_ps.tile([P, 512], F32, tag="pps", name="pps")
                    for fp in range(CP):
                        nc.tensor.matmul(
                            ps,
                            lhsT=oT2[fp][:, :, tq * P : (tq + 1) * P],
                            rhs=pslab[:, 2 * fp : 2 * fp + 2, nsl],
                            start=(fp == 0),
                            stop=(fp == CP - 1),
                            perf_mode=DROW,
                        )
                    nc.vector.tensor_tensor(ps, ps, psinv_bc[:, nsl], OP.mult)
                    nc.vector.tensor_tensor(
                        x2[tq][:, nsl], ps, x_own[tq][:, nsl], OP.add
                    )

        # ================= Phase 5: LN2 -> h2T2 =================
        with tc.tile_pool(name="ln2", bufs=2) as ln2:
            for t in range(QT):
                h = layernorm_tile(ln2, x2[t])
                ps = psum_wide.tile([P, C], BF16, tag="w", name=f"tr2{t}")
                for ft in range(CT):
                    nc.tensor.transpose(
                        ps[:, ft * P : (ft + 1) * P],
                        h[:, ft * P : (ft + 1) * P],
                        ident,
                    )
                for cp in range(CP):
                    nc.any.tensor_copy(
                        out=h2T2[cp][:, :, t * P : (t + 1) * P],
                        in_=ps[:, cp * 2 * P : (cp + 1) * 2 * P].rearrange(
                            "p (two f) -> p two f", two=2
                        ),
                    )

        # fold the fc2 bias into the residual once LN2 has consumed x2
        # (shortens the fc2 output chain to mult+add)
        for tq in range(QT):
            nc.any.tensor_tensor(x2[tq], x2[tq], f2b_bc, OP.add)

        # ================= Phase 6: FC1 + gelu -> actT2 =================
        with (
            tc.tile_pool(name="f1c", bufs=8) as f1c,
            tc.tile_pool(name="f1_ps", bufs=4, space="PSUM") as f1_ps,
        ):
            for hf in range(HT):
                ps = f1_ps.tile([P, NOWN], F32, tag="f1ps", name="f1ps")
                slab = f1c.tile([P, CT, P], mf1, tag="f1w", name="f1slab")
                nc.sync.dma_start(slab, io["f1w"][:, hf])
                if FC1_FP8:
                    for cp in range(CP):
                        nc.tensor.matmul(
                            ps,
                            lhsT=slab[:, 2 * cp : 2 * cp + 2, :],
                            rhs=h2T2[cp],
                            start=(cp == 0),
                            stop=(cp == CP - 1),
                            perf_mode=DROW,
                        )
                else:
                    for c in range(CT):
                        nc.tensor.matmul(
                            ps,
                            lhsT=slab[:, c, :],
                            rhs=h2T2[c // 2][:, c % 2, :],
                            start=(c == 0),
                            stop=(c == CT - 1),
                        )
                # gelu(ps * s + b), fused dequant+bias via activation
                nc.scalar.activation(
                    out=actT2[hf // 2][:, hf % 2, :],
                    in_=ps,
                    func=AF.Gelu,
                    bias=f1bT[:, hf : hf + 1],
                    scale=f1sT[:, hf : hf + 1],
                )

        # ================= Phase 7: FC2 + residual -> out =================
        # split-K: hidden tiles [0, 2*F2SPLIT) in fp8 DoubleRow, rest bf16
        NG = 4  # hidden-tile groups per DMA chunk
        N8G = 2 * F2SPLIT // NG  # fp8 groups
        NBG = (HT - 2 * F2SPLIT) // NG  # bf16 groups
        with (
            tc.tile_pool(name="f2c", bufs=3) as f2c,
            tc.tile_pool(name="f2_ps", bufs=1, space="PSUM") as f2_ps,
            tc.tile_pool(name="out_sb", bufs=2) as out_pool,
        ):
            for ns in range(2):
                nsl = slice(ns * 512, (ns + 1) * 512)
                pss = [
                    f2_ps.tile([P, 512], F32, tag=f"f2ps{tq}", name=f"f2ps{tq}")
                    for tq in range(QT)
                ]
                for g in range(N8G):
                    gw = f2c.tile([P, NG, 512], F8, tag="f2w8", name=f"f2w8g{g}")
                    nc.sync.dma_start(
                        gw, io["f2w8"][:, ns, g * NG : (g + 1) * NG, :]
                    )
                    for tq in range(QT):
                        for i in range(NG // 2):
                            hp = (g * NG) // 2 + i
                            nc.tensor.matmul(
                                pss[tq],
                                lhsT=actT2[hp][:, :, tq * P : (tq + 1) * P],
                                rhs=gw[:, 2 * i : 2 * i + 2, :],
                                start=(g == 0 and i == 0),
                                stop=False,
                                perf_mode=DROW,
                            )
                for g in range(NBG):
                    gw = f2c.tile(
                        [P, NG, 512], BF16, tag="f2wb", name=f"f2wbg{g}"
                    )
                    nc.sync.dma_start(
                        gw, io["f2wb"][:, ns, g * NG : (g + 1) * NG, :]
                    )
                    for tq in range(QT):
                        for i in range(NG):
                            hc = 2 * F2SPLIT + g * NG + i
                            nc.tensor.matmul(
                                pss[tq],
                                lhsT=actT2[hc // 2][
                                    :, hc % 2, tq * P : (tq + 1) * P
                                ],
                                rhs=gw[:, i, :],
                                start=False,
                                stop=(g == NBG - 1 and i == NG - 1),
                            )
                for tq in range(QT):
                    ot2 = out_pool.tile([P, 512], F32, tag="out_t", name="o")
                    nc.vector.tensor_tensor(
                        ot2, pss[tq], f2sinv_bc[:, nsl], OP.mult
                    )
                    nc.vector.tensor_tensor(ot2, ot2, x2[tq][:, nsl], OP.add)
                    nc.sync.dma_start(out_d[tq * P : (tq + 1) * P, nsl], ot2)


_PROGRAM = None


def _get_program():
    global _PROGRAM
    if _PROGRAM is None:
        _PROGRAM = build_program()
    return _PROGRAM


def _quant_cols(w, dtype):
    """per-output-column absmax quantization; returns (w_q, dequant_scales)"""
    import ml_dtypes

    w = np.asarray(w, np.float64)
    if dtype == "fp8":
        amax = np.abs(w).max(axis=0)
        amax = np.where(amax == 0, 1.0, amax)
        s = E4M3_MAX / amax
        wq = np.clip(w * s, -E4M3_MAX, E4M3_MAX).astype(ml_dtypes.float8_e4m3)
        return wq, (1.0 / s).astype(np.float32)
    else:
        wq = w.astype(ml_dtypes.bfloat16)
        return wq, np.ones(w.shape[1], np.float32)


def build_in_maps(inputs):
    import ml_dtypes

    x = np.asarray(inputs["x"], np.float32)  # [4, 1024, 1024]
    ln1_g = np.asarray(inputs["ln1_g"], np.float64)
    ln1_b = np.asarray(inputs["ln1_b"], np.float64)
    ln2_g = np.asarray(inputs["ln2_g"], np.float64)
    ln2_b = np.asarray(inputs["ln2_b"], np.float64)
    qkv_w = np.asarray(inputs["qkv_w"], np.float64)
    qkv_b = np.asarray(inputs["qkv_b"], np.float64)
    proj_w = np.asarray(inputs["proj_w"], np.float64)
    proj_b = np.asarray(inputs["proj_b"], np.float64)
    fc1_w = np.asarray(inputs["fc1_w"], np.float64)
    fc1_b = np.asarray(inputs["fc1_b"], np.float64)
    fc2_w = np.asarray(inputs["fc2_w"], np.float64)
    fc2_b = np.asarray(inputs["fc2_b"], np.float64)

    # Fold LN affine into the following matmul:
    #   (xhat*g + b) @ W == xhat @ (diag(g) W) + b @ W
    qkv_w_f = ln1_g[:, None] * qkv_w
    qkv_b_f = qkv_b + ln1_b @ qkv_w
    f1w_f = ln2_g[:, None] * fc1_w
    f1b_f = fc1_b + ln2_b @ fc1_w

    qw = qkv_w_f[:, :C]
    kw = qkv_w_f[:, C : 2 * C]
    vw = qkv_w_f[:, 2 * C :]
    vb = qkv_b_f[2 * C :]

    # --- Q/K: per-column fp8 quant, dequant scale applied on chip ---
    qw8, qsinv = _quant_cols(qw, "fp8")
    kw8, ksinv = _quant_cols(kw, "fp8")

    # --- V: per-column fp8 quant; on-chip the psum is rescaled by vg so the
    # fp8 V tile holds v*t with t = 24/||vw_col||; t and the v bias both fold
    # into the proj weights/bias ---
    vw8, vsinv = _quant_cols(vw, "fp8")
    vnorm = np.linalg.norm(vw, axis=0)
    vnorm = np.where(vnorm == 0, 1.0, vnorm)
    t_v = 24.0 / vnorm
    vg = (vsinv * t_v).astype(np.float32)  # psum -> fp8 V scaling

    # --- proj: fold t_v and v bias; per-column fp8 quant ---
    pw_eff = proj_w / t_v[:, None]
    pb_eff = proj_b + vb @ proj_w
    pw8, psinv = _quant_cols(pw_eff, "fp8")

    # --- fc1 ---
    f1w8, f1sinv = _quant_cols(f1w_f, "fp8" if FC1_FP8 else "bf16")

    # --- fc2 split-K: rows [0, 256*F2SPLIT) fp8 (col-scaled), rest bf16
    # pre-scaled by the same column scales so one dequant applies to both ---
    k8 = 2 * F2SPLIT * P
    amax = np.abs(fc2_w[:k8]).max(axis=0)
    amax = np.where(amax == 0, 1.0, amax)
    s2 = E4M3_MAX / amax
    f2hi = np.clip(fc2_w[:k8] * s2, -E4M3_MAX, E4M3_MAX).astype(
        ml_dtypes.float8_e4m3
    )
    f2lo = (fc2_w[k8:] * s2).astype(ml_dtypes.bfloat16)
    f2sinv = (1.0 / s2).astype(np.float32)

    # --- permute weights into SBUF slab layouts ---
    # q/k: [p, ft, c, f] from w[c*128+p, ft*128+f]
    def perm_kq(w8):
        return np.ascontiguousarray(
            w8.reshape(CT, P, CT, P).transpose(1, 2, 0, 3)
        )

    # v/proj: [p, c, n] from w[c*128+p, n]
    def perm_cn(w8):
        return np.ascontiguousarray(w8.reshape(CT, P, C).transpose(1, 0, 2))

    # fc1: [p, hf, c, f] from w[c*128+p, hf*128+f]
    f1wP = np.ascontiguousarray(
        f1w8.reshape(CT, P, HT, P).transpose(1, 2, 0, 3)
    )
    # fc2: [p, ns, hc, n] from w[hc*128+p, ns*512+n]
    f2wP8 = np.ascontiguousarray(
        f2hi.reshape(2 * F2SPLIT, P, 2, NOWN).transpose(1, 2, 0, 3)
    )
    f2wPb = np.ascontiguousarray(
        f2lo.reshape(HT - 2 * F2SPLIT, P, 2, NOWN).transpose(1, 2, 0, 3)
    )

    def tbias(b):  # [n*128] -> [128, n] per-partition layout
        return np.ascontiguousarray(
            np.asarray(b, np.float32).reshape(-1, P).T
        )

    common = dict(
        qw=perm_kq(qw8),
        kw=perm_kq(kw8),
        vw=perm_cn(vw8),
        pw=perm_cn(pw8),
        f1w=f1wP,
        f2w8=f2wP8,
        f2wb=f2wPb,
        qbt=tbias(qkv_b_f[:C]),
        kbt=tbias(qkv_b_f[C : 2 * C]),
        qst=tbias(qsinv),
        kst=tbias(ksinv),
        f1bt=tbias(f1b_f),
        f1st=tbias(f1sinv),
        vg=vg,
        psinv=psinv.astype(np.float32),
        pb=pb_eff.astype(np.float32),
        f2sinv=f2sinv.astype(np.float32),
        f2b=fc2_b.astype(np.float32),
    )
    in_maps = []
    for core in range(8):
        b, half = core // 2, core % 2
        own = x[b, half * NOWN : (half + 1) * NOWN, :]
        other = x[b, (1 - half) * NOWN : (2 - half) * NOWN, :]
        xp = np.ascontiguousarray(np.concatenate([own, other], axis=0))
        in_maps.append({**common, "x": xp})
    return in_maps


def kernel(**inputs):
    in_maps = build_in_maps(inputs)
    nc = _get_program()
    res = run_bass_kernel_spmd(nc, in_maps, core_ids=list(range(8)))
    outs = res.results

    y = np.empty((4, NTOK, C), np.float32)
    for core in range(8):
        b, half = core // 2, core % 2
        y[b, half * NOWN : (half + 1) * NOWN, :] = outs[core]["out"]
    return y


if __name__ == "__main__":
    prog = build_program()
    print("program built OK")


# revision 48
# speedup vs baseline: 1.3044x; 1.0591x over previous
"""Trainium2 Bass kernel for a dense transformer block (pre-LN, MHA + MLP).

Full inputs in, full outputs out. Sharding: 8 cores = (batch, seq-half).
Each core computes K/V over its batch element's full 1024 tokens and
Q/attention/MLP over its own 512 tokens (host permutes tokens so the core's
own half is always rows 0..511 — softmax over keys is permutation invariant).
No collectives needed.

v2: fp8 (e4m3) DoubleRow matmuls for QKV / AV / proj (and optionally
fc1/fc2), which stream 2 contraction rows per PE pass. Weights are
quantized per-output-column on the host (absmax -> +-240); dequant scales
fold into the existing bias-add / activation ops, or (for V) into the proj
weights themselves. V is computed in natural [token, feat] layout directly
(h stationary, weights moving), eliminating the separate V transpose pass.
The softmax denominator comes from a constant ones-column appended to V.

Host-side preprocessing folds LayerNorm affine params into the following
matmul weights:  (xhat*g + b) @ W == xhat @ (diag(g) W) + b @ W, and the
V bias into the proj bias: (o/d + vb) @ pw + pb == (o/d) @ pw + (vb@pw + pb).
"""

import sys

sys.path.insert(0, "/opt/trn_rl_repo")

import numpy as np

import concourse.bass as bass
import concourse.bacc as bacc
import concourse.mybir as mybir
import concourse.tile as tile
from concourse.bass_utils import run_bass_kernel_spmd
from concourse.masks import make_identity

P = 128
C = 1024
HEADS = 16
DH = 64
HID = 4096
NTOK = 1024  # tokens per batch element (kv length)
NOWN = 512  # tokens owned by this core (q length)
SCALE = DH ** -0.5
EPS = 1e-5

F32 = mybir.dt.float32
F32R = mybir.dt.float32r
BF16 = mybir.dt.bfloat16
F8 = mybir.dt.float8e4
AF = mybir.ActivationFunctionType
OP = mybir.AluOpType
DROW = mybir.MatmulPerfMode.DoubleRow

CT = C // P  # 8 column tiles of the model dim
CP = CT // 2  # 4 column-tile pairs
TT = NTOK // P  # 8 token tiles (kv)
QT = NOWN // P  # 4 token tiles (own)
HT = HID // P  # 32 hidden tiles
HP = HT // 2  # 16 hidden-tile pairs

VPAD = 128  # per-head padded width of the V tile (DH + ones col + pad);
# padded to 128 so the AV matmul streams full-height (65-row outputs
# measured ~43% slower per instruction)

E4M3_MAX = 240.0

# --- dtype config for the two MLP GEMMs (attention GEMMs are always fp8;
# the error sim shows attention fp8 contributes ~nothing to final error) ---
FC1_FP8 = False
# fc2 split-K: hidden-tile pairs [0, F2SPLIT) run fp8 DoubleRow, the rest
# bf16. Error sim: F2SPLIT=8 -> 1.31e-2 total (gate 2e-2).
F2SPLIT = 8


def build_program():
    nc = bacc.Bacc("TRN2", target_bir_lowering=False)
    mf1 = F8 if FC1_FP8 else BF16

    io = {}
    io["x"] = nc.dram_tensor("x", (NTOK, C), F32, kind="ExternalInput")
    # pre-permuted weights (host layout matches SBUF slabs)
    io["qw"] = nc.dram_tensor("qw", (P, CT, CT, P), F8, kind="ExternalInput")
    io["kw"] = nc.dram_tensor("kw", (P, CT, CT, P), F8, kind="ExternalInput")
    io["vw"] = nc.dram_tensor("vw", (P, CT, C), F8, kind="ExternalInput")
    io["pw"] = nc.dram_tensor("pw", (P, CT, C), F8, kind="ExternalInput")
    io["f1w"] = nc.dram_tensor("f1w", (P, HT, CT, P), mf1, kind="ExternalInput")
    io["f2w8"] = nc.dram_tensor(
        "f2w8", (P, 2, 2 * F2SPLIT, NOWN), F8, kind="ExternalInput"
    )
    io["f2wb"] = nc.dram_tensor(
        "f2wb", (P, 2, HT - 2 * F2SPLIT, NOWN), BF16, kind="ExternalInput"
    )
    # per-partition bias/scale tables, [128, n] layouts
    io["qbt"] = nc.dram_tensor("qbt", (P, CT), F32, kind="ExternalInput")
    io["kbt"] = nc.dram_tensor("kbt", (P, CT), F32, kind="ExternalInput")
    io["qst"] = nc.dram_tensor("qst", (P, CT), F32, kind="ExternalInput")
    io["kst"] = nc.dram_tensor("kst", (P, CT), F32, kind="ExternalInput")
    io["f1bt"] = nc.dram_tensor("f1bt", (P, HT), F32, kind="ExternalInput")
    io["f1st"] = nc.dram_tensor("f1st", (P, HT), F32, kind="ExternalInput")
    # free-dim vectors (broadcast across partitions on chip)
    io["vg"] = nc.dram_tensor("vg", (C,), F32, kind="ExternalInput")
    io["psinv"] = nc.dram_tensor("psinv", (C,), F32, kind="ExternalInput")
    io["pb"] = nc.dram_tensor("pb", (C,), F32, kind="ExternalInput")
    io["f2sinv"] = nc.dram_tensor("f2sinv", (C,), F32, kind="ExternalInput")
    io["f2b"] = nc.dram_tensor("f2b", (C,), F32, kind="ExternalInput")
    io["out"] = nc.dram_tensor("out", (NOWN, C), F32, kind="ExternalOutput")

    with tile.TileContext(nc) as tc:
        _emit(nc, tc, io)
    nc.compile()
    return nc


def _emit(nc, tc, io):
    x_d, out_d = io["x"], io["out"]
    mf1 = F8 if FC1_FP8 else BF16

    with (
        tc.tile_pool(name="consts", bufs=1) as consts,
        tc.tile_pool(name="persist", bufs=1) as persist,
        tc.tile_pool(name="big", bufs=1) as big,
        tc.tile_pool(name="psum_wide", bufs=2, space="PSUM") as psum_wide,
    ):
        # ---- constants (unique tags: each gets its own persistent slot) ----
        ident_f32 = consts.tile([P, P], F32, tag="idf")
        make_identity(nc, ident_f32)
        ident = consts.tile([P, P], BF16, tag="idr")
        nc.vector.tensor_copy(out=ident, in_=ident_f32)
        eps_tile = consts.tile([P, 1], F32, tag="eps")
        nc.vector.memset(eps_tile, EPS)
        qbT = consts.tile([P, CT], F32, tag="qbT")
        nc.sync.dma_start(qbT, io["qbt"][:, :])
        kbT = consts.tile([P, CT], F32, tag="kbT")
        nc.sync.dma_start(kbT, io["kbt"][:, :])
        qsT = consts.tile([P, CT], F32, tag="qsT")
        nc.sync.dma_start(qsT, io["qst"][:, :])
        ksT = consts.tile([P, CT], F32, tag="ksT")
        nc.sync.dma_start(ksT, io["kst"][:, :])
        f1bT = consts.tile([P, HT], F32, tag="f1bT")
        nc.sync.dma_start(f1bT, io["f1bt"][:, :])
        f1sT = consts.tile([P, HT], F32, tag="f1sT")
        nc.sync.dma_start(f1sT, io["f1st"][:, :])

        def bcast_const(src_d, n, tag):
            t = consts.tile([P, n], F32, tag=tag, name=tag)
            src = bass.AP(tensor=src_d, offset=0, ap=[[0, P], [1, n]])
            nc.sync.dma_start(t, src)
            return t

        vg_bc = bcast_const(io["vg"], C, "vg")
        psinv_bc = bcast_const(io["psinv"], C, "psv")
        pb_bc = bcast_const(io["pb"], C, "pbb")
        f2sinv_bc = bcast_const(io["f2sinv"], C, "f2s")
        f2b_bc = bcast_const(io["f2b"], C, "f2bb")

        # own x tiles (fp32, kept for the residual), one tile per token tile;
        # proj writes x2 = x + pb + proj_out back IN PLACE (saves SBUF)
        x_own = []
        for t in range(QT):
            xo = persist.tile([P, C], F32, tag=f"xo{t}", name=f"xo{t}")
            nc.sync.dma_start(xo, x_d[t * P : (t + 1) * P, :])
            x_own.append(xo)
        x2 = x_own

        # persistent weight slabs (single DMA each, reused across sweeps)
        kwslab = persist.tile([P, CT, CT, P], F8, tag="kws", name="kws")
        nc.sync.dma_start(kwslab, io["kw"][:, :, :, :])
        qwslab = persist.tile([P, CT, CT, P], F8, tag="qws", name="qws")
        nc.sync.dma_start(qwslab, io["qw"][:, :, :, :])
        vwslab = persist.tile([P, CT, C], F8, tag="vws", name="vws")
        nc.sync.dma_start(vwslab, io["vw"][:, :, :])
        pslab = persist.tile([P, CT, C], F8, tag="pws", name="pws")
        nc.sync.dma_start(pslab, io["pw"][:, :, :])

        def layernorm_tile(temps, xt):
            """xt: [128, C] fp32 -> returns normalized f32r tile [128, C]."""
            stats = temps.tile([P, 2, 6], F32, tag="ln_stats", name="st")
            for sg in range(2):
                nc.vector.bn_stats(
                    out=stats[:, sg, :], in_=xt[:, sg * 512 : (sg + 1) * 512]
                )
            mv = temps.tile([P, 2], F32, tag="ln_mv", name="mv")
            nc.vector.bn_aggr(out=mv[:], in_=stats[:])
            # (ln/exp-based rsqrt thrashes the Act tables against the
            # attention exp — 1283ns per reload; Sqrt + [P,1] DVE reciprocal
            # is cheap, the reciprocal runs 1 elem/lane on 128 lanes)
            rstd = temps.tile([P, 1], F32, tag="ln_rstd", name="rstd")
            nc.scalar.activation(
                out=rstd, in_=mv[:, 1:2], func=AF.Sqrt, bias=eps_tile, scale=1.0
            )
            # approx reciprocal is ~5x faster; SBUF input (incl. in-place) is
            # exact to ~3e-6 on HW — only PSUM inputs misread (measured)
            nc.vector.reciprocal_approx_fast(out=rstd, in_=rstd)
            nmr = temps.tile([P, 1], F32, tag="ln_nmr", name="nmr")
            nc.vector.tensor_tensor(nmr, mv[:, 0:1], rstd, OP.mult)
            nc.vector.tensor_scalar_mul(nmr, nmr, -1.0)
            # bf16 h: the PE transposes stream 1.0 c/row for bf16 vs 1.5 for
            # f32r, and the transpose PSUM halves to one bank
            h = temps.tile([P, C], BF16, tag="ln_h", name="h")
            nc.any.tensor_scalar(
                out=h,
                in0=xt,
                scalar1=rstd,
                scalar2=nmr,
                op0=OP.mult,
                op1=OP.add,
            )
            return h

        # ---- persistent activation tiles ----
        # hT2[(cp, t2)]: [P, 2, 512] fp8 — transposed LN1 output, c-tile pairs
        hT2 = {
            (cp, t2): big.tile(
                [P, 2, NOWN], F8, tag=f"hT{cp}_{t2}", name=f"hT{cp}_{t2}"
            )
            for cp in range(CP)
            for t2 in range(2)
        }
        # kT[(ft, t2)]: [P, 512] bf16 (QK stays bf16)
        kT = {
            (ft, t2): big.tile(
                [P, NOWN], BF16, tag=f"kT{ft}_{t2}", name=f"kT{ft}_{t2}"
            )
            for ft in range(CT)
            for t2 in range(2)
        }
        # qT: one zero-padded tile per head (own head's 64 rows at its
        # natural partition offset, other head's rows zero). QK can then run
        # full-height 128-contraction matmuls — measured ~35% faster than
        # the 64-row form — with kT packed as-is.
        qT = [
            big.tile([P, NOWN], BF16, tag=f"qT{hq}", name=f"qT{hq}")
            for hq in range(HEADS)
        ]
        for hq in range(HEADS):
            nc.vector.memset(qT[hq][:], 0.0)
        # vh[t2]: [P, 4, HEADS, VPAD] fp8 — V in natural token layout,
        # per-head padded; col DH holds 1.0 (softmax denominator trick)
        vh = [
            big.tile([P, QT, HEADS, VPAD], F8, tag=f"vh{t2}", name=f"vh{t2}")
            for t2 in range(2)
        ]
        # oT2[fp]: [P, 2, 512] fp8 — attention output, feature-tile pairs
        oT2 = [
            big.tile([P, 2, NOWN], F8, tag=f"oT{fp}", name=f"oT{fp}")
            for fp in range(CP)
        ]
        h2T2 = [
            big.tile([P, 2, NOWN], mf1, tag=f"h2T{cp}", name=f"h2T{cp}")
            for cp in range(CP)
        ]
        actT2 = [
            big.tile(
                [P, 2, NOWN],
                F8 if hp < F2SPLIT else BF16,
                tag=f"aT{hp}",
                name=f"aT{hp}",
            )
            for hp in range(HP)
        ]

        for t2 in range(2):
            nc.vector.memset(vh[t2][:], 0.0)
            nc.vector.memset(vh[t2][:, :, :, DH : DH + 1], 1.0)

        # ================= Phase 1: LN1 -> hT2 =================
        with (
            tc.tile_pool(name="ln1", bufs=2) as ln1,
            tc.tile_pool(name="xtmp", bufs=2) as xtmp,
        ):
            for t in range(TT):
                if t < QT:
                    xt = x_own[t]
                else:
                    xt = xtmp.tile([P, C], F32, tag="xt", name="xt")
                    nc.sync.dma_start(xt, x_d[t * P : (t + 1) * P, :])
                h = layernorm_tile(ln1, xt)
                t2, tb = t // QT, t % QT
                ps = psum_wide.tile([P, C], BF16, tag="tr", name=f"trp{t}")
                for ft in range(CT):
                    nc.tensor.transpose(
                        ps[:, ft * P : (ft + 1) * P],
                        h[:, ft * P : (ft + 1) * P],
                        ident,
                    )
                for cp in range(CP):
                    nc.any.tensor_copy(
                        out=hT2[(cp, t2)][:, :, tb * P : (tb + 1) * P],
                        in_=ps[:, cp * 2 * P : (cp + 1) * 2 * P].rearrange(
                            "p (two f) -> p two f", two=2
                        ),
                    )

        # ================= Phase 2: QKV =================
        HB = HEADS // 2  # heads per 512-wide V block
        with tc.tile_pool(name="qkv_psum", bufs=2, space="PSUM") as qkv_psum:

            def kq_sweep(t2, wslab, bT, sT, dst, per_head=False):
                """K or Q: transposed-output sweep; dst[ft] <- [P,512] bf16"""
                for ft in range(CT):
                    ps = qkv_psum.tile([P, NOWN], F32, tag="kvps", name="kvps")
                    for cp in range(CP):
                        nc.tensor.matmul(
                            ps,
                            lhsT=wslab[:, ft, 2 * cp : 2 * cp + 2, :],
                            rhs=hT2[(cp, t2)],
                            start=(cp == 0),
                            stop=(cp == CP - 1),
                            perf_mode=DROW,
                        )
                    if per_head:
                        # split into the two heads' zero-padded tiles,
                        # partition-aligned (head j keeps rows j*64..)
                        for j in range(2):
                            rows = slice(j * DH, (j + 1) * DH)
                            nc.any.tensor_scalar(
                                out=dst[2 * ft + j][rows, :],
                                in0=ps[rows, :],
                                scalar1=sT[rows, ft : ft + 1],
                                scalar2=bT[rows, ft : ft + 1],
                                op0=OP.mult,
                                op1=OP.add,
                            )
                    else:
                        nc.any.tensor_scalar(
                            out=dst[ft],
                            in0=ps,
                            scalar1=sT[:, ft : ft + 1],
                            scalar2=bT[:, ft : ft + 1],
                            op0=OP.mult,
                            op1=OP.add,
                        )

            def v_sweep(t2):
                """V in natural layout: h stationary, vw moving."""
                for tb in range(QT):
                    ps = qkv_psum.tile(
                        [P, HEADS, DH], F32, tag="vps", name=f"vps{t2}_{tb}"
                    )
                    for cp in range(CP):
                        for blk in range(2):
                            nc.tensor.matmul(
                                ps[:, blk * HB : (blk + 1) * HB, :],
                                lhsT=hT2[(cp, t2)][
                                    :, :, tb * P : (tb + 1) * P
                                ],
                                rhs=vwslab[
                                    :,
                                    2 * cp : 2 * cp + 2,
                                    blk * 512 : (blk + 1) * 512,
                                ],
                                start=(cp == 0),
                                stop=(cp == CP - 1),
                                perf_mode=DROW,
                            )
                    nc.any.tensor_tensor(
                        vh[t2][:, tb, :, :DH],
                        ps,
                        vg_bc[:, :].rearrange("p (h d) -> p h d", h=HEADS),
                        OP.mult,
                    )

            kq_sweep(0, kwslab, kbT, ksT, [kT[(f, 0)] for f in range(CT)])
            v_sweep(0)
            kq_sweep(0, qwslab, qbT, qsT, qT, per_head=True)
            kq_sweep(1, kwslab, kbT, ksT, [kT[(f, 1)] for f in range(CT)])
            v_sweep(1)

        # ================= Phase 3: attention =================
        with (
            tc.tile_pool(name="attn", bufs=3) as attn_pool,
            tc.tile_pool(name="attn_st", bufs=2, space="PSUM") as attn_st,
            tc.tile_pool(name="attn_ot", bufs=2, space="PSUM") as attn_ot,
        ):
            for h in range(HEADS):
                prow = (h % 2) * DH
                ftile = h // 2
                p_sb = attn_pool.tile([P, TT, NOWN], F8, tag="p_sb", name="p")
                for cp in range(CP):
                    st = attn_st.tile(
                        [P, 2, NOWN], F32, tag="st", name=f"st{h}_{cp}"
                    )
                    for j in range(2):
                        c = cp * 2 + j
                        # full-height lhsT: the other head's q rows are zero
                        kv_slice = kT[(ftile, c // QT)][
                            :, (c % QT) * P : (c % QT + 1) * P
                        ]
                        nc.tensor.matmul(
                            st[:, j, :],
                            lhsT=kv_slice,
                            rhs=qT[h],
                            start=True,
                            stop=True,
                        )
                    # p = exp(SCALE * s)   (fp8 out)
                    nc.scalar.activation(
                        out=p_sb[:, 2 * cp : 2 * cp + 2, :],
                        in_=st,
                        func=AF.Exp,
                        scale=SCALE,
                    )
                ot = attn_ot.tile([P, NOWN], F32, tag="ot", name="ot")
                for cp in range(CP):
                    t2, c2 = cp // 2, cp % 2
                    nc.tensor.matmul(
                        ot,
                        lhsT=vh[t2][:, 2 * c2 : 2 * c2 + 2, h, :],
                        rhs=p_sb[:, 2 * cp : 2 * cp + 2, :],
                        start=(cp == 0),
                        stop=(cp == CP - 1),
                        perf_mode=DROW,
                    )
                # softmax denominators arrive in row DH (ones column of vh).
                # Stage PSUM->SBUF first: reciprocal_approx_fast misreads
                # PSUM operands on HW (measured 0.38 rel err) but is exact
                # from SBUF, and ~5x faster than the exact DVE reciprocal.
                den = attn_pool.tile([1, NOWN], F32, tag="den", name="den")
                nc.any.tensor_copy(out=den, in_=ot[DH : DH + 1, :])
                rs = attn_pool.tile([1, NOWN], F32, tag="rs", name="rs")
                nc.vector.reciprocal_approx_fast(out=rs, in_=den)
                rsb = attn_pool.tile([DH, NOWN], F32, tag="rsb", name="rsb")
                nc.gpsimd.partition_broadcast(rsb, rs)
                nc.any.tensor_tensor(
                    oT2[ftile // 2][prow : prow + DH, ftile % 2, :],
                    ot[:DH, :],
                    rsb,
                    OP.mult,
                )

        # ================= Phase 4: proj + residual -> x2 (in place) ========
        # fold pb into x_own first (x_own already consumed by LN1; tile deps
        # order this correctly)
        for tq in range(QT):
            nc.any.tensor_tensor(x_own[tq], x_own[tq], pb_bc, OP.add)
        # tq-outer so x2[tq] completes both halves early -> LN2 tile tq can
        # start while proj continues on later tq
        with tc.tile_pool(name="proj_ps", bufs=4, space="PSUM") as proj_ps:
            for tq in range(QT):
                for ns in range(2):
                    nsl = slice(ns * 512, (ns + 1) * 512)
                    ps = proj_ps.tile([P, 512], F32, tag="pps", name="pps")
                    for fp in range(CP):
                        nc.tensor.matmul(
                            ps,
                            lhsT=oT2[fp][:, :, tq * P : (tq + 1) * P],
                            rhs=pslab[:, 2 * fp : 2 * fp + 2, nsl],
                            start=(fp == 0),
                            stop=(fp == CP - 1),
                            perf_mode=DROW,
                        )
                    nc.any.tensor_tensor(ps, ps, psinv_bc[:, nsl], OP.mult)
                    nc.any.tensor_tensor(
                        x2[tq][:, nsl], ps, x_own[tq][:, nsl], OP.add
                    )

        # ================= Phase 5: LN2 -> h2T2 =================
        with tc.tile_pool(name="ln2", bufs=2) as ln2:
            for t in range(QT):
                h = layernorm_tile(ln2, x2[t])
                ps = psum_wide.tile([P, C], BF16, tag="tr", name=f"tr2{t}")
                for ft in range(CT):
                    nc.tensor.transpose(
                        ps[:, ft * P : (ft + 1) * P],
                        h[:, ft * P : (ft + 1) * P],
                        ident,
                    )
                for cp in range(CP):
                    nc.any.tensor_copy(
                        out=h2T2[cp][:, :, t * P : (t + 1) * P],
                        in_=ps[:, cp * 2 * P : (cp + 1) * 2 * P].rearrange(
                            "p (two f) -> p two f", two=2
                        ),
                    )

        # fold the fc2 bias into the residual once LN2 has consumed x2
        # (shortens the fc2 output chain to mult+add)
        for tq in range(QT):
            nc.any.tensor_tensor(x2[tq], x2[tq], f2b_bc, OP.add)

        # ================= Phase 6: FC1 + gelu -> actT2 =================
        with (
            tc.tile_pool(name="f1c", bufs=5) as f1c,
            tc.tile_pool(name="f1_ps", bufs=4, space="PSUM") as f1_ps,
        ):
            for g in range(HT // 2):
                slab = f1c.tile([P, 2, CT, P], mf1, tag="f1w", name="f1slab")
                nc.sync.dma_start(slab, io["f1w"][:, 2 * g : 2 * g + 2])
                for j in range(2):
                    hf = 2 * g + j
                    ps = f1_ps.tile([P, NOWN], F32, tag="f1ps", name="f1ps")
                    if FC1_FP8:
                        for cp in range(CP):
                            nc.tensor.matmul(
                                ps,
                                lhsT=slab[:, j, 2 * cp : 2 * cp + 2, :],
                                rhs=h2T2[cp],
                                start=(cp == 0),
                                stop=(cp == CP - 1),
                                perf_mode=DROW,
                            )
                    else:
                        for c in range(CT):
                            nc.tensor.matmul(
                                ps,
                                lhsT=slab[:, j, c, :],
                                rhs=h2T2[c // 2][:, c % 2, :],
                                start=(c == 0),
                                stop=(c == CT - 1),
                            )
                    # gelu(ps * s + b), fused dequant+bias via activation
                    nc.scalar.activation(
                        out=actT2[hf // 2][:, hf % 2, :],
                        in_=ps,
                        func=AF.Gelu,
                        bias=f1bT[:, hf : hf + 1],
                        scale=f1sT[:, hf : hf + 1],
                    )

        # ================= Phase 7: FC2 + residual -> out =================
        # split-K: hidden tiles [0, 2*F2SPLIT) in fp8 DoubleRow, rest bf16
        NG = 4  # hidden-tile groups per DMA chunk
        N8G = 2 * F2SPLIT // NG  # fp8 groups
        NBG = (HT - 2 * F2SPLIT) // NG  # bf16 groups
        with (
            tc.tile_pool(name="f2c", bufs=3) as f2c,
            tc.tile_pool(name="f2_ps", bufs=1, space="PSUM") as f2_ps,
            tc.tile_pool(name="out_sb", bufs=2) as out_pool,
        ):
            for ns in range(2):
                nsl = slice(ns * 512, (ns + 1) * 512)
                pss = [
                    f2_ps.tile([P, 512], F32, tag=f"f2ps{tq}", name=f"f2ps{tq}")
                    for tq in range(QT)
                ]
                for g in range(N8G):
                    gw = f2c.tile([P, NG, 512], F8, tag="f2w8", name=f"f2w8g{g}")
                    nc.sync.dma_start(
                        gw, io["f2w8"][:, ns, g * NG : (g + 1) * NG, :]
                    )
                    for tq in range(QT):
                        for i in range(NG // 2):
                            hp = (g * NG) // 2 + i
                            nc.tensor.matmul(
                                pss[tq],
                                lhsT=actT2[hp][:, :, tq * P : (tq + 1) * P],
                                rhs=gw[:, 2 * i : 2 * i + 2, :],
                                start=(g == 0 and i == 0),
                                stop=False,
                                perf_mode=DROW,
                            )
                for g in range(NBG):
                    gw = f2c.tile(
                        [P, NG, 512], BF16, tag="f2wb", name=f"f2wbg{g}"
                    )
                    nc.sync.dma_start(
                        gw, io["f2wb"][:, ns, g * NG : (g + 1) * NG, :]
                    )
                    for tq in range(QT):
                        for i in range(NG):
                            hc = 2 * F2SPLIT + g * NG + i
                            nc.tensor.matmul(
                                pss[tq],
                                lhsT=actT2[hc // 2][
                                    :, hc % 2, tq * P : (tq + 1) * P
                                ],
                                rhs=gw[:, i, :],
                                start=False,
                                stop=(g == NBG - 1 and i == NG - 1),
                            )
                for tq in range(QT):
                    ot2 = out_pool.tile([P, 512], F32, tag="out_t", name="o")
                    nc.any.tensor_tensor(
                        ot2, pss[tq], f2sinv_bc[:, nsl], OP.mult
                    )
                    nc.any.tensor_tensor(ot2, ot2, x2[tq][:, nsl], OP.add)
                    nc.sync.dma_start(out_d[tq * P : (tq + 1) * P, nsl], ot2)


_PROGRAM = None


def _get_program():
    global _PROGRAM
    if _PROGRAM is None:
        _PROGRAM = build_program()
    return _PROGRAM


def _quant_cols(w, dtype):
    """per-output-column absmax quantization; returns (w_q, dequant_scales)"""
    import ml_dtypes

    w = np.asarray(w, np.float64)
    if dtype == "fp8":
        amax = np.abs(w).max(axis=0)
        amax = np.where(amax == 0, 1.0, amax)
        s = E4M3_MAX / amax
        wq = np.clip(w * s, -E4M3_MAX, E4M3_MAX).astype(ml_dtypes.float8_e4m3)
        return wq, (1.0 / s).astype(np.float32)
    else:
        wq = w.astype(ml_dtypes.bfloat16)
        return wq, np.ones(w.shape[1], np.float32)


def build_in_maps(inputs):
    import ml_dtypes

    x = np.asarray(inputs["x"], np.float32)  # [4, 1024, 1024]
    ln1_g = np.asarray(inputs["ln1_g"], np.float64)
    ln1_b = np.asarray(inputs["ln1_b"], np.float64)
    ln2_g = np.asarray(inputs["ln2_g"], np.float64)
    ln2_b = np.asarray(inputs["ln2_b"], np.float64)
    qkv_w = np.asarray(inputs["qkv_w"], np.float64)
    qkv_b = np.asarray(inputs["qkv_b"], np.float64)
    proj_w = np.asarray(inputs["proj_w"], np.float64)
    proj_b = np.asarray(inputs["proj_b"], np.float64)
    fc1_w = np.asarray(inputs["fc1_w"], np.float64)
    fc1_b = np.asarray(inputs["fc1_b"], np.float64)
    fc2_w = np.asarray(inputs["fc2_w"], np.float64)
    fc2_b = np.asarray(inputs["fc2_b"], np.float64)

    # Fold LN affine into the following matmul:
    #   (xhat*g + b) @ W == xhat @ (diag(g) W) + b @ W
    qkv_w_f = ln1_g[:, None] * qkv_w
    qkv_b_f = qkv_b + ln1_b @ qkv_w
    f1w_f = ln2_g[:, None] * fc1_w
    f1b_f = fc1_b + ln2_b @ fc1_w

    qw = qkv_w_f[:, :C]
    kw = qkv_w_f[:, C : 2 * C]
    vw = qkv_w_f[:, 2 * C :]
    vb = qkv_b_f[2 * C :]

    # --- Q/K: per-column fp8 quant, dequant scale applied on chip ---
    qw8, qsinv = _quant_cols(qw, "fp8")
    kw8, ksinv = _quant_cols(kw, "fp8")

    # --- V: per-column fp8 quant; on-chip the psum is rescaled by vg so the
    # fp8 V tile holds v*t with t = 24/||vw_col||; t and the v bias both fold
    # into the proj weights/bias ---
    vw8, vsinv = _quant_cols(vw, "fp8")
    vnorm = np.linalg.norm(vw, axis=0)
    vnorm = np.where(vnorm == 0, 1.0, vnorm)
    t_v = 24.0 / vnorm
    vg = (vsinv * t_v).astype(np.float32)  # psum -> fp8 V scaling

    # --- proj: fold t_v and v bias; per-column fp8 quant ---
    pw_eff = proj_w / t_v[:, None]
    pb_eff = proj_b + vb @ proj_w
    pw8, psinv = _quant_cols(pw_eff, "fp8")

    # --- fc1 ---
    f1w8, f1sinv = _quant_cols(f1w_f, "fp8" if FC1_FP8 else "bf16")

    # --- fc2 split-K: rows [0, 256*F2SPLIT) fp8 (col-scaled), rest bf16
    # pre-scaled by the same column scales so one dequant applies to both ---
    k8 = 2 * F2SPLIT * P
    amax = np.abs(fc2_w[:k8]).max(axis=0)
    amax = np.where(amax == 0, 1.0, amax)
    s2 = E4M3_MAX / amax
    f2hi = np.clip(fc2_w[:k8] * s2, -E4M3_MAX, E4M3_MAX).astype(
        ml_dtypes.float8_e4m3
    )
    f2lo = (fc2_w[k8:] * s2).astype(ml_dtypes.bfloat16)
    f2sinv = (1.0 / s2).astype(np.float32)

    # --- permute weights into SBUF slab layouts ---
    # q/k: [p, ft, c, f] from w[c*128+p, ft*128+f]
    def perm_kq(w8):
        return np.ascontiguousarray(
            w8.reshape(CT, P, CT, P).transpose(1, 2, 0, 3)
        )

    # v/proj: [p, c, n] from w[c*128+p, n]
    def perm_cn(w8):
        return np.ascontiguousarray(w8.reshape(CT, P, C).transpose(1, 0, 2))

    # fc1: [p, hf, c, f] from w[c*128+p, hf*128+f]
    f1wP = np.ascontiguousarray(
        f1w8.reshape(CT, P, HT, P).transpose(1, 2, 0, 3)
    )
    # fc2: [p, ns, hc, n] from w[hc*128+p, ns*512+n]
    f2wP8 = np.ascontiguousarray(
        f2hi.reshape(2 * F2SPLIT, P, 2, NOWN).transpose(1, 2, 0, 3)
    )
    f2wPb = np.ascontiguousarray(
        f2lo.reshape(HT - 2 * F2SPLIT, P, 2, NOWN).transpose(1, 2, 0, 3)
    )

    def tbias(b):  # [n*128] -> [128, n] per-partition layout
        return np.ascontiguousarray(
            np.asarray(b, np.float32).reshape(-1, P).T
        )

    common = dict(
        qw=perm_kq(qw8),
        kw=perm_kq(kw8),
        vw=perm_cn(vw8),
        pw=perm_cn(pw8),
        f1w=f1wP,
        f2w8=f2wP8,
        f2wb=f2wPb,
        qbt=tbias(qkv_b_f[:C]),
        kbt=tbias(qkv_b_f[C : 2 * C]),
        qst=tbias(qsinv),
        kst=tbias(ksinv),
        f1bt=tbias(f1b_f),
        f1st=tbias(f1sinv),
        vg=vg,
        psinv=psinv.astype(np.float32),
        pb=pb_eff.astype(np.float32),
        f2sinv=f2sinv.astype(np.float32),
        f2b=fc2_b.astype(np.float32),
    )
    in_maps = []
    for core in range(8):
        b, half = core // 2, core % 2
        own = x[b, half * NOWN : (half + 1) * NOWN, :]
        other = x[b, (1 - half) * NOWN : (2 - half) * NOWN, :]
        xp = np.ascontiguousarray(np.concatenate([own, other], axis=0))
        in_maps.append({**common, "x": xp})
    return in_maps


def kernel(**inputs):
    in_maps = build_in_maps(inputs)
    nc = _get_program()
    res = run_bass_kernel_spmd(nc, in_maps, core_ids=list(range(8)))
    outs = res.results

    y = np.empty((4, NTOK, C), np.float32)
    for core in range(8):
        b, half = core // 2, core % 2
        y[b, half * NOWN : (half + 1) * NOWN, :] = outs[core]["out"]
    return y


if __name__ == "__main__":
    prog = build_program()
    print("program built OK")


# revision 54
# speedup vs baseline: 1.3157x; 1.0087x over previous
"""Trainium2 Bass kernel for a dense transformer block (pre-LN, MHA + MLP).

Full inputs in, full outputs out. Sharding: 8 cores = (batch, seq-half).
Each core computes K/V over its batch element's full 1024 tokens and
Q/attention/MLP over its own 512 tokens (host permutes tokens so the core's
own half is always rows 0..511 — softmax over keys is permutation invariant).
No collectives needed.

v2: fp8 (e4m3) DoubleRow matmuls for QKV / AV / proj (and optionally
fc1/fc2), which stream 2 contraction rows per PE pass. Weights are
quantized per-output-column on the host (absmax -> +-240); dequant scales
fold into the existing bias-add / activation ops, or (for V) into the proj
weights themselves. V is computed in natural [token, feat] layout directly
(h stationary, weights moving), eliminating the separate V transpose pass.
The softmax denominator comes from a constant ones-column appended to V.

Host-side preprocessing folds LayerNorm affine params into the following
matmul weights:  (xhat*g + b) @ W == xhat @ (diag(g) W) + b @ W, and the
V bias into the proj bias: (o/d + vb) @ pw + pb == (o/d) @ pw + (vb@pw + pb).
"""

import sys

sys.path.insert(0, "/opt/trn_rl_repo")

import numpy as np

import concourse.bass as bass
import concourse.bacc as bacc
import concourse.mybir as mybir
import concourse.tile as tile
from concourse.bass_utils import run_bass_kernel_spmd
from concourse.masks import make_identity

P = 128
C = 1024
HEADS = 16
DH = 64
HID = 4096
NTOK = 1024  # tokens per batch element (kv length)
NOWN = 512  # tokens owned by this core (q length)
SCALE = DH ** -0.5
EPS = 1e-5

F32 = mybir.dt.float32
F32R = mybir.dt.float32r
BF16 = mybir.dt.bfloat16
F8 = mybir.dt.float8e4
AF = mybir.ActivationFunctionType
OP = mybir.AluOpType
DROW = mybir.MatmulPerfMode.DoubleRow

CT = C // P  # 8 column tiles of the model dim
CP = CT // 2  # 4 column-tile pairs
TT = NTOK // P  # 8 token tiles (kv)
QT = NOWN // P  # 4 token tiles (own)
HT = HID // P  # 32 hidden tiles
HP = HT // 2  # 16 hidden-tile pairs

VPAD = 128  # per-head padded width of the V tile (DH + ones col + pad);
# padded to 128 so the AV matmul streams full-height (65-row outputs
# measured ~43% slower per instruction)

E4M3_MAX = 240.0

# --- dtype config for the two MLP GEMMs (attention GEMMs are always fp8;
# the error sim shows attention fp8 contributes ~nothing to final error) ---
FC1_FP8 = False
# fc2 split-K: hidden-tile pairs [0, F2SPLIT) run fp8 DoubleRow, the rest
# bf16. Error sim: F2SPLIT=8 -> 1.31e-2 total (gate 2e-2).
F2SPLIT = 8


def build_program():
    nc = bacc.Bacc("TRN2", target_bir_lowering=False)
    mf1 = F8 if FC1_FP8 else BF16

    io = {}
    io["x"] = nc.dram_tensor("x", (NTOK, C), F32, kind="ExternalInput")
    # pre-permuted weights (host layout matches SBUF slabs)
    io["qw"] = nc.dram_tensor("qw", (P, CT, CT, P), F8, kind="ExternalInput")
    io["kw"] = nc.dram_tensor("kw", (P, CT, CT, P), F8, kind="ExternalInput")
    io["vw"] = nc.dram_tensor("vw", (P, CT, C), F8, kind="ExternalInput")
    io["pw"] = nc.dram_tensor("pw", (P, CT, C), F8, kind="ExternalInput")
    io["f1w"] = nc.dram_tensor("f1w", (P, HT, CT, P), mf1, kind="ExternalInput")
    io["f2w8"] = nc.dram_tensor(
        "f2w8", (P, 2, 2 * F2SPLIT, NOWN), F8, kind="ExternalInput"
    )
    io["f2wb"] = nc.dram_tensor(
        "f2wb", (P, 2, HT - 2 * F2SPLIT, NOWN), BF16, kind="ExternalInput"
    )
    # per-partition bias/scale tables, [128, n] layouts
    io["qbt"] = nc.dram_tensor("qbt", (P, CT), F32, kind="ExternalInput")
    io["kbt"] = nc.dram_tensor("kbt", (P, CT), F32, kind="ExternalInput")
    io["qst"] = nc.dram_tensor("qst", (P, CT), F32, kind="ExternalInput")
    io["kst"] = nc.dram_tensor("kst", (P, CT), F32, kind="ExternalInput")
    io["f1bt"] = nc.dram_tensor("f1bt", (P, HT), F32, kind="ExternalInput")
    io["f1st"] = nc.dram_tensor("f1st", (P, HT), F32, kind="ExternalInput")
    # free-dim vectors (broadcast across partitions on chip)
    io["vg"] = nc.dram_tensor("vg", (C,), F32, kind="ExternalInput")
    io["psinv"] = nc.dram_tensor("psinv", (C,), F32, kind="ExternalInput")
    io["pb"] = nc.dram_tensor("pb", (C,), F32, kind="ExternalInput")
    io["f2sinv"] = nc.dram_tensor("f2sinv", (C,), F32, kind="ExternalInput")
    io["f2b"] = nc.dram_tensor("f2b", (C,), F32, kind="ExternalInput")
    io["out"] = nc.dram_tensor("out", (NOWN, C), F32, kind="ExternalOutput")

    with tile.TileContext(nc) as tc:
        _emit(nc, tc, io)
    nc.compile()
    return nc


def _emit(nc, tc, io):
    x_d, out_d = io["x"], io["out"]
    mf1 = F8 if FC1_FP8 else BF16

    with (
        tc.tile_pool(name="consts", bufs=1) as consts,
        tc.tile_pool(name="persist", bufs=1) as persist,
        tc.tile_pool(name="big", bufs=1) as big,
        tc.tile_pool(name="psum_wide", bufs=2, space="PSUM") as psum_wide,
    ):
        # ---- constants (unique tags: each gets its own persistent slot) ----
        ident_f32 = consts.tile([P, P], F32, tag="idf")
        make_identity(nc, ident_f32)
        ident = consts.tile([P, P], BF16, tag="idr")
        nc.vector.tensor_copy(out=ident, in_=ident_f32)
        eps_tile = consts.tile([P, 1], F32, tag="eps")
        nc.vector.memset(eps_tile, EPS)
        qbT = consts.tile([P, CT], F32, tag="qbT")
        nc.sync.dma_start(qbT, io["qbt"][:, :])
        kbT = consts.tile([P, CT], F32, tag="kbT")
        nc.sync.dma_start(kbT, io["kbt"][:, :])
        qsT = consts.tile([P, CT], F32, tag="qsT")
        nc.sync.dma_start(qsT, io["qst"][:, :])
        ksT = consts.tile([P, CT], F32, tag="ksT")
        nc.sync.dma_start(ksT, io["kst"][:, :])
        f1bT = consts.tile([P, HT], F32, tag="f1bT")
        nc.sync.dma_start(f1bT, io["f1bt"][:, :])
        f1sT = consts.tile([P, HT], F32, tag="f1sT")
        nc.sync.dma_start(f1sT, io["f1st"][:, :])

        def bcast_const(src_d, n, tag):
            t = consts.tile([P, n], F32, tag=tag, name=tag)
            src = bass.AP(tensor=src_d, offset=0, ap=[[0, P], [1, n]])
            nc.sync.dma_start(t, src)
            return t

        vg_bc = bcast_const(io["vg"], C, "vg")
        psinv_bc = bcast_const(io["psinv"], C, "psv")
        pb_bc = bcast_const(io["pb"], C, "pbb")
        f2sinv_bc = bcast_const(io["f2sinv"], C, "f2s")
        f2b_bc = bcast_const(io["f2b"], C, "f2bb")

        # own x tiles (fp32, kept for the residual), one tile per token tile;
        # proj writes x2 = x + pb + proj_out back IN PLACE (saves SBUF)
        x_own = []
        for t in range(QT):
            xo = persist.tile([P, C], F32, tag=f"xo{t}", name=f"xo{t}")
            nc.sync.dma_start(xo, x_d[t * P : (t + 1) * P, :])
            x_own.append(xo)
        x2 = x_own

        # persistent weight slabs (single DMA each, reused across sweeps)
        kwslab = persist.tile([P, CT, CT, P], F8, tag="kws", name="kws")
        nc.sync.dma_start(kwslab, io["kw"][:, :, :, :])
        qwslab = persist.tile([P, CT, CT, P], F8, tag="qws", name="qws")
        nc.sync.dma_start(qwslab, io["qw"][:, :, :, :])
        vwslab = persist.tile([P, CT, C], F8, tag="vws", name="vws")
        nc.sync.dma_start(vwslab, io["vw"][:, :, :])
        pslab = persist.tile([P, CT, C], F8, tag="pws", name="pws")
        nc.sync.dma_start(pslab, io["pw"][:, :, :])

        def layernorm_tile(temps, xt):
            """xt: [128, C] fp32 -> returns normalized f32r tile [128, C]."""
            stats = temps.tile([P, 2, 6], F32, tag="ln_stats", name="st")
            for sg in range(2):
                nc.vector.bn_stats(
                    out=stats[:, sg, :], in_=xt[:, sg * 512 : (sg + 1) * 512]
                )
            mv = temps.tile([P, 2], F32, tag="ln_mv", name="mv")
            nc.vector.bn_aggr(out=mv[:], in_=stats[:])
            # (ln/exp-based rsqrt thrashes the Act tables against the
            # attention exp — 1283ns per reload; Sqrt + [P,1] DVE reciprocal
            # is cheap, the reciprocal runs 1 elem/lane on 128 lanes)
            rstd = temps.tile([P, 1], F32, tag="ln_rstd", name="rstd")
            nc.scalar.activation(
                out=rstd, in_=mv[:, 1:2], func=AF.Sqrt, bias=eps_tile, scale=1.0
            )
            # approx reciprocal is ~5x faster; SBUF input (incl. in-place) is
            # exact to ~3e-6 on HW — only PSUM inputs misread (measured)
            nc.vector.reciprocal_approx_fast(out=rstd, in_=rstd)
            nmr = temps.tile([P, 1], F32, tag="ln_nmr", name="nmr")
            nc.vector.tensor_tensor(nmr, mv[:, 0:1], rstd, OP.mult)
            nc.vector.tensor_scalar_mul(nmr, nmr, -1.0)
            # bf16 h: the PE transposes stream 1.0 c/row for bf16 vs 1.5 for
            # f32r, and the transpose PSUM halves to one bank
            h = temps.tile([P, C], BF16, tag="ln_h", name="h")
            nc.any.tensor_scalar(
                out=h,
                in0=xt,
                scalar1=rstd,
                scalar2=nmr,
                op0=OP.mult,
                op1=OP.add,
            )
            return h

        # ---- persistent activation tiles ----
        # hT2[(cp, t2)]: [P, 2, 512] fp8 — transposed LN1 output, c-tile pairs
        hT2 = {
            (cp, t2): big.tile(
                [P, 2, NOWN], F8, tag=f"hT{cp}_{t2}", name=f"hT{cp}_{t2}"
            )
            for cp in range(CP)
            for t2 in range(2)
        }
        # kT[(ft, t2)]: [P, 512] bf16 (QK stays bf16)
        kT = {
            (ft, t2): big.tile(
                [P, NOWN], BF16, tag=f"kT{ft}_{t2}", name=f"kT{ft}_{t2}"
            )
            for ft in range(CT)
            for t2 in range(2)
        }
        # qT: one zero-padded tile per head (own head's 64 rows at its
        # natural partition offset, other head's rows zero). QK can then run
        # full-height 128-contraction matmuls — measured ~35% faster than
        # the 64-row form — with kT packed as-is.
        qT = [
            big.tile([P, NOWN], BF16, tag=f"qT{hq}", name=f"qT{hq}")
            for hq in range(HEADS)
        ]
        for hq in range(HEADS):
            nc.vector.memset(qT[hq][:], 0.0)
        # vh[q]: [P, 2, HEADS, VPAD] fp8 — V in natural token layout, one
        # tile per k-token-tile PAIR (matches the AV DoubleRow pair slice, so
        # each AV matmul depends on just its quarter of V, not all of it);
        # col DH holds 1.0 (softmax denominator trick)
        vh = [
            big.tile([P, 2, HEADS, VPAD], F8, tag=f"vh{q}", name=f"vh{q}")
            for q in range(QT)
        ]
        # oT2[fp]: [P, 2, 512] fp8 — attention output, feature-tile pairs
        oT2 = [
            big.tile([P, 2, NOWN], F8, tag=f"oT{fp}", name=f"oT{fp}")
            for fp in range(CP)
        ]
        h2T2 = [
            big.tile([P, 2, NOWN], mf1, tag=f"h2T{cp}", name=f"h2T{cp}")
            for cp in range(CP)
        ]
        actT2 = [
            big.tile(
                [P, 2, NOWN],
                F8 if hp < F2SPLIT else BF16,
                tag=f"aT{hp}",
                name=f"aT{hp}",
            )
            for hp in range(HP)
        ]

        for q in range(QT):
            nc.vector.memset(vh[q][:], 0.0)
            nc.vector.memset(vh[q][:, :, :, DH : DH + 1], 1.0)

        # ================= Phase 1: LN1 -> hT2 =================
        with (
            tc.tile_pool(name="ln1", bufs=2) as ln1,
            tc.tile_pool(name="xtmp", bufs=2) as xtmp,
        ):
            for t in range(TT):
                if t < QT:
                    xt = x_own[t]
                else:
                    xt = xtmp.tile([P, C], F32, tag="xt", name="xt")
                    nc.sync.dma_start(xt, x_d[t * P : (t + 1) * P, :])
                h = layernorm_tile(ln1, xt)
                t2, tb = t // QT, t % QT
                ps = psum_wide.tile([P, C], BF16, tag="tr", name=f"trp{t}")
                for ft in range(CT):
                    nc.tensor.transpose(
                        ps[:, ft * P : (ft + 1) * P],
                        h[:, ft * P : (ft + 1) * P],
                        ident,
                    )
                for cp in range(CP):
                    nc.any.tensor_copy(
                        out=hT2[(cp, t2)][:, :, tb * P : (tb + 1) * P],
                        in_=ps[:, cp * 2 * P : (cp + 1) * 2 * P].rearrange(
                            "p (two f) -> p two f", two=2
                        ),
                    )

        # ================= Phase 2: QKV =================
        HB = HEADS // 2  # heads per 512-wide V block
        with tc.tile_pool(name="qkv_psum", bufs=2, space="PSUM") as qkv_psum:

            def kq_sweep(t2, wslab, bT, sT, dst, per_head=False):
                """K or Q: transposed-output sweep; dst[ft] <- [P,512] bf16"""
                for ft in range(CT):
                    ps = qkv_psum.tile([P, NOWN], F32, tag="kvps", name="kvps")
                    for cp in range(CP):
                        nc.tensor.matmul(
                            ps,
                            lhsT=wslab[:, ft, 2 * cp : 2 * cp + 2, :],
                            rhs=hT2[(cp, t2)],
                            start=(cp == 0),
                            stop=(cp == CP - 1),
                            perf_mode=DROW,
                        )
                    if per_head:
                        # split into the two heads' zero-padded tiles,
                        # partition-aligned (head j keeps rows j*64..)
                        for j in range(2):
                            rows = slice(j * DH, (j + 1) * DH)
                            nc.any.tensor_scalar(
                                out=dst[2 * ft + j][rows, :],
                                in0=ps[rows, :],
                                scalar1=sT[rows, ft : ft + 1],
                                scalar2=bT[rows, ft : ft + 1],
                                op0=OP.mult,
                                op1=OP.add,
                            )
                    else:
                        nc.any.tensor_scalar(
                            out=dst[ft],
                            in0=ps,
                            scalar1=sT[:, ft : ft + 1],
                            scalar2=bT[:, ft : ft + 1],
                            op0=OP.mult,
                            op1=OP.add,
                        )

            def v_sweep(t2):
                """V in natural layout: h stationary, vw moving."""
                for tb in range(QT):
                    ps = qkv_psum.tile(
                        [P, HEADS, DH], F32, tag="vps", name=f"vps{t2}_{tb}"
                    )
                    for cp in range(CP):
                        for blk in range(2):
                            nc.tensor.matmul(
                                ps[:, blk * HB : (blk + 1) * HB, :],
                                lhsT=hT2[(cp, t2)][
                                    :, :, tb * P : (tb + 1) * P
                                ],
                                rhs=vwslab[
                                    :,
                                    2 * cp : 2 * cp + 2,
                                    blk * 512 : (blk + 1) * 512,
                                ],
                                start=(cp == 0),
                                stop=(cp == CP - 1),
                                perf_mode=DROW,
                            )
                    nc.any.tensor_tensor(
                        vh[t2 * 2 + tb // 2][:, tb % 2, :, :DH],
                        ps,
                        vg_bc[:, :].rearrange("p (h d) -> p h d", h=HEADS),
                        OP.mult,
                    )

            kq_sweep(0, kwslab, kbT, ksT, [kT[(f, 0)] for f in range(CT)])
            v_sweep(0)
            kq_sweep(0, qwslab, qbT, qsT, qT, per_head=True)
            kq_sweep(1, kwslab, kbT, ksT, [kT[(f, 1)] for f in range(CT)])
            v_sweep(1)

        # ================= Phase 3: attention =================
        with (
            tc.tile_pool(name="attn", bufs=3) as attn_pool,
            tc.tile_pool(name="attn_st", bufs=2, space="PSUM") as attn_st,
            tc.tile_pool(name="attn_ot", bufs=2, space="PSUM") as attn_ot,
        ):
            for h in range(HEADS):
                prow = (h % 2) * DH
                ftile = h // 2
                p_sb = attn_pool.tile([P, TT, NOWN], F8, tag="p_sb", name="p")
                for cp in range(CP):
                    st = attn_st.tile(
                        [P, 2, NOWN], F32, tag="st", name=f"st{h}_{cp}"
                    )
                    for j in range(2):
                        c = cp * 2 + j
                        # full-height lhsT: the other head's q rows are zero
                        kv_slice = kT[(ftile, c // QT)][
                            :, (c % QT) * P : (c % QT + 1) * P
                        ]
                        nc.tensor.matmul(
                            st[:, j, :],
                            lhsT=kv_slice,
                            rhs=qT[h],
                            start=True,
                            stop=True,
                        )
                    # p = exp(SCALE * s)   (fp8 out)
                    nc.scalar.activation(
                        out=p_sb[:, 2 * cp : 2 * cp + 2, :],
                        in_=st,
                        func=AF.Exp,
                        scale=SCALE,
                    )
                ot = attn_ot.tile([P, NOWN], F32, tag="ot", name="ot")
                for cp in range(CP):
                    nc.tensor.matmul(
                        ot,
                        lhsT=vh[cp][:, :, h, :],
                        rhs=p_sb[:, 2 * cp : 2 * cp + 2, :],
                        start=(cp == 0),
                        stop=(cp == CP - 1),
                        perf_mode=DROW,
                    )
                # softmax denominators arrive in row DH (ones column of vh).
                # Stage PSUM->SBUF first: reciprocal_approx_fast misreads
                # PSUM operands on HW (measured 0.38 rel err) but is exact
                # from SBUF, and ~5x faster than the exact DVE reciprocal.
                den = attn_pool.tile([1, NOWN], F32, tag="den", name="den")
                nc.any.tensor_copy(out=den, in_=ot[DH : DH + 1, :])
                rs = attn_pool.tile([1, NOWN], F32, tag="rs", name="rs")
                nc.vector.reciprocal_approx_fast(out=rs, in_=den)
                rsb = attn_pool.tile([DH, NOWN], F32, tag="rsb", name="rsb")
                nc.gpsimd.partition_broadcast(rsb, rs)
                nc.any.tensor_tensor(
                    oT2[ftile // 2][prow : prow + DH, ftile % 2, :],
                    ot[:DH, :],
                    rsb,
                    OP.mult,
                )

        # ================= Phase 4: proj + residual -> x2 (in place) ========
        # fold pb into x_own first (x_own already consumed by LN1; tile deps
        # order this correctly)
        for tq in range(QT):
            nc.any.tensor_tensor(x_own[tq], x_own[tq], pb_bc, OP.add)
        # Two waves of 4 psum chains, matmuls emitted round-robin across
        # chains fp-major: the PE stream is in-order, so emitting one chain's
        # fp0..fp3 consecutively would let its fp3 (which needs the LAST
        # heads' oT2) block every later chain's independent fp0-2 matmuls.
        # tq-grouped waves also let LN2 of tq 0/1 overlap wave 2.
        with tc.tile_pool(name="proj_ps", bufs=1, space="PSUM") as proj_ps:
            for wave in range(2):
                chains = [
                    (tq, ns)
                    for tq in (2 * wave, 2 * wave + 1)
                    for ns in range(2)
                ]
                pss = {
                    c: proj_ps.tile(
                        [P, 512], F32, tag=f"pps{i}", name=f"pps{c[0]}_{c[1]}"
                    )
                    for i, c in enumerate(chains)
                }
                for fp in range(CP):
                    for tq, ns in chains:
                        nsl = slice(ns * 512, (ns + 1) * 512)
                        nc.tensor.matmul(
                            pss[(tq, ns)],
                            lhsT=oT2[fp][:, :, tq * P : (tq + 1) * P],
                            rhs=pslab[:, 2 * fp : 2 * fp + 2, nsl],
                            start=(fp == 0),
                            stop=(fp == CP - 1),
                            perf_mode=DROW,
                        )
                for tq, ns in chains:
                    nsl = slice(ns * 512, (ns + 1) * 512)
                    ps = pss[(tq, ns)]
                    nc.any.tensor_tensor(ps, ps, psinv_bc[:, nsl], OP.mult)
                    nc.any.tensor_tensor(
                        x2[tq][:, nsl], ps, x_own[tq][:, nsl], OP.add
                    )

        # ================= Phase 5: LN2 -> h2T2 =================
        with tc.tile_pool(name="ln2", bufs=2) as ln2:
            for t in range(QT):
                h = layernorm_tile(ln2, x2[t])
                ps = psum_wide.tile([P, C], BF16, tag="tr", name=f"tr2{t}")
                for ft in range(CT):
                    nc.tensor.transpose(
                        ps[:, ft * P : (ft + 1) * P],
                        h[:, ft * P : (ft + 1) * P],
                        ident,
                    )
                for cp in range(CP):
                    nc.any.tensor_copy(
                        out=h2T2[cp][:, :, t * P : (t + 1) * P],
                        in_=ps[:, cp * 2 * P : (cp + 1) * 2 * P].rearrange(
                            "p (two f) -> p two f", two=2
                        ),
                    )

        # fold the fc2 bias into the residual once LN2 has consumed x2
        # (shortens the fc2 output chain to mult+add)
        for tq in range(QT):
            nc.any.tensor_tensor(x2[tq], x2[tq], f2b_bc, OP.add)

        # ================= Phase 6: FC1 + gelu -> actT2 =================
        with (
            tc.tile_pool(name="f1c", bufs=7) as f1c,
            tc.tile_pool(name="f1_ps", bufs=4, space="PSUM") as f1_ps,
        ):
            for g in range(HT // 2):
                slab = f1c.tile([P, 2, CT, P], mf1, tag="f1w", name="f1slab")
                nc.sync.dma_start(slab, io["f1w"][:, 2 * g : 2 * g + 2])
                for j in range(2):
                    hf = 2 * g + j
                    ps = f1_ps.tile([P, NOWN], F32, tag="f1ps", name="f1ps")
                    if FC1_FP8:
                        for cp in range(CP):
                            nc.tensor.matmul(
                                ps,
                                lhsT=slab[:, j, 2 * cp : 2 * cp + 2, :],
                                rhs=h2T2[cp],
                                start=(cp == 0),
                                stop=(cp == CP - 1),
                                perf_mode=DROW,
                            )
                    else:
                        for c in range(CT):
                            nc.tensor.matmul(
                                ps,
                                lhsT=slab[:, j, c, :],
                                rhs=h2T2[c // 2][:, c % 2, :],
                                start=(c == 0),
                                stop=(c == CT - 1),
                            )
                    # gelu(ps * s + b), fused dequant+bias via activation
                    nc.scalar.activation(
                        out=actT2[hf // 2][:, hf % 2, :],
                        in_=ps,
                        func=AF.Gelu,
                        bias=f1bT[:, hf : hf + 1],
                        scale=f1sT[:, hf : hf + 1],
                    )

        # ================= Phase 7: FC2 + residual -> out =================
        # split-K: hidden tiles [0, 2*F2SPLIT) in fp8 DoubleRow, rest bf16
        NG = 4  # hidden-tile groups per DMA chunk
        N8G = 2 * F2SPLIT // NG  # fp8 groups
        NBG = (HT - 2 * F2SPLIT) // NG  # bf16 groups
        with (
            tc.tile_pool(name="f2c", bufs=3) as f2c,
            tc.tile_pool(name="f2_ps", bufs=1, space="PSUM") as f2_ps,
            tc.tile_pool(name="out_sb", bufs=2) as out_pool,
        ):
            for ns in range(2):
                nsl = slice(ns * 512, (ns + 1) * 512)
                pss = [
                    f2_ps.tile([P, 512], F32, tag=f"f2ps{tq}", name=f"f2ps{tq}")
                    for tq in range(QT)
                ]
                for g in range(N8G):
                    gw = f2c.tile([P, NG, 512], F8, tag="f2w8", name=f"f2w8g{g}")
                    nc.sync.dma_start(
                        gw, io["f2w8"][:, ns, g * NG : (g + 1) * NG, :]
                    )
                    for tq in range(QT):
                        for i in range(NG // 2):
                            hp = (g * NG) // 2 + i
                            nc.tensor.matmul(
                                pss[tq],
                                lhsT=actT2[hp][:, :, tq * P : (tq + 1) * P],
                                rhs=gw[:, 2 * i : 2 * i + 2, :],
                                start=(g == 0 and i == 0),
                                stop=False,
                                perf_mode=DROW,
                            )
                for g in range(NBG):
                    gw = f2c.tile(
                        [P, NG, 512], BF16, tag="f2wb", name=f"f2wbg{g}"
                    )
                    nc.sync.dma_start(
                        gw, io["f2wb"][:, ns, g * NG : (g + 1) * NG, :]
                    )
                    for tq in range(QT):
                        for i in range(NG):
                            hc = 2 * F2SPLIT + g * NG + i
                            nc.tensor.matmul(
                                pss[tq],
                                lhsT=actT2[hc // 2][
                                    :, hc % 2, tq * P : (tq + 1) * P
                                ],
                                rhs=gw[:, i, :],
                                start=False,
                                stop=(g == NBG - 1 and i == NG - 1),
                            )
                for tq in range(QT):
                    ot2 = out_pool.tile([P, 512], F32, tag="out_t", name="o")
                    nc.any.tensor_tensor(
                        ot2, pss[tq], f2sinv_bc[:, nsl], OP.mult
                    )
                    nc.any.tensor_tensor(ot2, ot2, x2[tq][:, nsl], OP.add)
                    nc.sync.dma_start(out_d[tq * P : (tq + 1) * P, nsl], ot2)


_PROGRAM = None


def _get_program():
    global _PROGRAM
    if _PROGRAM is None:
        _PROGRAM = build_program()
    return _PROGRAM


def _quant_cols(w, dtype):
    """per-output-column absmax quantization; returns (w_q, dequant_scales)"""
    import ml_dtypes

    w = np.asarray(w, np.float64)
    if dtype == "fp8":
        amax = np.abs(w).max(axis=0)
        amax = np.where(amax == 0, 1.0, amax)
        s = E4M3_MAX / amax
        wq = np.clip(w * s, -E4M3_MAX, E4M3_MAX).astype(ml_dtypes.float8_e4m3)
        return wq, (1.0 / s).astype(np.float32)
    else:
        wq = w.astype(ml_dtypes.bfloat16)
        return wq, np.ones(w.shape[1], np.float32)


def build_in_maps(inputs):
    import ml_dtypes

    x = np.asarray(inputs["x"], np.float32)  # [4, 1024, 1024]
    ln1_g = np.asarray(inputs["ln1_g"], np.float64)
    ln1_b = np.asarray(inputs["ln1_b"], np.float64)
    ln2_g = np.asarray(inputs["ln2_g"], np.float64)
    ln2_b = np.asarray(inputs["ln2_b"], np.float64)
    qkv_w = np.asarray(inputs["qkv_w"], np.float64)
    qkv_b = np.asarray(inputs["qkv_b"], np.float64)
    proj_w = np.asarray(inputs["proj_w"], np.float64)
    proj_b = np.asarray(inputs["proj_b"], np.float64)
    fc1_w = np.asarray(inputs["fc1_w"], np.float64)
    fc1_b = np.asarray(inputs["fc1_b"], np.float64)
    fc2_w = np.asarray(inputs["fc2_w"], np.float64)
    fc2_b = np.asarray(inputs["fc2_b"], np.float64)

    # Fold LN affine into the following matmul:
    #   (xhat*g + b) @ W == xhat @ (diag(g) W) + b @ W
    qkv_w_f = ln1_g[:, None] * qkv_w
    qkv_b_f = qkv_b + ln1_b @ qkv_w
    f1w_f = ln2_g[:, None] * fc1_w
    f1b_f = fc1_b + ln2_b @ fc1_w

    qw = qkv_w_f[:, :C]
    kw = qkv_w_f[:, C : 2 * C]
    vw = qkv_w_f[:, 2 * C :]
    vb = qkv_b_f[2 * C :]

    # --- Q/K: per-column fp8 quant, dequant scale applied on chip ---
    qw8, qsinv = _quant_cols(qw, "fp8")
    kw8, ksinv = _quant_cols(kw, "fp8")

    # --- V: per-column fp8 quant; on-chip the psum is rescaled by vg so the
    # fp8 V tile holds v*t with t = 24/||vw_col||; t and the v bias both fold
    # into the proj weights/bias ---
    vw8, vsinv = _quant_cols(vw, "fp8")
    vnorm = np.linalg.norm(vw, axis=0)
    vnorm = np.where(vnorm == 0, 1.0, vnorm)
    t_v = 24.0 / vnorm
    vg = (vsinv * t_v).astype(np.float32)  # psum -> fp8 V scaling

    # --- proj: fold t_v and v bias; per-column fp8 quant ---
    pw_eff = proj_w / t_v[:, None]
    pb_eff = proj_b + vb @ proj_w
    pw8, psinv = _quant_cols(pw_eff, "fp8")

    # --- fc1 ---
    f1w8, f1sinv = _quant_cols(f1w_f, "fp8" if FC1_FP8 else "bf16")

    # --- fc2 split-K: rows [0, 256*F2SPLIT) fp8 (col-scaled), rest bf16
    # pre-scaled by the same column scales so one dequant applies to both ---
    k8 = 2 * F2SPLIT * P
    amax = np.abs(fc2_w[:k8]).max(axis=0)
    amax = np.where(amax == 0, 1.0, amax)
    s2 = E4M3_MAX / amax
    f2hi = np.clip(fc2_w[:k8] * s2, -E4M3_MAX, E4M3_MAX).astype(
        ml_dtypes.float8_e4m3
    )
    f2lo = (fc2_w[k8:] * s2).astype(ml_dtypes.bfloat16)
    f2sinv = (1.0 / s2).astype(np.float32)

    # --- permute weights into SBUF slab layouts ---
    # q/k: [p, ft, c, f] from w[c*128+p, ft*128+f]
    def perm_kq(w8):
        return np.ascontiguousarray(
            w8.reshape(CT, P, CT, P).transpose(1, 2, 0, 3)
        )

    # v/proj: [p, c, n] from w[c*128+p, n]
    def perm_cn(w8):
        return np.ascontiguousarray(w8.reshape(CT, P, C).transpose(1, 0, 2))

    # fc1: [p, hf, c, f] from w[c*128+p, hf*128+f]
    f1wP = np.ascontiguousarray(
        f1w8.reshape(CT, P, HT, P).transpose(1, 2, 0, 3)
    )
    # fc2: [p, ns, hc, n] from w[hc*128+p, ns*512+n]
    f2wP8 = np.ascontiguousarray(
        f2hi.reshape(2 * F2SPLIT, P, 2, NOWN).transpose(1, 2, 0, 3)
    )
    f2wPb = np.ascontiguousarray(
        f2lo.reshape(HT - 2 * F2SPLIT, P, 2, NOWN).transpose(1, 2, 0, 3)
    )

    def tbias(b):  # [n*128] -> [128, n] per-partition layout
        return np.ascontiguousarray(
            np.asarray(b, np.float32).reshape(-1, P).T
        )

    common = dict(
        qw=perm_kq(qw8),
        kw=perm_kq(kw8),
        vw=perm_cn(vw8),
        pw=perm_cn(pw8),
        f1w=f1wP,
        f2w8=f2wP8,
        f2wb=f2wPb,
        qbt=tbias(qkv_b_f[:C]),
        kbt=tbias(qkv_b_f[C : 2 * C]),
        qst=tbias(qsinv),
        kst=tbias(ksinv),
        f1bt=tbias(f1b_f),
        f1st=tbias(f1sinv),
        vg=vg,
        psinv=psinv.astype(np.float32),
        pb=pb_eff.astype(np.float32),
        f2sinv=f2sinv.astype(np.float32),
        f2b=fc2_b.astype(np.float32),
    )
    in_maps = []
    for core in range(8):
        b, half = core // 2, core % 2
        own = x[b, half * NOWN : (half + 1) * NOWN, :]
        other = x[b, (1 - half) * NOWN : (2 - half) * NOWN, :]
        xp = np.ascontiguousarray(np.concatenate([own, other], axis=0))
        in_maps.append({**common, "x": xp})
    return in_maps


def kernel(**inputs):
    in_maps = build_in_maps(inputs)
    nc = _get_program()
    res = run_bass_kernel_spmd(nc, in_maps, core_ids=list(range(8)))
    outs = res.results

    y = np.empty((4, NTOK, C), np.float32)
    for core in range(8):
        b, half = core // 2, core % 2
        y[b, half * NOWN : (half + 1) * NOWN, :] = outs[core]["out"]
    return y


if __name__ == "__main__":
    prog = build_program()
    print("program built OK")


# revision 58
# speedup vs baseline: 1.3782x; 1.0475x over previous
"""Trainium2 Bass kernel for a dense transformer block (pre-LN, MHA + MLP).

Full inputs in, full outputs out. Sharding: 8 cores = (batch, seq-half).
Each core computes K/V over its batch element's full 1024 tokens and
Q/attention/MLP over its own 512 tokens (host permutes tokens so the core's
own half is always rows 0..511 — softmax over keys is permutation invariant).
No collectives needed.

v2: fp8 (e4m3) DoubleRow matmuls for QKV / AV / proj (and optionally
fc1/fc2), which stream 2 contraction rows per PE pass. Weights are
quantized per-output-column on the host (absmax -> +-240); dequant scales
fold into the existing bias-add / activation ops, or (for V) into the proj
weights themselves. V is computed in natural [token, feat] layout directly
(h stationary, weights moving), eliminating the separate V transpose pass.
The softmax denominator comes from a constant ones-column appended to V.

Host-side preprocessing folds LayerNorm affine params into the following
matmul weights:  (xhat*g + b) @ W == xhat @ (diag(g) W) + b @ W, and the
V bias into the proj bias: (o/d + vb) @ pw + pb == (o/d) @ pw + (vb@pw + pb).
"""

import sys

sys.path.insert(0, "/opt/trn_rl_repo")

import numpy as np

import concourse.bass as bass
import concourse.bacc as bacc
import concourse.mybir as mybir
import concourse.tile as tile
from concourse.bass_utils import run_bass_kernel_spmd
from concourse.masks import make_identity

P = 128
C = 1024
HEADS = 16
DH = 64
HID = 4096
NTOK = 1024  # tokens per batch element (kv length)
NOWN = 512  # tokens owned by this core (q length)
SCALE = DH ** -0.5
EPS = 1e-5

F32 = mybir.dt.float32
F32R = mybir.dt.float32r
BF16 = mybir.dt.bfloat16
F8 = mybir.dt.float8e4
AF = mybir.ActivationFunctionType
OP = mybir.AluOpType
DROW = mybir.MatmulPerfMode.DoubleRow

CT = C // P  # 8 column tiles of the model dim
CP = CT // 2  # 4 column-tile pairs
TT = NTOK // P  # 8 token tiles (kv)
QT = NOWN // P  # 4 token tiles (own)
HT = HID // P  # 32 hidden tiles
HP = HT // 2  # 16 hidden-tile pairs

VPAD = 128  # per-head padded width of the V tile (DH + ones col + pad);
# padded to 128 so the AV matmul streams full-height (65-row outputs
# measured ~43% slower per instruction)

E4M3_MAX = 240.0

# --- dtype config for the two MLP GEMMs (attention GEMMs are always fp8;
# the error sim shows attention fp8 contributes ~nothing to final error) ---
FC1_FP8 = False
# fc2 split-K: hidden-tile pairs [0, F2SPLIT) run fp8 DoubleRow, the rest
# bf16. Error sim: F2SPLIT=12 -> 1.62e-2 total (gate 2e-2).
F2SPLIT = 12


def build_program():
    nc = bacc.Bacc("TRN2", target_bir_lowering=False)
    mf1 = F8 if FC1_FP8 else BF16

    io = {}
    io["x"] = nc.dram_tensor("x", (NTOK, C), F32, kind="ExternalInput")
    # pre-permuted weights (host layout matches SBUF slabs)
    io["qw"] = nc.dram_tensor("qw", (P, CT, CT, P), F8, kind="ExternalInput")
    io["kw"] = nc.dram_tensor("kw", (P, CT, CT, P), F8, kind="ExternalInput")
    io["vw"] = nc.dram_tensor("vw", (P, CT, C), F8, kind="ExternalInput")
    io["pw"] = nc.dram_tensor("pw", (P, CT, C), F8, kind="ExternalInput")
    io["f1w"] = nc.dram_tensor("f1w", (P, HT, CT, P), mf1, kind="ExternalInput")
    io["f2w8"] = nc.dram_tensor(
        "f2w8", (P, 2, 2 * F2SPLIT, NOWN), F8, kind="ExternalInput"
    )
    io["f2wb"] = nc.dram_tensor(
        "f2wb", (P, 2, HT - 2 * F2SPLIT, NOWN), BF16, kind="ExternalInput"
    )
    # per-partition bias/scale tables, [128, n] layouts
    io["qbt"] = nc.dram_tensor("qbt", (P, CT), F32, kind="ExternalInput")
    io["kbt"] = nc.dram_tensor("kbt", (P, CT), F32, kind="ExternalInput")
    io["qst"] = nc.dram_tensor("qst", (P, CT), F32, kind="ExternalInput")
    io["kst"] = nc.dram_tensor("kst", (P, CT), F32, kind="ExternalInput")
    io["f1bt"] = nc.dram_tensor("f1bt", (P, HT), F32, kind="ExternalInput")
    io["f1st"] = nc.dram_tensor("f1st", (P, HT), F32, kind="ExternalInput")
    # free-dim vectors (broadcast across partitions on chip)
    io["vg"] = nc.dram_tensor("vg", (C,), F32, kind="ExternalInput")
    io["psinv"] = nc.dram_tensor("psinv", (C,), F32, kind="ExternalInput")
    io["pb"] = nc.dram_tensor("pb", (C,), F32, kind="ExternalInput")
    io["f2sinv"] = nc.dram_tensor("f2sinv", (C,), F32, kind="ExternalInput")
    io["f2b"] = nc.dram_tensor("f2b", (C,), F32, kind="ExternalInput")
    io["out"] = nc.dram_tensor("out", (NOWN, C), F32, kind="ExternalOutput")

    with tile.TileContext(nc) as tc:
        _emit(nc, tc, io)
    nc.compile()
    return nc


def _emit(nc, tc, io):
    x_d, out_d = io["x"], io["out"]
    mf1 = F8 if FC1_FP8 else BF16

    with (
        tc.tile_pool(name="consts", bufs=1) as consts,
        tc.tile_pool(name="persist", bufs=1) as persist,
        tc.tile_pool(name="big", bufs=1) as big,
        tc.tile_pool(name="psum_wide", bufs=2, space="PSUM") as psum_wide,
    ):
        # ---- constants (unique tags: each gets its own persistent slot) ----
        ident_f32 = consts.tile([P, P], F32, tag="idf")
        make_identity(nc, ident_f32)
        ident = consts.tile([P, P], BF16, tag="idr")
        nc.vector.tensor_copy(out=ident, in_=ident_f32)
        eps_tile = consts.tile([P, 1], F32, tag="eps")
        nc.vector.memset(eps_tile, EPS)
        qbT = consts.tile([P, CT], F32, tag="qbT")
        nc.sync.dma_start(qbT, io["qbt"][:, :])
        kbT = consts.tile([P, CT], F32, tag="kbT")
        nc.sync.dma_start(kbT, io["kbt"][:, :])
        qsT = consts.tile([P, CT], F32, tag="qsT")
        nc.sync.dma_start(qsT, io["qst"][:, :])
        ksT = consts.tile([P, CT], F32, tag="ksT")
        nc.sync.dma_start(ksT, io["kst"][:, :])
        f1bT = consts.tile([P, HT], F32, tag="f1bT")
        nc.sync.dma_start(f1bT, io["f1bt"][:, :])
        f1sT = consts.tile([P, HT], F32, tag="f1sT")
        nc.sync.dma_start(f1sT, io["f1st"][:, :])

        def bcast_const(src_d, n, tag):
            t = consts.tile([P, n], F32, tag=tag, name=tag)
            src = bass.AP(tensor=src_d, offset=0, ap=[[0, P], [1, n]])
            nc.sync.dma_start(t, src)
            return t

        vg_bc = bcast_const(io["vg"], C, "vg")
        psinv_bc = bcast_const(io["psinv"], C, "psv")
        pb_bc = bcast_const(io["pb"], C, "pbb")
        f2sinv_bc = bcast_const(io["f2sinv"], C, "f2s")
        f2b_bc = bcast_const(io["f2b"], C, "f2bb")

        # own x tiles (fp32, kept for the residual), one tile per token tile;
        # proj writes x2 = x + pb + proj_out back IN PLACE (saves SBUF)
        x_own = []
        for t in range(QT):
            xo = persist.tile([P, C], F32, tag=f"xo{t}", name=f"xo{t}")
            nc.sync.dma_start(xo, x_d[t * P : (t + 1) * P, :])
            x_own.append(xo)
        x2 = x_own

        # persistent weight slabs (single DMA each, reused across sweeps)
        kwslab = persist.tile([P, CT, CT, P], F8, tag="kws", name="kws")
        nc.sync.dma_start(kwslab, io["kw"][:, :, :, :])
        qwslab = persist.tile([P, CT, CT, P], F8, tag="qws", name="qws")
        nc.sync.dma_start(qwslab, io["qw"][:, :, :, :])
        vwslab = persist.tile([P, CT, C], F8, tag="vws", name="vws")
        nc.sync.dma_start(vwslab, io["vw"][:, :, :])
        pslab = persist.tile([P, CT, C], F8, tag="pws", name="pws")
        nc.sync.dma_start(pslab, io["pw"][:, :, :])

        def layernorm_tile(temps, xt):
            """xt: [128, C] fp32 -> returns normalized f32r tile [128, C]."""
            stats = temps.tile([P, 2, 6], F32, tag="ln_stats", name="st")
            for sg in range(2):
                nc.vector.bn_stats(
                    out=stats[:, sg, :], in_=xt[:, sg * 512 : (sg + 1) * 512]
                )
            mv = temps.tile([P, 2], F32, tag="ln_mv", name="mv")
            nc.vector.bn_aggr(out=mv[:], in_=stats[:])
            # (ln/exp-based rsqrt thrashes the Act tables against the
            # attention exp — 1283ns per reload; Sqrt + [P,1] DVE reciprocal
            # is cheap, the reciprocal runs 1 elem/lane on 128 lanes)
            rstd = temps.tile([P, 1], F32, tag="ln_rstd", name="rstd")
            nc.scalar.activation(
                out=rstd, in_=mv[:, 1:2], func=AF.Sqrt, bias=eps_tile, scale=1.0
            )
            # approx reciprocal is ~5x faster; SBUF input (incl. in-place) is
            # exact to ~3e-6 on HW — only PSUM inputs misread (measured)
            nc.vector.reciprocal_approx_fast(out=rstd, in_=rstd)
            nmr = temps.tile([P, 1], F32, tag="ln_nmr", name="nmr")
            nc.vector.tensor_tensor(nmr, mv[:, 0:1], rstd, OP.mult)
            nc.vector.tensor_scalar_mul(nmr, nmr, -1.0)
            # bf16 h: the PE transposes stream 1.0 c/row for bf16 vs 1.5 for
            # f32r, and the transpose PSUM halves to one bank
            h = temps.tile([P, C], BF16, tag="ln_h", name="h")
            nc.any.tensor_scalar(
                out=h,
                in0=xt,
                scalar1=rstd,
                scalar2=nmr,
                op0=OP.mult,
                op1=OP.add,
            )
            return h

        # ---- persistent activation tiles ----
        # hT2[(cp, t2)]: [P, 2, 512] fp8 — transposed LN1 output, c-tile pairs
        hT2 = {
            (cp, t2): big.tile(
                [P, 2, NOWN], F8, tag=f"hT{cp}_{t2}", name=f"hT{cp}_{t2}"
            )
            for cp in range(CP)
            for t2 in range(2)
        }
        # kT[(ft, t2)]: [P, 512] bf16 (QK stays bf16)
        kT = {
            (ft, t2): big.tile(
                [P, NOWN], BF16, tag=f"kT{ft}_{t2}", name=f"kT{ft}_{t2}"
            )
            for ft in range(CT)
            for t2 in range(2)
        }
        # qT: one zero-padded tile per head (own head's 64 rows at its
        # natural partition offset, other head's rows zero). QK can then run
        # full-height 128-contraction matmuls — measured ~35% faster than
        # the 64-row form — with kT packed as-is.
        qT = [
            big.tile([P, NOWN], BF16, tag=f"qT{hq}", name=f"qT{hq}")
            for hq in range(HEADS)
        ]
        for hq in range(HEADS):
            nc.vector.memset(qT[hq][:], 0.0)
        # vh[q]: [P, 2, HEADS, VPAD] fp8 — V in natural token layout, one
        # tile per k-token-tile PAIR (matches the AV DoubleRow pair slice, so
        # each AV matmul depends on just its quarter of V, not all of it);
        # col DH holds 1.0 (softmax denominator trick)
        vh = [
            big.tile([P, 2, HEADS, VPAD], F8, tag=f"vh{q}", name=f"vh{q}")
            for q in range(QT)
        ]
        # oT2[fp]: [P, 2, 512] fp8 — attention output, feature-tile pairs
        oT2 = [
            big.tile([P, 2, NOWN], F8, tag=f"oT{fp}", name=f"oT{fp}")
            for fp in range(CP)
        ]
        h2T2 = [
            big.tile([P, 2, NOWN], mf1, tag=f"h2T{cp}", name=f"h2T{cp}")
            for cp in range(CP)
        ]
        actT2 = [
            big.tile(
                [P, 2, NOWN],
                F8 if hp < F2SPLIT else BF16,
                tag=f"aT{hp}",
                name=f"aT{hp}",
            )
            for hp in range(HP)
        ]

        for q in range(QT):
            nc.vector.memset(vh[q][:], 0.0)
            nc.vector.memset(vh[q][:, :, :, DH : DH + 1], 1.0)

        # ================= Phase 1: LN1 -> hT2 =================
        with (
            tc.tile_pool(name="ln1", bufs=2) as ln1,
            tc.tile_pool(name="xtmp", bufs=2) as xtmp,
        ):
            for t in range(TT):
                if t < QT:
                    xt = x_own[t]
                else:
                    xt = xtmp.tile([P, C], F32, tag="xt", name="xt")
                    nc.sync.dma_start(xt, x_d[t * P : (t + 1) * P, :])
                h = layernorm_tile(ln1, xt)
                t2, tb = t // QT, t % QT
                ps = psum_wide.tile([P, C], BF16, tag="tr", name=f"trp{t}")
                for ft in range(CT):
                    nc.tensor.transpose(
                        ps[:, ft * P : (ft + 1) * P],
                        h[:, ft * P : (ft + 1) * P],
                        ident,
                    )
                for cp in range(CP):
                    nc.any.tensor_copy(
                        out=hT2[(cp, t2)][:, :, tb * P : (tb + 1) * P],
                        in_=ps[:, cp * 2 * P : (cp + 1) * 2 * P].rearrange(
                            "p (two f) -> p two f", two=2
                        ),
                    )

        # ================= Phase 2: QKV =================
        HB = HEADS // 2  # heads per 512-wide V block
        with tc.tile_pool(name="qkv_psum", bufs=2, space="PSUM") as qkv_psum:

            def kq_sweep(t2, wslab, bT, sT, dst, per_head=False):
                """K or Q: transposed-output sweep; dst[ft] <- [P,512] bf16"""
                for ft in range(CT):
                    ps = qkv_psum.tile([P, NOWN], F32, tag="kvps", name="kvps")
                    for cp in range(CP):
                        nc.tensor.matmul(
                            ps,
                            lhsT=wslab[:, ft, 2 * cp : 2 * cp + 2, :],
                            rhs=hT2[(cp, t2)],
                            start=(cp == 0),
                            stop=(cp == CP - 1),
                            perf_mode=DROW,
                        )
                    if per_head:
                        # split into the two heads' zero-padded tiles,
                        # partition-aligned (head j keeps rows j*64..)
                        for j in range(2):
                            rows = slice(j * DH, (j + 1) * DH)
                            nc.any.tensor_scalar(
                                out=dst[2 * ft + j][rows, :],
                                in0=ps[rows, :],
                                scalar1=sT[rows, ft : ft + 1],
                                scalar2=bT[rows, ft : ft + 1],
                                op0=OP.mult,
                                op1=OP.add,
                            )
                    else:
                        nc.any.tensor_scalar(
                            out=dst[ft],
                            in0=ps,
                            scalar1=sT[:, ft : ft + 1],
                            scalar2=bT[:, ft : ft + 1],
                            op0=OP.mult,
                            op1=OP.add,
                        )

            def v_sweep(t2):
                """V in natural layout: h stationary, vw moving."""
                for tb in range(QT):
                    ps = qkv_psum.tile(
                        [P, HEADS, DH], F32, tag="vps", name=f"vps{t2}_{tb}"
                    )
                    for cp in range(CP):
                        for blk in range(2):
                            nc.tensor.matmul(
                                ps[:, blk * HB : (blk + 1) * HB, :],
                                lhsT=hT2[(cp, t2)][
                                    :, :, tb * P : (tb + 1) * P
                                ],
                                rhs=vwslab[
                                    :,
                                    2 * cp : 2 * cp + 2,
                                    blk * 512 : (blk + 1) * 512,
                                ],
                                start=(cp == 0),
                                stop=(cp == CP - 1),
                                perf_mode=DROW,
                            )
                    nc.any.tensor_tensor(
                        vh[t2 * 2 + tb // 2][:, tb % 2, :, :DH],
                        ps,
                        vg_bc[:, :].rearrange("p (h d) -> p h d", h=HEADS),
                        OP.mult,
                    )

            kq_sweep(0, kwslab, kbT, ksT, [kT[(f, 0)] for f in range(CT)])
            v_sweep(0)
            kq_sweep(0, qwslab, qbT, qsT, qT, per_head=True)
            kq_sweep(1, kwslab, kbT, ksT, [kT[(f, 1)] for f in range(CT)])
            v_sweep(1)

        # ================= Phase 3: attention =================
        with (
            tc.tile_pool(name="attn", bufs=3) as attn_pool,
            tc.tile_pool(name="attn_st", bufs=2, space="PSUM") as attn_st,
            tc.tile_pool(name="attn_ot", bufs=2, space="PSUM") as attn_ot,
        ):
            for h in range(HEADS):
                prow = (h % 2) * DH
                ftile = h // 2
                p_sb = attn_pool.tile([P, TT, NOWN], F8, tag="p_sb", name="p")
                for cp in range(CP):
                    st = attn_st.tile(
                        [P, 2, NOWN], F32, tag="st", name=f"st{h}_{cp}"
                    )
                    for j in range(2):
                        c = cp * 2 + j
                        # full-height lhsT: the other head's q rows are zero
                        kv_slice = kT[(ftile, c // QT)][
                            :, (c % QT) * P : (c % QT + 1) * P
                        ]
                        nc.tensor.matmul(
                            st[:, j, :],
                            lhsT=kv_slice,
                            rhs=qT[h],
                            start=True,
                            stop=True,
                        )
                    # p = exp(SCALE * s)   (fp8 out)
                    nc.scalar.activation(
                        out=p_sb[:, 2 * cp : 2 * cp + 2, :],
                        in_=st,
                        func=AF.Exp,
                        scale=SCALE,
                    )
                ot = attn_ot.tile([P, NOWN], F32, tag="ot", name="ot")
                for cp in range(CP):
                    nc.tensor.matmul(
                        ot,
                        lhsT=vh[cp][:, :, h, :],
                        rhs=p_sb[:, 2 * cp : 2 * cp + 2, :],
                        start=(cp == 0),
                        stop=(cp == CP - 1),
                        perf_mode=DROW,
                    )
                # softmax denominators arrive in row DH (ones column of vh).
                # Stage PSUM->SBUF first: reciprocal_approx_fast misreads
                # PSUM operands on HW (measured 0.38 rel err) but is exact
                # from SBUF, and ~5x faster than the exact DVE reciprocal.
                den = attn_pool.tile([1, NOWN], F32, tag="den", name="den")
                nc.any.tensor_copy(out=den, in_=ot[DH : DH + 1, :])
                rs = attn_pool.tile([1, NOWN], F32, tag="rs", name="rs")
                nc.vector.reciprocal_approx_fast(out=rs, in_=den)
                rsb = attn_pool.tile([DH, NOWN], F32, tag="rsb", name="rsb")
                nc.gpsimd.partition_broadcast(rsb, rs)
                nc.any.tensor_tensor(
                    oT2[ftile // 2][prow : prow + DH, ftile % 2, :],
                    ot[:DH, :],
                    rsb,
                    OP.mult,
                )

        # ================= Phase 4: proj + residual -> x2 (in place) ========
        # fold pb into x_own first (x_own already consumed by LN1; tile deps
        # order this correctly)
        for tq in range(QT):
            nc.any.tensor_tensor(x_own[tq], x_own[tq], pb_bc, OP.add)
        # Two waves of 4 psum chains, matmuls emitted round-robin across
        # chains fp-major: the PE stream is in-order, so emitting one chain's
        # fp0..fp3 consecutively would let its fp3 (which needs the LAST
        # heads' oT2) block every later chain's independent fp0-2 matmuls.
        # tq-grouped waves also let LN2 of tq 0/1 overlap wave 2.
        with tc.tile_pool(name="proj_ps", bufs=1, space="PSUM") as proj_ps:
            for wave in range(2):
                chains = [
                    (tq, ns)
                    for tq in (2 * wave, 2 * wave + 1)
                    for ns in range(2)
                ]
                pss = {
                    c: proj_ps.tile(
                        [P, 512], F32, tag=f"pps{i}", name=f"pps{c[0]}_{c[1]}"
                    )
                    for i, c in enumerate(chains)
                }
                for fp in range(CP):
                    for tq, ns in chains:
                        nsl = slice(ns * 512, (ns + 1) * 512)
                        nc.tensor.matmul(
                            pss[(tq, ns)],
                            lhsT=oT2[fp][:, :, tq * P : (tq + 1) * P],
                            rhs=pslab[:, 2 * fp : 2 * fp + 2, nsl],
                            start=(fp == 0),
                            stop=(fp == CP - 1),
                            perf_mode=DROW,
                        )
                for tq, ns in chains:
                    nsl = slice(ns * 512, (ns + 1) * 512)
                    ps = pss[(tq, ns)]
                    nc.any.tensor_tensor(ps, ps, psinv_bc[:, nsl], OP.mult)
                    nc.any.tensor_tensor(
                        x2[tq][:, nsl], ps, x_own[tq][:, nsl], OP.add
                    )

        # ================= Phases 5-7 share outer pools so the fc1/fc2
        # weight DMAs can issue during proj/LN2 (a pool's open barrier holds
        # its DMAs until the previous phase's pools close)
        NG = 4  # hidden-tile groups per DMA chunk
        N8G = 2 * F2SPLIT // NG  # fp8 groups
        NBG = (HT - 2 * F2SPLIT) // NG  # bf16 groups
        phase567 = (
            tc.tile_pool(name="f2c", bufs=3),
            tc.tile_pool(name="f2_ps", bufs=1, space="PSUM"),
            tc.tile_pool(name="out_sb", bufs=2),
            tc.tile_pool(name="f1c", bufs=5),
            tc.tile_pool(name="f1_ps", bufs=2, space="PSUM"),
        )
        with (
            phase567[0] as f2c,
            phase567[1] as f2_ps,
            phase567[2] as out_pool,
            phase567[3] as f1c,
            phase567[4] as f1_ps,
        ):
          # ================= Phase 5: LN2 -> h2T2 =================
          with tc.tile_pool(name="ln2", bufs=2) as ln2:
            for t in range(QT):
                h = layernorm_tile(ln2, x2[t])
                ps = psum_wide.tile([P, C], BF16, tag="tr", name=f"tr2{t}")
                for ft in range(CT):
                    nc.tensor.transpose(
                        ps[:, ft * P : (ft + 1) * P],
                        h[:, ft * P : (ft + 1) * P],
                        ident,
                    )
                for cp in range(CP):
                    nc.any.tensor_copy(
                        out=h2T2[cp][:, :, t * P : (t + 1) * P],
                        in_=ps[:, cp * 2 * P : (cp + 1) * 2 * P].rearrange(
                            "p (two f) -> p two f", two=2
                        ),
                    )

          # fold the fc2 bias into the residual once LN2 has consumed x2
          # (shortens the fc2 output chain to mult+add)
          for tq in range(QT):
            nc.any.tensor_tensor(x2[tq], x2[tq], f2b_bc, OP.add)

          # ================= Phase 6: FC1 + gelu -> actT2 =================
          if True:
            for g in range(HT // 2):
                slab = f1c.tile([P, 2, CT, P], mf1, tag="f1w", name="f1slab")
                nc.sync.dma_start(slab, io["f1w"][:, 2 * g : 2 * g + 2])
                for j in range(2):
                    hf = 2 * g + j
                    ps = f1_ps.tile([P, NOWN], F32, tag="f1ps", name="f1ps")
                    if FC1_FP8:
                        for cp in range(CP):
                            nc.tensor.matmul(
                                ps,
                                lhsT=slab[:, j, 2 * cp : 2 * cp + 2, :],
                                rhs=h2T2[cp],
                                start=(cp == 0),
                                stop=(cp == CP - 1),
                                perf_mode=DROW,
                            )
                    else:
                        for c in range(CT):
                            nc.tensor.matmul(
                                ps,
                                lhsT=slab[:, j, c, :],
                                rhs=h2T2[c // 2][:, c % 2, :],
                                start=(c == 0),
                                stop=(c == CT - 1),
                            )
                    # gelu(ps * s + b), fused dequant+bias via activation
                    nc.scalar.activation(
                        out=actT2[hf // 2][:, hf % 2, :],
                        in_=ps,
                        func=AF.Gelu,
                        bias=f1bT[:, hf : hf + 1],
                        scale=f1sT[:, hf : hf + 1],
                    )

          # ================= Phase 7: FC2 + residual -> out ===============
          # split-K: hidden tiles [0, 2*F2SPLIT) in fp8 DoubleRow, rest bf16
          if True:
            for ns in range(2):
                nsl = slice(ns * 512, (ns + 1) * 512)
                pss = [
                    f2_ps.tile([P, 512], F32, tag=f"f2ps{tq}", name=f"f2ps{tq}")
                    for tq in range(QT)
                ]
                for g in range(N8G):
                    gw = f2c.tile([P, NG, 512], F8, tag="f2w8", name=f"f2w8g{g}")
                    nc.sync.dma_start(
                        gw, io["f2w8"][:, ns, g * NG : (g + 1) * NG, :]
                    )
                    for tq in range(QT):
                        for i in range(NG // 2):
                            hp = (g * NG) // 2 + i
                            nc.tensor.matmul(
                                pss[tq],
                                lhsT=actT2[hp][:, :, tq * P : (tq + 1) * P],
                                rhs=gw[:, 2 * i : 2 * i + 2, :],
                                start=(g == 0 and i == 0),
                                stop=False,
                                perf_mode=DROW,
                            )
                for g in range(NBG):
                    gw = f2c.tile(
                        [P, NG, 512], BF16, tag="f2wb", name=f"f2wbg{g}"
                    )
                    nc.sync.dma_start(
                        gw, io["f2wb"][:, ns, g * NG : (g + 1) * NG, :]
                    )
                    for tq in range(QT):
                        for i in range(NG):
                            hc = 2 * F2SPLIT + g * NG + i
                            nc.tensor.matmul(
                                pss[tq],
                                lhsT=actT2[hc // 2][
                                    :, hc % 2, tq * P : (tq + 1) * P
                                ],
                                rhs=gw[:, i, :],
                                start=False,
                                stop=(g == NBG - 1 and i == NG - 1),
                            )
                for tq in range(QT):
                    ot2 = out_pool.tile([P, 512], F32, tag="out_t", name="o")
                    nc.any.tensor_tensor(
                        ot2, pss[tq], f2sinv_bc[:, nsl], OP.mult
                    )
                    nc.any.tensor_tensor(ot2, ot2, x2[tq][:, nsl], OP.add)
                    nc.sync.dma_start(out_d[tq * P : (tq + 1) * P, nsl], ot2)


_PROGRAM = None


def _get_program():
    global _PROGRAM
    if _PROGRAM is None:
        _PROGRAM = build_program()
    return _PROGRAM


def _quant_cols(w, dtype):
    """per-output-column absmax quantization; returns (w_q, dequant_scales)"""
    import ml_dtypes

    w = np.asarray(w, np.float64)
    if dtype == "fp8":
        amax = np.abs(w).max(axis=0)
        amax = np.where(amax == 0, 1.0, amax)
        s = E4M3_MAX / amax
        wq = np.clip(w * s, -E4M3_MAX, E4M3_MAX).astype(ml_dtypes.float8_e4m3)
        return wq, (1.0 / s).astype(np.float32)
    else:
        wq = w.astype(ml_dtypes.bfloat16)
        return wq, np.ones(w.shape[1], np.float32)


def build_in_maps(inputs):
    import ml_dtypes

    x = np.asarray(inputs["x"], np.float32)  # [4, 1024, 1024]
    ln1_g = np.asarray(inputs["ln1_g"], np.float64)
    ln1_b = np.asarray(inputs["ln1_b"], np.float64)
    ln2_g = np.asarray(inputs["ln2_g"], np.float64)
    ln2_b = np.asarray(inputs["ln2_b"], np.float64)
    qkv_w = np.asarray(inputs["qkv_w"], np.float64)
    qkv_b = np.asarray(inputs["qkv_b"], np.float64)
    proj_w = np.asarray(inputs["proj_w"], np.float64)
    proj_b = np.asarray(inputs["proj_b"], np.float64)
    fc1_w = np.asarray(inputs["fc1_w"], np.float64)
    fc1_b = np.asarray(inputs["fc1_b"], np.float64)
    fc2_w = np.asarray(inputs["fc2_w"], np.float64)
    fc2_b = np.asarray(inputs["fc2_b"], np.float64)

    # Fold LN affine into the following matmul:
    #   (xhat*g + b) @ W == xhat @ (diag(g) W) + b @ W
    qkv_w_f = ln1_g[:, None] * qkv_w
    qkv_b_f = qkv_b + ln1_b @ qkv_w
    f1w_f = ln2_g[:, None] * fc1_w
    f1b_f = fc1_b + ln2_b @ fc1_w

    qw = qkv_w_f[:, :C]
    kw = qkv_w_f[:, C : 2 * C]
    vw = qkv_w_f[:, 2 * C :]
    vb = qkv_b_f[2 * C :]

    # --- Q/K: per-column fp8 quant, dequant scale applied on chip ---
    qw8, qsinv = _quant_cols(qw, "fp8")
    kw8, ksinv = _quant_cols(kw, "fp8")

    # --- V: per-column fp8 quant; on-chip the psum is rescaled by vg so the
    # fp8 V tile holds v*t with t = 24/||vw_col||; t and the v bias both fold
    # into the proj weights/bias ---
    vw8, vsinv = _quant_cols(vw, "fp8")
    vnorm = np.linalg.norm(vw, axis=0)
    vnorm = np.where(vnorm == 0, 1.0, vnorm)
    t_v = 24.0 / vnorm
    vg = (vsinv * t_v).astype(np.float32)  # psum -> fp8 V scaling

    # --- proj: fold t_v and v bias; per-column fp8 quant ---
    pw_eff = proj_w / t_v[:, None]
    pb_eff = proj_b + vb @ proj_w
    pw8, psinv = _quant_cols(pw_eff, "fp8")

    # --- fc1 ---
    f1w8, f1sinv = _quant_cols(f1w_f, "fp8" if FC1_FP8 else "bf16")

    # --- fc2 split-K: rows [0, 256*F2SPLIT) fp8 (col-scaled), rest bf16
    # pre-scaled by the same column scales so one dequant applies to both ---
    k8 = 2 * F2SPLIT * P
    amax = np.abs(fc2_w[:k8]).max(axis=0)
    amax = np.where(amax == 0, 1.0, amax)
    s2 = E4M3_MAX / amax
    f2hi = np.clip(fc2_w[:k8] * s2, -E4M3_MAX, E4M3_MAX).astype(
        ml_dtypes.float8_e4m3
    )
    f2lo = (fc2_w[k8:] * s2).astype(ml_dtypes.bfloat16)
    f2sinv = (1.0 / s2).astype(np.float32)

    # --- permute weights into SBUF slab layouts ---
    # q/k: [p, ft, c, f] from w[c*128+p, ft*128+f]
    def perm_kq(w8):
        return np.ascontiguousarray(
            w8.reshape(CT, P, CT, P).transpose(1, 2, 0, 3)
        )

    # v/proj: [p, c, n] from w[c*128+p, n]
    def perm_cn(w8):
        return np.ascontiguousarray(w8.reshape(CT, P, C).transpose(1, 0, 2))

    # fc1: [p, hf, c, f] from w[c*128+p, hf*128+f]
    f1wP = np.ascontiguousarray(
        f1w8.reshape(CT, P, HT, P).transpose(1, 2, 0, 3)
    )
    # fc2: [p, ns, hc, n] from w[hc*128+p, ns*512+n]
    f2wP8 = np.ascontiguousarray(
        f2hi.reshape(2 * F2SPLIT, P, 2, NOWN).transpose(1, 2, 0, 3)
    )
    f2wPb = np.ascontiguousarray(
        f2lo.reshape(HT - 2 * F2SPLIT, P, 2, NOWN).transpose(1, 2, 0, 3)
    )

    def tbias(b):  # [n*128] -> [128, n] per-partition layout
        return np.ascontiguousarray(
            np.asarray(b, np.float32).reshape(-1, P).T
        )

    common = dict(
        qw=perm_kq(qw8),
        kw=perm_kq(kw8),
        vw=perm_cn(vw8),
        pw=perm_cn(pw8),
        f1w=f1wP,
        f2w8=f2wP8,
        f2wb=f2wPb,
        qbt=tbias(qkv_b_f[:C]),
        kbt=tbias(qkv_b_f[C : 2 * C]),
        qst=tbias(qsinv),
        kst=tbias(ksinv),
        f1bt=tbias(f1b_f),
        f1st=tbias(f1sinv),
        vg=vg,
        psinv=psinv.astype(np.float32),
        pb=pb_eff.astype(np.float32),
        f2sinv=f2sinv.astype(np.float32),
        f2b=fc2_b.astype(np.float32),
    )
    in_maps = []
    for core in range(8):
        b, half = core // 2, core % 2
        own = x[b, half * NOWN : (half + 1) * NOWN, :]
        other = x[b, (1 - half) * NOWN : (2 - half) * NOWN, :]
        xp = np.ascontiguousarray(np.concatenate([own, other], axis=0))
        in_maps.append({**common, "x": xp})
    return in_maps


def kernel(**inputs):
    in_maps = build_in_maps(inputs)
    nc = _get_program()
    res = run_bass_kernel_spmd(nc, in_maps, core_ids=list(range(8)))
    outs = res.results

    y = np.empty((4, NTOK, C), np.float32)
    for core in range(8):
        b, half = core // 2, core % 2
        y[b, half * NOWN : (half + 1) * NOWN, :] = outs[core]["out"]
    return y


if __name__ == "__main__":
    prog = build_program()
    print("program built OK")


# revision 59
# speedup vs baseline: 1.4036x; 1.0184x over previous
"""Trainium2 Bass kernel for a dense transformer block (pre-LN, MHA + MLP).

Full inputs in, full outputs out. Sharding: 8 cores = (batch, seq-half).
Each core computes K/V over its batch element's full 1024 tokens and
Q/attention/MLP over its own 512 tokens (host permutes tokens so the core's
own half is always rows 0..511 — softmax over keys is permutation invariant).
No collectives needed.

v2: fp8 (e4m3) DoubleRow matmuls for QKV / AV / proj (and optionally
fc1/fc2), which stream 2 contraction rows per PE pass. Weights are
quantized per-output-column on the host (absmax -> +-240); dequant scales
fold into the existing bias-add / activation ops, or (for V) into the proj
weights themselves. V is computed in natural [token, feat] layout directly
(h stationary, weights moving), eliminating the separate V transpose pass.
The softmax denominator comes from a constant ones-column appended to V.

Host-side preprocessing folds LayerNorm affine params into the following
matmul weights:  (xhat*g + b) @ W == xhat @ (diag(g) W) + b @ W, and the
V bias into the proj bias: (o/d + vb) @ pw + pb == (o/d) @ pw + (vb@pw + pb).
"""

import sys

sys.path.insert(0, "/opt/trn_rl_repo")

import numpy as np

import concourse.bass as bass
import concourse.bacc as bacc
import concourse.mybir as mybir
import concourse.tile as tile
from concourse.bass_utils import run_bass_kernel_spmd
from concourse.masks import make_identity

P = 128
C = 1024
HEADS = 16
DH = 64
HID = 4096
NTOK = 1024  # tokens per batch element (kv length)
NOWN = 512  # tokens owned by this core (q length)
SCALE = DH ** -0.5
EPS = 1e-5

F32 = mybir.dt.float32
F32R = mybir.dt.float32r
BF16 = mybir.dt.bfloat16
F8 = mybir.dt.float8e4
AF = mybir.ActivationFunctionType
OP = mybir.AluOpType
DROW = mybir.MatmulPerfMode.DoubleRow

CT = C // P  # 8 column tiles of the model dim
CP = CT // 2  # 4 column-tile pairs
TT = NTOK // P  # 8 token tiles (kv)
QT = NOWN // P  # 4 token tiles (own)
HT = HID // P  # 32 hidden tiles
HP = HT // 2  # 16 hidden-tile pairs

VPAD = 128  # per-head padded width of the V tile (DH + ones col + pad);
# padded to 128 so the AV matmul streams full-height (65-row outputs
# measured ~43% slower per instruction)

E4M3_MAX = 240.0

# --- dtype config for the two MLP GEMMs (attention GEMMs are always fp8;
# the error sim shows attention fp8 contributes ~nothing to final error) ---
FC1_FP8 = False
# fc2 split-K: hidden-tile pairs [0, F2SPLIT) run fp8 DoubleRow, the rest
# bf16. Error sim: F2SPLIT=12 -> 1.62e-2 total (gate 2e-2).
F2SPLIT = 12


def build_program():
    nc = bacc.Bacc("TRN2", target_bir_lowering=False)
    mf1 = F8 if FC1_FP8 else BF16

    io = {}
    io["x"] = nc.dram_tensor("x", (NTOK, C), F32, kind="ExternalInput")
    # pre-permuted weights (host layout matches SBUF slabs)
    io["qw"] = nc.dram_tensor("qw", (P, CT, CT, P), F8, kind="ExternalInput")
    io["kw"] = nc.dram_tensor("kw", (P, CT, CT, P), F8, kind="ExternalInput")
    io["vw"] = nc.dram_tensor("vw", (P, CT, C), F8, kind="ExternalInput")
    io["pw"] = nc.dram_tensor("pw", (P, CT, C), F8, kind="ExternalInput")
    io["f1w"] = nc.dram_tensor("f1w", (P, HT, CT, P), mf1, kind="ExternalInput")
    io["f2w8"] = nc.dram_tensor(
        "f2w8", (P, 2, 2 * F2SPLIT, NOWN), F8, kind="ExternalInput"
    )
    io["f2wb"] = nc.dram_tensor(
        "f2wb", (P, 2, HT - 2 * F2SPLIT, NOWN), BF16, kind="ExternalInput"
    )
    # per-partition bias/scale tables, [128, n] layouts
    io["qbt"] = nc.dram_tensor("qbt", (P, CT), F32, kind="ExternalInput")
    io["kbt"] = nc.dram_tensor("kbt", (P, CT), F32, kind="ExternalInput")
    io["qst"] = nc.dram_tensor("qst", (P, CT), F32, kind="ExternalInput")
    io["kst"] = nc.dram_tensor("kst", (P, CT), F32, kind="ExternalInput")
    io["f1bt"] = nc.dram_tensor("f1bt", (P, HT), F32, kind="ExternalInput")
    io["f1st"] = nc.dram_tensor("f1st", (P, HT), F32, kind="ExternalInput")
    # free-dim vectors (broadcast across partitions on chip)
    io["vg"] = nc.dram_tensor("vg", (C,), F32, kind="ExternalInput")
    io["psinv"] = nc.dram_tensor("psinv", (C,), F32, kind="ExternalInput")
    io["pb"] = nc.dram_tensor("pb", (C,), F32, kind="ExternalInput")
    io["f2sinv"] = nc.dram_tensor("f2sinv", (C,), F32, kind="ExternalInput")
    io["f2b"] = nc.dram_tensor("f2b", (C,), F32, kind="ExternalInput")
    io["out"] = nc.dram_tensor("out", (NOWN, C), F32, kind="ExternalOutput")

    with tile.TileContext(nc) as tc:
        _emit(nc, tc, io)
    nc.compile()
    return nc


def _emit(nc, tc, io):
    x_d, out_d = io["x"], io["out"]
    mf1 = F8 if FC1_FP8 else BF16

    with (
        tc.tile_pool(name="consts", bufs=1) as consts,
        tc.tile_pool(name="persist", bufs=1) as persist,
        tc.tile_pool(name="big", bufs=1) as big,
        tc.tile_pool(name="psum_wide", bufs=2, space="PSUM") as psum_wide,
    ):
        # ---- constants (unique tags: each gets its own persistent slot) ----
        ident_f32 = consts.tile([P, P], F32, tag="idf")
        make_identity(nc, ident_f32)
        ident = consts.tile([P, P], BF16, tag="idr")
        nc.vector.tensor_copy(out=ident, in_=ident_f32)
        eps_tile = consts.tile([P, 1], F32, tag="eps")
        nc.vector.memset(eps_tile, EPS)
        qbT = consts.tile([P, CT], F32, tag="qbT")
        nc.sync.dma_start(qbT, io["qbt"][:, :])
        kbT = consts.tile([P, CT], F32, tag="kbT")
        nc.sync.dma_start(kbT, io["kbt"][:, :])
        qsT = consts.tile([P, CT], F32, tag="qsT")
        nc.sync.dma_start(qsT, io["qst"][:, :])
        ksT = consts.tile([P, CT], F32, tag="ksT")
        nc.sync.dma_start(ksT, io["kst"][:, :])
        f1bT = consts.tile([P, HT], F32, tag="f1bT")
        nc.sync.dma_start(f1bT, io["f1bt"][:, :])
        f1sT = consts.tile([P, HT], F32, tag="f1sT")
        nc.sync.dma_start(f1sT, io["f1st"][:, :])

        def bcast_const(src_d, n, tag):
            t = consts.tile([P, n], F32, tag=tag, name=tag)
            src = bass.AP(tensor=src_d, offset=0, ap=[[0, P], [1, n]])
            nc.sync.dma_start(t, src)
            return t

        vg_bc = bcast_const(io["vg"], C, "vg")
        psinv_bc = bcast_const(io["psinv"], C, "psv")
        pb_bc = bcast_const(io["pb"], C, "pbb")
        f2sinv_bc = bcast_const(io["f2sinv"], C, "f2s")
        f2b_bc = bcast_const(io["f2b"], C, "f2bb")

        # own x tiles (fp32, kept for the residual), one tile per token tile;
        # proj writes x2 = x + pb + proj_out back IN PLACE (saves SBUF)
        x_own = []
        for t in range(QT):
            xo = persist.tile([P, C], F32, tag=f"xo{t}", name=f"xo{t}")
            nc.sync.dma_start(xo, x_d[t * P : (t + 1) * P, :])
            x_own.append(xo)
        x2 = x_own

        # persistent weight slabs (single DMA each, reused across sweeps)
        kwslab = persist.tile([P, CT, CT, P], F8, tag="kws", name="kws")
        nc.sync.dma_start(kwslab, io["kw"][:, :, :, :])
        qwslab = persist.tile([P, CT, CT, P], F8, tag="qws", name="qws")
        nc.sync.dma_start(qwslab, io["qw"][:, :, :, :])
        vwslab = persist.tile([P, CT, C], F8, tag="vws", name="vws")
        nc.sync.dma_start(vwslab, io["vw"][:, :, :])
        pslab = persist.tile([P, CT, C], F8, tag="pws", name="pws")
        nc.sync.dma_start(pslab, io["pw"][:, :, :])

        def layernorm_tile(temps, xt):
            """xt: [128, C] fp32 -> returns normalized f32r tile [128, C]."""
            stats = temps.tile([P, 2, 6], F32, tag="ln_stats", name="st")
            for sg in range(2):
                nc.vector.bn_stats(
                    out=stats[:, sg, :], in_=xt[:, sg * 512 : (sg + 1) * 512]
                )
            mv = temps.tile([P, 2], F32, tag="ln_mv", name="mv")
            nc.vector.bn_aggr(out=mv[:], in_=stats[:])
            # (ln/exp-based rsqrt thrashes the Act tables against the
            # attention exp — 1283ns per reload; Sqrt + [P,1] DVE reciprocal
            # is cheap, the reciprocal runs 1 elem/lane on 128 lanes)
            rstd = temps.tile([P, 1], F32, tag="ln_rstd", name="rstd")
            nc.scalar.activation(
                out=rstd, in_=mv[:, 1:2], func=AF.Sqrt, bias=eps_tile, scale=1.0
            )
            # approx reciprocal is ~5x faster; SBUF input (incl. in-place) is
            # exact to ~3e-6 on HW — only PSUM inputs misread (measured)
            nc.vector.reciprocal_approx_fast(out=rstd, in_=rstd)
            nmr = temps.tile([P, 1], F32, tag="ln_nmr", name="nmr")
            nc.vector.tensor_tensor(nmr, mv[:, 0:1], rstd, OP.mult)
            nc.vector.tensor_scalar_mul(nmr, nmr, -1.0)
            # bf16 h: the PE transposes stream 1.0 c/row for bf16 vs 1.5 for
            # f32r, and the transpose PSUM halves to one bank
            h = temps.tile([P, C], BF16, tag="ln_h", name="h")
            nc.any.tensor_scalar(
                out=h,
                in0=xt,
                scalar1=rstd,
                scalar2=nmr,
                op0=OP.mult,
                op1=OP.add,
            )
            return h

        # ---- persistent activation tiles ----
        # hT2[(cp, t2)]: [P, 2, 512] fp8 — transposed LN1 output, c-tile pairs
        hT2 = {
            (cp, t2): big.tile(
                [P, 2, NOWN], F8, tag=f"hT{cp}_{t2}", name=f"hT{cp}_{t2}"
            )
            for cp in range(CP)
            for t2 in range(2)
        }
        # kT[(ft, t2)]: [P, 512] bf16 (QK stays bf16)
        kT = {
            (ft, t2): big.tile(
                [P, NOWN], BF16, tag=f"kT{ft}_{t2}", name=f"kT{ft}_{t2}"
            )
            for ft in range(CT)
            for t2 in range(2)
        }
        # qT: one zero-padded tile per head (own head's 64 rows at its
        # natural partition offset, other head's rows zero). QK can then run
        # full-height 128-contraction matmuls — measured ~35% faster than
        # the 64-row form — with kT packed as-is.
        qT = [
            big.tile([P, NOWN], BF16, tag=f"qT{hq}", name=f"qT{hq}")
            for hq in range(HEADS)
        ]
        for hq in range(HEADS):
            nc.vector.memset(qT[hq][:], 0.0)
        # vh[q]: [P, 2, HEADS, VPAD] fp8 — V in natural token layout, one
        # tile per k-token-tile PAIR (matches the AV DoubleRow pair slice, so
        # each AV matmul depends on just its quarter of V, not all of it);
        # col DH holds 1.0 (softmax denominator trick)
        vh = [
            big.tile([P, 2, HEADS, VPAD], F8, tag=f"vh{q}", name=f"vh{q}")
            for q in range(QT)
        ]
        # oT2[fp]: [P, 2, 512] fp8 — attention output, feature-tile pairs
        oT2 = [
            big.tile([P, 2, NOWN], F8, tag=f"oT{fp}", name=f"oT{fp}")
            for fp in range(CP)
        ]
        h2T2 = [
            big.tile([P, 2, NOWN], mf1, tag=f"h2T{cp}", name=f"h2T{cp}")
            for cp in range(CP)
        ]
        actT2 = [
            big.tile(
                [P, 2, NOWN],
                F8 if hp < F2SPLIT else BF16,
                tag=f"aT{hp}",
                name=f"aT{hp}",
            )
            for hp in range(HP)
        ]

        for q in range(QT):
            nc.vector.memset(vh[q][:], 0.0)
            nc.vector.memset(vh[q][:, :, :, DH : DH + 1], 1.0)

        # ================= Phase 1: LN1 -> hT2 =================
        with (
            tc.tile_pool(name="ln1", bufs=2) as ln1,
            tc.tile_pool(name="xtmp", bufs=2) as xtmp,
        ):
            for t in range(TT):
                if t < QT:
                    xt = x_own[t]
                else:
                    xt = xtmp.tile([P, C], F32, tag="xt", name="xt")
                    nc.sync.dma_start(xt, x_d[t * P : (t + 1) * P, :])
                h = layernorm_tile(ln1, xt)
                t2, tb = t // QT, t % QT
                ps = psum_wide.tile([P, C], BF16, tag="tr", name=f"trp{t}")
                for ft in range(CT):
                    nc.tensor.transpose(
                        ps[:, ft * P : (ft + 1) * P],
                        h[:, ft * P : (ft + 1) * P],
                        ident,
                    )
                for cp in range(CP):
                    nc.any.tensor_copy(
                        out=hT2[(cp, t2)][:, :, tb * P : (tb + 1) * P],
                        in_=ps[:, cp * 2 * P : (cp + 1) * 2 * P].rearrange(
                            "p (two f) -> p two f", two=2
                        ),
                    )

        # ================= Phase 2: QKV =================
        HB = HEADS // 2  # heads per 512-wide V block
        with tc.tile_pool(name="qkv_psum", bufs=2, space="PSUM") as qkv_psum:

            def kq_sweep(t2, wslab, bT, sT, dst, per_head=False):
                """K or Q: transposed-output sweep; dst[ft] <- [P,512] bf16"""
                for ft in range(CT):
                    ps = qkv_psum.tile([P, NOWN], F32, tag="kvps", name="kvps")
                    for cp in range(CP):
                        nc.tensor.matmul(
                            ps,
                            lhsT=wslab[:, ft, 2 * cp : 2 * cp + 2, :],
                            rhs=hT2[(cp, t2)],
                            start=(cp == 0),
                            stop=(cp == CP - 1),
                            perf_mode=DROW,
                        )
                    if per_head:
                        # split into the two heads' zero-padded tiles,
                        # partition-aligned (head j keeps rows j*64..)
                        for j in range(2):
                            rows = slice(j * DH, (j + 1) * DH)
                            nc.any.tensor_scalar(
                                out=dst[2 * ft + j][rows, :],
                                in0=ps[rows, :],
                                scalar1=sT[rows, ft : ft + 1],
                                scalar2=bT[rows, ft : ft + 1],
                                op0=OP.mult,
                                op1=OP.add,
                            )
                    else:
                        nc.any.tensor_scalar(
                            out=dst[ft],
                            in0=ps,
                            scalar1=sT[:, ft : ft + 1],
                            scalar2=bT[:, ft : ft + 1],
                            op0=OP.mult,
                            op1=OP.add,
                        )

            def v_sweep(t2):
                """V in natural layout: h stationary, vw moving."""
                for tb in range(QT):
                    ps = qkv_psum.tile(
                        [P, HEADS, DH], F32, tag="vps", name=f"vps{t2}_{tb}"
                    )
                    for cp in range(CP):
                        for blk in range(2):
                            nc.tensor.matmul(
                                ps[:, blk * HB : (blk + 1) * HB, :],
                                lhsT=hT2[(cp, t2)][
                                    :, :, tb * P : (tb + 1) * P
                                ],
                                rhs=vwslab[
                                    :,
                                    2 * cp : 2 * cp + 2,
                                    blk * 512 : (blk + 1) * 512,
                                ],
                                start=(cp == 0),
                                stop=(cp == CP - 1),
                                perf_mode=DROW,
                            )
                    nc.any.tensor_tensor(
                        vh[t2 * 2 + tb // 2][:, tb % 2, :, :DH],
                        ps,
                        vg_bc[:, :].rearrange("p (h d) -> p h d", h=HEADS),
                        OP.mult,
                    )

            kq_sweep(0, kwslab, kbT, ksT, [kT[(f, 0)] for f in range(CT)])
            v_sweep(0)
            kq_sweep(0, qwslab, qbT, qsT, qT, per_head=True)
            kq_sweep(1, kwslab, kbT, ksT, [kT[(f, 1)] for f in range(CT)])
            v_sweep(1)

        # ================= Phase 3: attention =================
        with (
            tc.tile_pool(name="attn", bufs=3) as attn_pool,
            tc.tile_pool(name="attn_st", bufs=2, space="PSUM") as attn_st,
            tc.tile_pool(name="attn_ot", bufs=2, space="PSUM") as attn_ot,
        ):
            for h in range(HEADS):
                prow = (h % 2) * DH
                ftile = h // 2
                p_sb = attn_pool.tile([P, TT, NOWN], F8, tag="p_sb", name="p")
                for cp in range(CP):
                    st = attn_st.tile(
                        [P, 2, NOWN], F32, tag="st", name=f"st{h}_{cp}"
                    )
                    for j in range(2):
                        c = cp * 2 + j
                        # full-height lhsT: the other head's q rows are zero
                        kv_slice = kT[(ftile, c // QT)][
                            :, (c % QT) * P : (c % QT + 1) * P
                        ]
                        nc.tensor.matmul(
                            st[:, j, :],
                            lhsT=kv_slice,
                            rhs=qT[h],
                            start=True,
                            stop=True,
                        )
                    # p = exp(SCALE * s)   (fp8 out)
                    nc.scalar.activation(
                        out=p_sb[:, 2 * cp : 2 * cp + 2, :],
                        in_=st,
                        func=AF.Exp,
                        scale=SCALE,
                    )
                ot = attn_ot.tile([P, NOWN], F32, tag="ot", name="ot")
                for cp in range(CP):
                    nc.tensor.matmul(
                        ot,
                        lhsT=vh[cp][:, :, h, :],
                        rhs=p_sb[:, 2 * cp : 2 * cp + 2, :],
                        start=(cp == 0),
                        stop=(cp == CP - 1),
                        perf_mode=DROW,
                    )
                # softmax denominators arrive in row DH (ones column of vh).
                # Stage PSUM->SBUF first: reciprocal_approx_fast misreads
                # PSUM operands on HW (measured 0.38 rel err) but is exact
                # from SBUF, and ~5x faster than the exact DVE reciprocal.
                den = attn_pool.tile([1, NOWN], F32, tag="den", name="den")
                nc.any.tensor_copy(out=den, in_=ot[DH : DH + 1, :])
                rs = attn_pool.tile([1, NOWN], F32, tag="rs", name="rs")
                nc.vector.reciprocal_approx_fast(out=rs, in_=den)
                rsb = attn_pool.tile([DH, NOWN], F32, tag="rsb", name="rsb")
                nc.gpsimd.partition_broadcast(rsb, rs)
                nc.any.tensor_tensor(
                    oT2[ftile // 2][prow : prow + DH, ftile % 2, :],
                    ot[:DH, :],
                    rsb,
                    OP.mult,
                )

        # ================= Phase 4: proj + residual -> x2 (in place) ========
        # fold pb into x_own first (x_own already consumed by LN1; tile deps
        # order this correctly)
        for tq in range(QT):
            nc.any.tensor_tensor(x_own[tq], x_own[tq], pb_bc, OP.add)
        # Two waves of 4 psum chains, matmuls emitted round-robin across
        # chains fp-major: the PE stream is in-order, so emitting one chain's
        # fp0..fp3 consecutively would let its fp3 (which needs the LAST
        # heads' oT2) block every later chain's independent fp0-2 matmuls.
        # tq-grouped waves also let LN2 of tq 0/1 overlap wave 2.
        with tc.tile_pool(name="proj_ps", bufs=1, space="PSUM") as proj_ps:
            for tqs in ([0, 1], [2], [3]):
                chains = [(tq, ns) for tq in tqs for ns in range(2)]
                pss = {
                    c: proj_ps.tile(
                        [P, 512], F32, tag=f"pps{i}", name=f"pps{c[0]}_{c[1]}"
                    )
                    for i, c in enumerate(chains)
                }
                for fp in range(CP):
                    for tq, ns in chains:
                        nsl = slice(ns * 512, (ns + 1) * 512)
                        nc.tensor.matmul(
                            pss[(tq, ns)],
                            lhsT=oT2[fp][:, :, tq * P : (tq + 1) * P],
                            rhs=pslab[:, 2 * fp : 2 * fp + 2, nsl],
                            start=(fp == 0),
                            stop=(fp == CP - 1),
                            perf_mode=DROW,
                        )
                for tq, ns in chains:
                    nsl = slice(ns * 512, (ns + 1) * 512)
                    ps = pss[(tq, ns)]
                    nc.any.tensor_tensor(ps, ps, psinv_bc[:, nsl], OP.mult)
                    nc.any.tensor_tensor(
                        x2[tq][:, nsl], ps, x_own[tq][:, nsl], OP.add
                    )

        # ================= Phases 5-7 share outer pools so the fc1/fc2
        # weight DMAs can issue during proj/LN2 (a pool's open barrier holds
        # its DMAs until the previous phase's pools close)
        NG = 4  # hidden-tile groups per DMA chunk
        N8G = 2 * F2SPLIT // NG  # fp8 groups
        NBG = (HT - 2 * F2SPLIT) // NG  # bf16 groups
        with (
            tc.tile_pool(name="f2c", bufs=3) as f2c,
            tc.tile_pool(name="out_sb", bufs=2) as out_pool,
            tc.tile_pool(name="f1c", bufs=5) as f1c,
        ):
          # ================= Phase 5: LN2 -> h2T2 =================
          with tc.tile_pool(name="ln2", bufs=2) as ln2:
            for t in range(QT):
                h = layernorm_tile(ln2, x2[t])
                ps = psum_wide.tile([P, C], BF16, tag="tr", name=f"tr2{t}")
                for ft in range(CT):
                    nc.tensor.transpose(
                        ps[:, ft * P : (ft + 1) * P],
                        h[:, ft * P : (ft + 1) * P],
                        ident,
                    )
                for cp in range(CP):
                    nc.any.tensor_copy(
                        out=h2T2[cp][:, :, t * P : (t + 1) * P],
                        in_=ps[:, cp * 2 * P : (cp + 1) * 2 * P].rearrange(
                            "p (two f) -> p two f", two=2
                        ),
                    )

          # fold the fc2 bias into the residual once LN2 has consumed x2
          # (shortens the fc2 output chain to mult+add)
          for tq in range(QT):
            nc.any.tensor_tensor(x2[tq], x2[tq], f2b_bc, OP.add)

          # ================= Phase 6: FC1 + gelu -> actT2 =================
          with tc.tile_pool(name="f1_ps", bufs=2, space="PSUM") as f1_ps:
            for g in range(HT // 2):
                slab = f1c.tile([P, 2, CT, P], mf1, tag="f1w", name="f1slab")
                nc.sync.dma_start(slab, io["f1w"][:, 2 * g : 2 * g + 2])
                for j in range(2):
                    hf = 2 * g + j
                    ps = f1_ps.tile([P, NOWN], F32, tag="f1ps", name="f1ps")
                    if FC1_FP8:
                        for cp in range(CP):
                            nc.tensor.matmul(
                                ps,
                                lhsT=slab[:, j, 2 * cp : 2 * cp + 2, :],
                                rhs=h2T2[cp],
                                start=(cp == 0),
                                stop=(cp == CP - 1),
                                perf_mode=DROW,
                            )
                    else:
                        for c in range(CT):
                            nc.tensor.matmul(
                                ps,
                                lhsT=slab[:, j, c, :],
                                rhs=h2T2[c // 2][:, c % 2, :],
                                start=(c == 0),
                                stop=(c == CT - 1),
                            )
                    # gelu(ps * s + b), fused dequant+bias via activation
                    nc.scalar.activation(
                        out=actT2[hf // 2][:, hf % 2, :],
                        in_=ps,
                        func=AF.Gelu,
                        bias=f1bT[:, hf : hf + 1],
                        scale=f1sT[:, hf : hf + 1],
                    )

          # ================= Phase 7: FC2 + residual -> out ===============
          # split-K: hidden tiles [0, 2*F2SPLIT) in fp8 DoubleRow, rest bf16
          with tc.tile_pool(name="f2_ps", bufs=1, space="PSUM") as f2_ps:
            for ns in range(2):
                nsl = slice(ns * 512, (ns + 1) * 512)
                pss = [
                    f2_ps.tile(
                        [P, 512],
                        F32,
                        tag=f"f2ps{(ns * QT + tq) % 6}",
                        name=f"f2ps{ns}_{tq}",
                    )
                    for tq in range(QT)
                ]
                for g in range(N8G):
                    gw = f2c.tile([P, NG, 512], F8, tag="f2w8", name=f"f2w8g{g}")
                    nc.sync.dma_start(
                        gw, io["f2w8"][:, ns, g * NG : (g + 1) * NG, :]
                    )
                    for tq in range(QT):
                        for i in range(NG // 2):
                            hp = (g * NG) // 2 + i
                            nc.tensor.matmul(
                                pss[tq],
                                lhsT=actT2[hp][:, :, tq * P : (tq + 1) * P],
                                rhs=gw[:, 2 * i : 2 * i + 2, :],
                                start=(g == 0 and i == 0),
                                stop=False,
                                perf_mode=DROW,
                            )
                for g in range(NBG):
                    gw = f2c.tile(
                        [P, NG, 512], BF16, tag="f2wb", name=f"f2wbg{g}"
                    )
                    nc.sync.dma_start(
                        gw, io["f2wb"][:, ns, g * NG : (g + 1) * NG, :]
                    )
                    for tq in range(QT):
                        for i in range(NG):
                            hc = 2 * F2SPLIT + g * NG + i
                            nc.tensor.matmul(
                                pss[tq],
                                lhsT=actT2[hc // 2][
                                    :, hc % 2, tq * P : (tq + 1) * P
                                ],
                                rhs=gw[:, i, :],
                                start=False,
                                stop=(g == NBG - 1 and i == NG - 1),
                            )
                for tq in range(QT):
                    ot2 = out_pool.tile([P, 512], F32, tag="out_t", name="o")
                    nc.any.tensor_tensor(
                        ot2, pss[tq], f2sinv_bc[:, nsl], OP.mult
                    )
                    nc.any.tensor_tensor(ot2, ot2, x2[tq][:, nsl], OP.add)
                    nc.sync.dma_start(out_d[tq * P : (tq + 1) * P, nsl], ot2)


_PROGRAM = None


def _get_program():
    global _PROGRAM
    if _PROGRAM is None:
        _PROGRAM = build_program()
    return _PROGRAM


def _quant_cols(w, dtype):
    """per-output-column absmax quantization; returns (w_q, dequant_scales)"""
    import ml_dtypes

    w = np.asarray(w, np.float64)
    if dtype == "fp8":
        amax = np.abs(w).max(axis=0)
        amax = np.where(amax == 0, 1.0, amax)
        s = E4M3_MAX / amax
        wq = np.clip(w * s, -E4M3_MAX, E4M3_MAX).astype(ml_dtypes.float8_e4m3)
        return wq, (1.0 / s).astype(np.float32)
    else:
        wq = w.astype(ml_dtypes.bfloat16)
        return wq, np.ones(w.shape[1], np.float32)


def build_in_maps(inputs):
    import ml_dtypes

    x = np.asarray(inputs["x"], np.float32)  # [4, 1024, 1024]
    ln1_g = np.asarray(inputs["ln1_g"], np.float64)
    ln1_b = np.asarray(inputs["ln1_b"], np.float64)
    ln2_g = np.asarray(inputs["ln2_g"], np.float64)
    ln2_b = np.asarray(inputs["ln2_b"], np.float64)
    qkv_w = np.asarray(inputs["qkv_w"], np.float64)
    qkv_b = np.asarray(inputs["qkv_b"], np.float64)
    proj_w = np.asarray(inputs["proj_w"], np.float64)
    proj_b = np.asarray(inputs["proj_b"], np.float64)
    fc1_w = np.asarray(inputs["fc1_w"], np.float64)
    fc1_b = np.asarray(inputs["fc1_b"], np.float64)
    fc2_w = np.asarray(inputs["fc2_w"], np.float64)
    fc2_b = np.asarray(inputs["fc2_b"], np.float64)

    # Fold LN affine into the following matmul:
    #   (xhat*g + b) @ W == xhat @ (diag(g) W) + b @ W
    qkv_w_f = ln1_g[:, None] * qkv_w
    qkv_b_f = qkv_b + ln1_b @ qkv_w
    f1w_f = ln2_g[:, None] * fc1_w
    f1b_f = fc1_b + ln2_b @ fc1_w

    qw = qkv_w_f[:, :C]
    kw = qkv_w_f[:, C : 2 * C]
    vw = qkv_w_f[:, 2 * C :]
    vb = qkv_b_f[2 * C :]

    # --- Q/K: per-column fp8 quant, dequant scale applied on chip ---
    qw8, qsinv = _quant_cols(qw, "fp8")
    kw8, ksinv = _quant_cols(kw, "fp8")

    # --- V: per-column fp8 quant; on-chip the psum is rescaled by vg so the
    # fp8 V tile holds v*t with t = 24/||vw_col||; t and the v bias both fold
    # into the proj weights/bias ---
    vw8, vsinv = _quant_cols(vw, "fp8")
    vnorm = np.linalg.norm(vw, axis=0)
    vnorm = np.where(vnorm == 0, 1.0, vnorm)
    t_v = 24.0 / vnorm
    vg = (vsinv * t_v).astype(np.float32)  # psum -> fp8 V scaling

    # --- proj: fold t_v and v bias; per-column fp8 quant ---
    pw_eff = proj_w / t_v[:, None]
    pb_eff = proj_b + vb @ proj_w
    pw8, psinv = _quant_cols(pw_eff, "fp8")

    # --- fc1 ---
    f1w8, f1sinv = _quant_cols(f1w_f, "fp8" if FC1_FP8 else "bf16")

    # --- fc2 split-K: rows [0, 256*F2SPLIT) fp8 (col-scaled), rest bf16
    # pre-scaled by the same column scales so one dequant applies to both ---
    k8 = 2 * F2SPLIT * P
    amax = np.abs(fc2_w[:k8]).max(axis=0)
    amax = np.where(amax == 0, 1.0, amax)
    s2 = E4M3_MAX / amax
    f2hi = np.clip(fc2_w[:k8] * s2, -E4M3_MAX, E4M3_MAX).astype(
        ml_dtypes.float8_e4m3
    )
    f2lo = (fc2_w[k8:] * s2).astype(ml_dtypes.bfloat16)
    f2sinv = (1.0 / s2).astype(np.float32)

    # --- permute weights into SBUF slab layouts ---
    # q/k: [p, ft, c, f] from w[c*128+p, ft*128+f]
    def perm_kq(w8):
        return np.ascontiguousarray(
            w8.reshape(CT, P, CT, P).transpose(1, 2, 0, 3)
        )

    # v/proj: [p, c, n] from w[c*128+p, n]
    def perm_cn(w8):
        return np.ascontiguousarray(w8.reshape(CT, P, C).transpose(1, 0, 2))

    # fc1: [p, hf, c, f] from w[c*128+p, hf*128+f]
    f1wP = np.ascontiguousarray(
        f1w8.reshape(CT, P, HT, P).transpose(1, 2, 0, 3)
    )
    # fc2: [p, ns, hc, n] from w[hc*128+p, ns*512+n]
    f2wP8 = np.ascontiguousarray(
        f2hi.reshape(2 * F2SPLIT, P, 2, NOWN).transpose(1, 2, 0, 3)
    )
    f2wPb = np.ascontiguousarray(
        f2lo.reshape(HT - 2 * F2SPLIT, P, 2, NOWN).transpose(1, 2, 0, 3)
    )

    def tbias(b):  # [n*128] -> [128, n] per-partition layout
        return np.ascontiguousarray(
            np.asarray(b, np.float32).reshape(-1, P).T
        )

    common = dict(
        qw=perm_kq(qw8),
        kw=perm_kq(kw8),
        vw=perm_cn(vw8),
        pw=perm_cn(pw8),
        f1w=f1wP,
        f2w8=f2wP8,
        f2wb=f2wPb,
        qbt=tbias(qkv_b_f[:C]),
        kbt=tbias(qkv_b_f[C : 2 * C]),
        qst=tbias(qsinv),
        kst=tbias(ksinv),
        f1bt=tbias(f1b_f),
        f1st=tbias(f1sinv),
        vg=vg,
        psinv=psinv.astype(np.float32),
        pb=pb_eff.astype(np.float32),
        f2sinv=f2sinv.astype(np.float32),
        f2b=fc2_b.astype(np.float32),
    )
    in_maps = []
    for core in range(8):
        b, half = core // 2, core % 2
        own = x[b, half * NOWN : (half + 1) * NOWN, :]
        other = x[b, (1 - half) * NOWN : (2 - half) * NOWN, :]
        xp = np.ascontiguousarray(np.concatenate([own, other], axis=0))
        in_maps.append({**common, "x": xp})
    return in_maps


def kernel(**inputs):
    in_maps = build_in_maps(inputs)
    nc = _get_program()
    res = run_bass_kernel_spmd(nc, in_maps, core_ids=list(range(8)))
    outs = res.results

    y = np.empty((4, NTOK, C), np.float32)
    for core in range(8):
        b, half = core // 2, core % 2
        y[b, half * NOWN : (half + 1) * NOWN, :] = outs[core]["out"]
    return y


if __name__ == "__main__":
    prog = build_program()
    print("program built OK")
